# revision 1
# baseline (speedup 1.0000x reference)
"""Multi-head attention (RMSNorm-QK + RoPE + softmax + proj) on 8 Trainium2 cores.

Sharding: core c handles batch b = c//4 and heads [3*(c%4), 3*(c%4)+3).
Each core computes qkv for its heads, flash-style attention, and a partial
projection over its heads' channels; the host sums the 4 partials per batch.

Layout tricks (all fp32, matmuls in float32r at 1 cyc/row):
 - q^T/k^T layout [head_dim, tokens]; head-dim rows permuted so the RoPE
   half-swap is an intra-quadrant stream_shuffle.
 - RMS-norm: sum(q^2) via ones-pair matmul; rsqrt = exp(-0.5*ln(x)) so the
   whole kernel uses one ACT table set (natural_log_exp_and_others).
 - softmax without max-subtraction (logits bounded by RMS norm); denominators
   via an appended ones-column in the PV matmul; 1/denom on DVE.
 - qkv/proj biases via K=1 matmul rows.
"""
import sys

for _p in ("/opt/trn_rl_repo", "/opt/trn_rl_repo/concourse"):
    if _p not in sys.path:
        sys.path.insert(0, _p)

import numpy as np
from contextlib import ExitStack

import concourse.bass as bass
import concourse.tile as tile
import concourse.mybir as mybir
from concourse.bass_utils import run_bass_kernel_spmd

F32 = mybir.dt.float32
F32R = mybir.dt.float32r
AF = mybir.ActivationFunctionType

B, N, C = 2, 2048, 768
H, HD = 12, 64
HP = 3            # heads per core
NCORES = 8
CCH = C // 128    # 6 contraction chunks
NT = N // 512     # 4 token tiles of 512
KB = N // 128     # 16 k-blocks of 128
EPS = 1e-6

SWAP_MASK = [(i + 16) % 32 for i in range(32)]
# head-dim permutation: pair-exchange (d <-> d+32) becomes intra-quadrant
PERM = np.concatenate([np.arange(0, 16), np.arange(32, 48),
                       np.arange(16, 32), np.arange(48, 64)])
SIGN = np.where(PERM < 32, -1.0, 1.0).astype(np.float32)

_NC_CACHE = {}


def build_nc(split_waits=True):
    nc = bass.Bass(target_bir_lowering=True)
    xT = nc.declare_dram_parameter("xT", [C, N], F32R, isOutput=False)
    wqk = nc.declare_dram_parameter("wqk", [C, HP * 128], F32R, isOutput=False)
    wv = nc.declare_dram_parameter("wv", [C, 256], F32R, isOutput=False)
    bqk = nc.declare_dram_parameter("bqk", [1, HP * 128], F32R, isOutput=False)
    bv = nc.declare_dram_parameter("bv", [1, 256], F32R, isOutput=False)
    cos2w = nc.declare_dram_parameter("cos2w", [128, N], F32, isOutput=False)
    sinSw = nc.declare_dram_parameter("sinSw", [128, N], F32, isOutput=False)
    sel4 = nc.declare_dram_parameter("sel4", [128, 512], F32R, isOutput=False)
    wp = nc.declare_dram_parameter("wp", [HP * HD, C], F32R, isOutput=False)
    onesd = nc.declare_dram_parameter("onesd", [128, 512], F32R, isOutput=False)
    onespd = nc.declare_dram_parameter("onespd", [128, 2], F32R, isOutput=False)
    vones = nc.declare_dram_parameter("vones", [128, HP * KB], F32R, isOutput=False)
    out = nc.declare_dram_parameter("out", [N, C], F32, isOutput=True)

    with tile.TileContext(nc) as tc, ExitStack() as ctx:
        sb = ctx.enter_context(tc.tile_pool(name="sb", bufs=1))
        tp = ctx.enter_context(tc.tile_pool(name="tp", bufs=2))
        pe = ctx.enter_context(tc.tile_pool(name="pe", bufs=3))   # pexp
        tp1 = ctx.enter_context(tc.tile_pool(name="tp1", bufs=1))
        fps = ctx.enter_context(tc.tile_pool(name="fps", bufs=2, space="PSUM"))
        sA = ctx.enter_context(tc.tile_pool(name="sA", bufs=1, space="PSUM"))
        sB = ctx.enter_context(tc.tile_pool(name="sB", bufs=1, space="PSUM"))
        oA = ctx.enter_context(tc.tile_pool(name="oA", bufs=1, space="PSUM"))
        oB = ctx.enter_context(tc.tile_pool(name="oB", bufs=1, space="PSUM"))


        # ---------- prologue: loads + consts ----------
        wqk_sb, wv_sb, xs = [], [], []
        for c in range(CCH):
            t = sb.tile([128, HP * 128], F32R, tag=f"wqk{c}")
            nc.sync.dma_start(t[:], wqk[c * 128:(c + 1) * 128, :])
            wqk_sb.append(t)
        for c in range(CCH):
            t = sb.tile([128, N], F32R, tag=f"x{c}")
            nc.sync.dma_start(t[:, 0:1024], xT[c * 128:(c + 1) * 128, 0:1024])
            nc.gpsimd.dma_start(t[:, 1024:2048], xT[c * 128:(c + 1) * 128, 1024:2048])
            xs.append(t)
        for c in range(CCH):
            t = sb.tile([128, 256], F32R, tag=f"wv{c}")
            nc.gpsimd.dma_start(t[:], wv[c * 128:(c + 1) * 128, :])
            wv_sb.append(t)
        bqk_sb = sb.tile([1, HP * 128], F32R, tag="bqk")
        nc.sync.dma_start(bqk_sb[:], bqk[:, :])
        bv_sb = sb.tile([1, 256], F32R, tag="bv")
        nc.gpsimd.dma_start(bv_sb[:], bv[:, :])
        cos_sb = sb.tile([128, N], F32, tag="cos")
        nc.gpsimd.dma_start(cos_sb[:], cos2w[:, :])
        sin_sb = sb.tile([128, N], F32, tag="sin")
        nc.gpsimd.dma_start(sin_sb[:], sinSw[:, :])
        sel_sb = sb.tile([128, 512], F32R, tag="sel")
        nc.gpsimd.dma_start(sel_sb[:], sel4[:, :])
        wp0_sb = sb.tile([128, C], F32R, tag="wp0")
        nc.gpsimd.dma_start(wp0_sb[:], wp[0:128, :])
        wp1_sb = sb.tile([64, C], F32R, tag="wp1")
        nc.gpsimd.dma_start(wp1_sb[:], wp[128:192, :])

        ones_row = sb.tile([1, 512], F32R, tag="ones_row")
        nc.gpsimd.dma_start(ones_row[:], onesd[0:1, :])
        onesp = sb.tile([128, 2], F32R, tag="onesp")
        nc.gpsimd.dma_start(onesp[:], onespd[:, :])
        ones64 = sb.tile([1, 64], F32R, tag="ones64")
        nc.gpsimd.dma_start(ones64[:], onesd[0:1, 0:64])
        eps_t = sb.tile([128, 1], F32, tag="eps")
        nc.gpsimd.memset(eps_t[:], EPS)
        v3i = sb.tile([128, HP * KB * 65], F32R, tag="v3i")  # [v_h(kb) | 1] blocks
        nc.gpsimd.dma_start(
            v3i[:].rearrange("p (b n) -> p b n", n=65)[:, :, 64:65],
            vones[:, :, None])

        # qT/kT packed by head pairs so S-matmul operands share a base partition
        q12 = sb.tile([128, N], F32R, tag="q12")   # qT(0) rows 0:64, qT(1) rows 64:128
        k12 = sb.tile([128, N], F32R, tag="k12")
        q3 = sb.tile([64, N], F32R, tag="q3")
        k3 = sb.tile([64, N], F32R, tag="k3")

        def qT(h):
            return (q12[0:64], q12[64:128], q3[:])[h]

        def kT(h):
            return (k12[0:64], k12[64:128], k3[:])[h]

        oall_a = sb.tile([128, N], F32R, tag="oall_a")   # heads 0,1 O^T
        oall_b = sb.tile([64, N], F32R, tag="oall_b")    # head 2 O^T
        t4_all = sb.tile([128, N], F32, tag="t4_all")
        s_sb = sb.tile([128, 512], F32, tag="s_sb")
        nc.gpsimd.memset(s_sb[:], 1.0)
        lnv = sb.tile([128, 512], F32, tag="lnv")
        sv = sb.tile([128, 512], F32R, tag="sv")

        def mm(out_ap, lhsT, rhs, start, stop):
            nc.tensor.matmul(out_ap, lhsT.bitcast(F32R), rhs.bitcast(F32R),
                             start=start, stop=stop, skip_group_check=True)

        # ---------- qkv for head h ----------
        def qkv_passA(h, t):
            ts = slice(t * 512, (t + 1) * 512)
            qk_ps = fps.tile([128, 512], F32, tag="flex")
            for c in range(CCH):
                mm(qk_ps[:], wqk_sb[c][:, h * 128:(h + 1) * 128],
                   xs[c][:, ts], c == 0, False)
            mm(qk_ps[:], bqk_sb[:, h * 128:(h + 1) * 128], ones_row[:],
               False, True)
            t1 = tp1.tile([128, 512], F32, tag="t1")
            nc.vector.tensor_mul(t1[:], qk_ps[:], cos_sb[:, ts])
            t2 = tp.tile([128, 512], F32, tag="t2")
            nc.vector.stream_shuffle(t2[:], qk_ps[:], SWAP_MASK)
            sq = tp.tile([128, 512], F32R, tag="sq")
            nc.vector.tensor_mul(sq[:], t2[:], t2[:])
            t3 = tp1.tile([128, 512], F32, tag="t3")
            nc.vector.tensor_mul(t3[:], t2[:], sin_sb[:, ts])
            mm(qk_ps[0:2, :], onesp[:], sq[:], True, True)
            nc.vector.tensor_copy(s_sb[32 * t:32 * t + 2, :], qk_ps[0:2, :])
            nc.vector.tensor_add(t4_all[:, ts], t1[:], t3[:])

        def qkv_finish(h):
            nc.scalar.activation(lnv[:], s_sb[:], AF.Ln,
                                 bias=eps_t[:], scale=1.0 / HD)
            nc.scalar.activation(sv[:], lnv[:], AF.Exp, bias=0.0, scale=-0.5)
            for t in range(NT):
                ts = slice(t * 512, (t + 1) * 512)
                sqk_ps = fps.tile([128, 512], F32, tag="flex")
                mm(sqk_ps[:], sel_sb[:, t * 128:(t + 1) * 128], sv[:],
                   True, True)
                nc.vector.tensor_mul(qT(h)[:, ts], t4_all[0:64, ts],
                                     sqk_ps[0:64, :])
                nc.vector.tensor_mul(kT(h)[:, ts], t4_all[64:128, ts],
                                     sqk_ps[64:128, :])

        def qkv(h):
            for t in range(NT):
                qkv_passA(h, t)
            qkv_finish(h)

        # ---------- v for all heads ----------
        def vphase_tt(tt):
            v_ps = fps.tile([128, 256], F32, tag="flex")
            for c in range(CCH):
                mm(v_ps[:], xs[c][:, tt * 128:(tt + 1) * 128], wv_sb[c][:],
                   c == 0, False)
            mm(v_ps[:], ones_row[0:1, 0:128], bv_sb[:], False, True)
            # strided copy of 3 head-blocks into v3i (+ ones col at 64)
            dst = v3i[:].rearrange("p (h k n) -> p h k n", h=HP, k=KB)
            nc.vector.tensor_copy(
                dst[:, :, tt, 0:64],
                v_ps[:, 0:192].rearrange("p (h n) -> p h n", h=HP))

        # ---------- attention ----------
        # 16 k-blocks in groups of 2 (one 2-bank PSUM tile per group)
        G2 = [(2 * g, 2 * g + 1) for g in range(8)]

        def epilogue(h, qt, o_ps):
            qs = slice(qt * 512, (qt + 1) * 512)
            ld = tp1.tile([1, 512], F32, tag="ld")
            nc.scalar.activation(ld[:], o_ps[64:65, :], AF.Ln,
                                 bias=0.0, scale=1.0)
            rec = tp1.tile([1, 512], F32R, tag="rec")
            nc.scalar.activation(rec[:], ld[:], AF.Exp, bias=0.0, scale=-1.0)
            rec_ps = fps.tile([64, 512], F32, tag="flex")
            mm(rec_ps[:], ones64[:], rec[:], True, True)
            rec_b = tp1.tile([64, 512], F32, tag="rec_b")
            nc.vector.tensor_copy(rec_b[:], rec_ps[:])
            if h < 2:
                dst = oall_a[h * 64:(h + 1) * 64, qs]
            else:
                dst = oall_b[:, qs]
            nc.vector.tensor_mul(dst, o_ps[0:64, :], rec_b[:])

        def smm(spool, h, kbs, qs):
            s_ps = spool.tile([128, 1024], F32, tag="s")
            for j, kb in enumerate(kbs):
                mm(s_ps[:, j * 512:(j + 1) * 512],
                   kT(h)[:, kb * 128:(kb + 1) * 128], qT(h)[:, qs], True, True)
            return s_ps

        def pexp_of(s_ps):
            px = pe.tile([128, 1024], F32R, tag="pexp")
            nc.scalar.activation(px[:], s_ps[:], AF.Exp, bias=0.0, scale=0.125)
            return px

        def omm(o_ps, h, kbs, px):
            for j, kb in enumerate(kbs):
                mm(o_ps[:], v3i[:, (h * KB + kb) * 65:(h * KB + kb) * 65 + 65],
                   px[:, j * 512:(j + 1) * 512], kb == 0, kb == KB - 1)

        # ---------- partial projection (token tiles of one q-tile) ----------
        def proj_qt(qt):
            for tt in range(4 * qt, 4 * qt + 4):
                po = tp.tile([128, C], F32, tag="po")
                for half in range(2):
                    cs = slice(half * 384, (half + 1) * 384)
                    p_ps = fps.tile([128, 512], F32, tag="flex")
                    mm(p_ps[:, 0:384], oall_a[:, tt * 128:(tt + 1) * 128],
                       wp0_sb[:, cs], True, False)
                    mm(p_ps[:, 0:384], oall_b[:, tt * 128:(tt + 1) * 128],
                       wp1_sb[:, cs], False, True)
                    nc.vector.tensor_copy(po[:, cs], p_ps[:, 0:384])
                nc.sync.dma_start(out[tt * 128:(tt + 1) * 128, :], po[:])


        def attn_single(h, extra=None):
            for qt in range(NT):
                qs = slice(qt * 512, (qt + 1) * 512)
                o_ps = (oA if qt % 2 == 0 else oB).tile([65, 512], F32, tag="o")
                for g, kbs in enumerate(G2):
                    s_ps = smm(sA if g % 2 == 0 else sB, h, kbs, qs)
                    px = pexp_of(s_ps)
                    omm(o_ps, h, kbs, px)
                epilogue(h, qt, o_ps)
                if extra is not None:
                    extra(qt)

        def attn_pair(h0, h1):
            # h0/h1 S-matmuls sit in different PE row-groups (base partition
            # 0 vs 64) and different PSUM banks -> they run concurrently.
            for qt in range(NT):
                qs = slice(qt * 512, (qt + 1) * 512)
                o0 = oA.tile([65, 512], F32, tag="o")
                o1 = oB.tile([65, 512], F32, tag="o")
                for kbs in G2:
                    s0 = smm(sA, h0, kbs, qs)
                    s1 = smm(sB, h1, kbs, qs)
                    px0 = pexp_of(s0)
                    omm(o0, h0, kbs, px0)
                    px1 = pexp_of(s1)
                    omm(o1, h1, kbs, px1)
                epilogue(h0, qt, o0)
                epilogue(h1, qt, o1)
                proj_qt(qt)

        def prep_next(qt):
            if qt == 0:
                qkv_passA(1, 0)
            elif qt == 1:
                qkv_passA(1, 1)
                qkv_passA(1, 2)
                qkv_passA(1, 3)
            elif qt == 2:
                qkv_finish(1)
                qkv_passA(2, 0)
                qkv_passA(2, 1)
            else:
                qkv_passA(2, 2)
                qkv_passA(2, 3)
                qkv_finish(2)

        qkv(0)
        for tt in range(KB):
            vphase_tt(tt)
        attn_single(0, extra=prep_next)
        attn_pair(1, 2)

    if split_waits:
        _split_waits(nc)
    return nc


def _split_waits(nc):
    """This walrus build lowers at most one sync-wait per instruction (the
    matmul LDW struct rejects 2+). Move excess waits onto NoOps inserted
    just before, on the same engine queue — queues are in-order, so the
    constraint is preserved exactly."""
    k = 0
    for fn in nc.m.functions:
        for bb in fn.blocks:
            il = bb.instructions
            idx = 0
            while idx < len(il):
                inst = il[idx]
                si = inst.sync_info
                eng = getattr(inst, "engine", None)
                if (si is not None and len(si.on_wait) > 1
                        and eng is not None
                        and str(eng) != "EngineType.Unassigned"):
                    waits = list(si.on_wait)
                    inst.sync_info = mybir.SyncInfo(
                        on_wait=[waits[-1]], on_update=list(si.on_update))
                    for w in waits[:-1]:
                        nop = mybir.InstNoOp(
                            name=f"I-waitnop-{k}", engine=eng, ins=[], outs=[],
                            sync_info=mybir.SyncInfo(on_wait=[w], on_update=[]))
                        k += 1
                        il.insert(idx, nop)
                        idx += 1
                idx += 1


def _prep_core_inputs(core, x, rope_cos, rope_sin, qkv_kernel, qkv_bias,
                      proj_kernel, proj_bias, q_norm_w, k_norm_w):
    b = core // 4
    heads = [3 * (core % 4) + i for i in range(HP)]

    wq = qkv_kernel.reshape(C, 3, H, HD)
    bq = qkv_bias.reshape(3, H, HD)

    xT = np.ascontiguousarray(x[b].T, dtype=np.float32)

    wqk = np.empty((C, HP * 128), np.float32)
    bqk = np.empty((1, HP * 128), np.float32)
    for i, h in enumerate(heads):
        wqk[:, i * 128:i * 128 + 64] = wq[:, 0, h, PERM]
        wqk[:, i * 128 + 64:(i + 1) * 128] = wq[:, 1, h, PERM]
        bqk[0, i * 128:i * 128 + 64] = bq[0, h, PERM]
        bqk[0, i * 128 + 64:(i + 1) * 128] = bq[1, h, PERM]

    wv = np.zeros((C, 256), np.float32)
    bv = np.zeros((1, 256), np.float32)
    for i, h in enumerate(heads):
        wv[:, i * 64:(i + 1) * 64] = wq[:, 2, h, :]
        bv[0, i * 64:(i + 1) * 64] = bq[2, h, :]

    cosT = rope_cos.T  # (HD, N)
    sinT = rope_sin.T
    cos2w = np.empty((128, N), np.float32)
    sinSw = np.empty((128, N), np.float32)
    cos2w[0:64] = cosT[PERM] * q_norm_w[PERM][:, None]
    cos2w[64:128] = cosT[PERM] * k_norm_w[PERM][:, None]
    sinSw[0:64] = SIGN[:, None] * sinT[PERM] * q_norm_w[PERM][:, None]
    sinSw[64:128] = SIGN[:, None] * sinT[PERM] * k_norm_w[PERM][:, None]

    onesd = np.ones((128, 512), np.float32)
    onespd = np.zeros((128, 2), np.float32)
    onespd[0:64, 0] = 1.0    # col0: ones on q rows
    onespd[64:128, 1] = 1.0  # col1: ones on k rows
    vones = np.ones((128, HP * KB), np.float32)

    sel4 = np.zeros((128, 512), np.float32)
    for t in range(NT):
        sel4[32 * t, t * 128:t * 128 + 64] = 1.0
        sel4[32 * t + 1, t * 128 + 64:(t + 1) * 128] = 1.0

    rows = np.concatenate([np.arange(h * HD, (h + 1) * HD) for h in heads])
    wp = np.ascontiguousarray(proj_kernel[rows, :], dtype=np.float32)

    return {"xT": xT, "wqk": wqk, "wv": wv, "bqk": bqk, "bv": bv,
            "cos2w": cos2w, "sinSw": sinSw, "sel4": sel4,
            "wp": wp, "onesd": onesd, "onespd": onespd, "vones": vones}


def kernel(x, rope_cos, rope_sin, qkv_kernel, qkv_bias, proj_kernel,
           proj_bias, q_norm_w, k_norm_w, _trace=False):
    args = [np.asarray(a, dtype=np.float32) for a in
            (x, rope_cos, rope_sin, qkv_kernel, qkv_bias, proj_kernel,
             proj_bias, q_norm_w, k_norm_w)]
    in_maps = [_prep_core_inputs(c, *args) for c in range(NCORES)]

    if "nc" not in _NC_CACHE:
        _NC_CACHE["nc"] = build_nc()
    nc = _NC_CACHE["nc"]

    res = run_bass_kernel_spmd(nc, in_maps, core_ids=list(range(NCORES)),
                               trace=_trace)
    parts = [res.results[c]["out"] for c in range(NCORES)]
    out = np.empty((B, N, C), np.float32)
    pb = np.asarray(proj_bias, dtype=np.float32)
    for b in range(B):
        out[b] = parts[4 * b] + parts[4 * b + 1] + parts[4 * b + 2] + parts[4 * b + 3] + pb
    if _trace:
        kernel.last_results = res
    return out



# revision 13
# speedup vs baseline: 1.4885x; 1.4885x over previous
"""Multi-head attention (RMSNorm-QK + RoPE + softmax + proj) on 8 Trainium2 cores.

Sharding: core c handles batch b = c//4 and heads [3*(c%4), 3*(c%4)+3).
Each core computes qkv for its heads, flash-style attention, and a partial
projection over its heads' channels; the host sums the 4 partials per batch.

Layout tricks (bf16 data path, fp32 PSUM accumulation):
 - q^T/k^T layout [head_dim, tokens]; head-dim rows permuted so the RoPE
   half-swap is an intra-quadrant stream_shuffle.
 - RMS-norm: sum(q^2) via ones-pair matmul; rsqrt = exp(-0.5*ln(x)) so the
   whole kernel uses one ACT table set (natural_log_exp_and_others).
 - rsqrt scales broadcast across partitions on the (idle) GPSIMD engine.
 - softmax without max-subtraction (logits bounded by RMS norm); denominators
   via an appended ones-column in the PV matmul; 1/denom on DVE reciprocal.
 - projection partials DMA'd straight from PSUM to DRAM.
"""
import sys

for _p in ("/opt/trn_rl_repo", "/opt/trn_rl_repo/concourse"):
    if _p not in sys.path:
        sys.path.insert(0, _p)

import numpy as np
import ml_dtypes
from contextlib import ExitStack

import concourse.bass as bass
import concourse.tile as tile
import concourse.mybir as mybir
from concourse.bass_utils import run_bass_kernel_spmd

F32 = mybir.dt.float32
BF16 = mybir.dt.bfloat16
AF = mybir.ActivationFunctionType
BF = ml_dtypes.bfloat16

B, N, C = 2, 2048, 768
H, HD = 12, 64
HP = 3            # heads per core
NCORES = 8
CCH = C // 128    # 6 contraction chunks
NT = N // 512     # 4 token tiles of 512
KB = N // 128     # 16 k-blocks of 128
EPS = 1e-6

SWAP_MASK = [(i + 16) % 32 for i in range(32)]
# head-dim permutation: pair-exchange (d <-> d+32) becomes intra-quadrant
PERM = np.concatenate([np.arange(0, 16), np.arange(32, 48),
                       np.arange(16, 32), np.arange(48, 64)])
SIGN = np.where(PERM < 32, -1.0, 1.0).astype(np.float32)

_NC_CACHE = {}


def build_nc(split_waits=True, zero_bias=True):
    nc = bass.Bass(target_bir_lowering=True)
    xT = nc.declare_dram_parameter("xT", [C, N], BF16, isOutput=False)
    wqk = nc.declare_dram_parameter("wqk", [C, HP * 128], BF16, isOutput=False)
    wv = nc.declare_dram_parameter("wv", [C, 192], BF16, isOutput=False)
    bqk = nc.declare_dram_parameter("bqk", [1, HP * 128], BF16, isOutput=False)
    bv = nc.declare_dram_parameter("bv", [1, 192], BF16, isOutput=False)
    cos2w = nc.declare_dram_parameter("cos2w", [128, N], BF16, isOutput=False)
    sinSw = nc.declare_dram_parameter("sinSw", [128, N], BF16, isOutput=False)
    sel4 = nc.declare_dram_parameter("sel4", [128, 512], BF16, isOutput=False)
    ident = nc.declare_dram_parameter("ident", [128, 128], BF16, isOutput=False)
    wp = nc.declare_dram_parameter("wp", [HP * HD, C], BF16, isOutput=False)
    onesd = nc.declare_dram_parameter("onesd", [128, 512], BF16, isOutput=False)
    onespd = nc.declare_dram_parameter("onespd", [128, 2], BF16, isOutput=False)
    vones = nc.declare_dram_parameter("vones", [128, HP * KB], BF16, isOutput=False)
    out = nc.declare_dram_parameter("out", [N, C], BF16, isOutput=True)

    with tile.TileContext(nc) as tc, ExitStack() as ctx:
        sb = ctx.enter_context(tc.tile_pool(name="sb", bufs=1))
        tp = ctx.enter_context(tc.tile_pool(name="tp", bufs=2))
        pe = ctx.enter_context(tc.tile_pool(name="pe", bufs=4))   # pexp
        tp1 = ctx.enter_context(tc.tile_pool(name="tp1", bufs=2))
        fps = ctx.enter_context(tc.tile_pool(name="fps", bufs=2, space="PSUM"))
        sA = ctx.enter_context(tc.tile_pool(name="sA", bufs=1, space="PSUM"))
        sB = ctx.enter_context(tc.tile_pool(name="sB", bufs=1, space="PSUM"))
        oA = ctx.enter_context(tc.tile_pool(name="oA", bufs=1, space="PSUM"))
        oB = ctx.enter_context(tc.tile_pool(name="oB", bufs=1, space="PSUM"))

        # ---------- prologue: loads + consts ----------
        # Consolidated DMAs (one descriptor set each) to dodge per-DMA queue
        # overheads; SP carries x halves, ACT carries weights/tables, Pool
        # only runs memsets so the lead-in rope muls aren't queued behind DMA.
        xall = sb.tile([128, CCH * N], BF16, tag="xall")
        xs = [xall[:, c * N:(c + 1) * N] for c in range(CCH)]
        wqk_all = sb.tile([128, CCH * HP * 128], BF16, tag="wqk_all")
        wqk_sb = [wqk_all[:, c * HP * 128:(c + 1) * HP * 128]
                  for c in range(CCH)]
        wv_all = sb.tile([128, CCH * 192], BF16, tag="wv_all")
        wv_sb = [wv_all[:, c * 192:(c + 1) * 192] for c in range(CCH)]
        cos_sb = sb.tile([128, N], BF16, tag="cos")
        sin_sb = sb.tile([128, N], BF16, tag="sin")
        onesp = sb.tile([128, 2], BF16, tag="onesp")

        xsrc = xT[:, :].rearrange("(c p) n -> p c n", c=CCH)
        xdst = xall[:].rearrange("p (c n) -> p c n", c=CCH)
        nc.scalar.dma_start(
            wqk_all[:].rearrange("p (c n) -> p c n", c=CCH),
            wqk[:, :].rearrange("(c p) n -> p c n", c=CCH))
        zrow = sb.tile([1, 260], BF16, tag="zrow")
        nc.gpsimd.memset(zrow[:], 0.0)
        nc.sync.dma_start(xdst[:, :, 0:512], xsrc[:, :, 0:512])
        nc.scalar.dma_start(
            wv_all[:].rearrange("p (c n) -> p c n", c=CCH),
            wv[:, :].rearrange("(c p) n -> p c n", c=CCH))
        nc.scalar.dma_start(cos_sb[:], cos2w[:, :])
        nc.sync.dma_start(xdst[:, :, 512:1024], xsrc[:, :, 512:1024])
        nc.sync.dma_start(sin_sb[:], sinSw[:, :])
        nc.sync.dma_start(onesp[:], onespd[:, :])
        nc.sync.dma_start(xdst[:, :, 1024:1536], xsrc[:, :, 1024:1536])
        nc.sync.dma_start(xdst[:, :, 1536:2048], xsrc[:, :, 1536:2048])

        eps_t = sb.tile([128, 1], F32, tag="eps")
        nc.gpsimd.memset(eps_t[:], EPS)
        v3i = sb.tile([128, HP * KB * 65], BF16, tag="v3i")  # [v_h(kb) | 1] blocks
        nc.gpsimd.memset(
            v3i[:].rearrange("p (b n) -> p b n", n=65)[:, :, 64:65], 1.0)

        sel_sb = sb.tile([128, 512], BF16, tag="sel")
        nc.sync.dma_start(sel_sb[:], sel4[:, :])
        ident_sb = sb.tile([128, 128], BF16, tag="ident")
        nc.sync.dma_start(ident_sb[:], ident[:, :])
        bqk_sb = sb.tile([1, HP * 128], BF16, tag="bqk")
        bv_sb = sb.tile([1, 192], BF16, tag="bv")
        ones_row = sb.tile([1, 512], BF16, tag="ones_row")
        if not zero_bias:
            nc.sync.dma_start(bqk_sb[:], bqk[:, :])
            nc.sync.dma_start(bv_sb[:], bv[:, :])
            nc.sync.dma_start(ones_row[:], onesd[0:1, :])
        wp0_sb = sb.tile([128, C], BF16, tag="wp0")
        nc.scalar.dma_start(wp0_sb[:], wp[0:128, :])
        wp1_sb = sb.tile([64, C], BF16, tag="wp1")
        nc.scalar.dma_start(wp1_sb[:], wp[128:192, :])

        # qT/kT packed by head pairs so S-matmul operands share a base partition
        q12 = sb.tile([128, N], BF16, tag="q12")   # qT(0) rows 0:64, qT(1) rows 64:128
        k12 = sb.tile([128, N], BF16, tag="k12")
        q3 = sb.tile([64, N], BF16, tag="q3")
        k3 = sb.tile([64, N], BF16, tag="k3")

        def qT(h):
            return (q12[0:64], q12[64:128], q3[:])[h]

        def kT(h):
            return (k12[0:64], k12[64:128], k3[:])[h]

        oall_a = sb.tile([128, N], BF16, tag="oall_a")   # heads 0,1 O^T
        oall_b = sb.tile([64, N], BF16, tag="oall_b")    # head 2 O^T
        t4_all = sb.tile([128, N], BF16, tag="t4_all")
        s_sb = sb.tile([128, 512], F32, tag="s_sb")
        nc.gpsimd.memset(s_sb[:], 1.0)
        lnv = sb.tile([128, 512], F32, tag="lnv")
        sv = sb.tile([128, 512], BF16, tag="sv")

        def mm(out_ap, lhsT, rhs, start, stop):
            nc.tensor.matmul(out_ap, lhsT, rhs,
                             start=start, stop=stop, skip_group_check=True)

        # ---------- qkv for head h ----------
        def qkv_passA(h, t):
            ts = slice(t * 512, (t + 1) * 512)
            qk_ps = fps.tile([128, 512], F32, tag="flex")
            for c in range(CCH):
                mm(qk_ps[:], wqk_sb[c][:, h * 128:(h + 1) * 128],
                   xs[c][:, ts], c == 0, zero_bias and c == CCH - 1)
            if not zero_bias:
                mm(qk_ps[:], bqk_sb[:, h * 128:(h + 1) * 128], ones_row[:],
                   False, True)
            t2 = tp.tile([128, 512], F32, tag="t2")
            nc.vector.stream_shuffle(t2[:], qk_ps[:], SWAP_MASK)
            t1 = tp1.tile([128, 512], BF16, tag="t1")
            nc.vector.tensor_mul(t1[:], qk_ps[:], cos_sb[:, ts])
            sq = tp.tile([128, 512], BF16, tag="sq")
            nc.gpsimd.tensor_mul(sq[:], t2[:], t2[:])
            t3 = tp1.tile([128, 512], BF16, tag="t3")
            nc.gpsimd.tensor_mul(t3[:], t2[:], sin_sb[:, ts])
            sm_ps = fps.tile([2, 512], F32, tag="flex")
            mm(sm_ps[:], onesp[:], sq[:], True, True)
            nc.vector.tensor_copy(s_sb[32 * t:32 * t + 2, :], sm_ps[:])
            nc.vector.tensor_add(t4_all[:, ts], t1[:], t3[:])

        def finish_tile(h, t):
            ts = slice(t * 512, (t + 1) * 512)
            sqk_ps = fps.tile([128, 512], F32, tag="flex")
            mm(sqk_ps[:], sel_sb[:, t * 128:(t + 1) * 128], sv[:],
               True, True)
            nc.vector.tensor_mul(qT(h)[:, ts], t4_all[0:64, ts],
                                 sqk_ps[0:64, :])
            nc.vector.tensor_mul(kT(h)[:, ts], t4_all[64:128, ts],
                                 sqk_ps[64:128, :])

        def qkv_finish(h):
            nc.scalar.activation(lnv[:], s_sb[:], AF.Ln,
                                 bias=eps_t[:], scale=1.0 / HD)
            nc.scalar.activation(sv[:], lnv[:], AF.Exp, bias=0.0, scale=-0.5)
            for t in range(NT):
                finish_tile(h, t)

        def qkv_finish0_lnexp(t):
            # head 0 (lead-in): per-tile Ln/Exp so kT(0) completes ~5us
            # earlier; ACT is idle here so the extra instructions are free.
            rows = slice(32 * t, 32 * t + 2)
            nc.scalar.activation(lnv[rows, :], s_sb[rows, :], AF.Ln,
                                 bias=eps_t[rows, :], scale=1.0 / HD)
            nc.scalar.activation(sv[rows, :], lnv[rows, :], AF.Exp,
                                 bias=0.0, scale=-0.5)

        def qkv(h):
            for t in range(NT):
                qkv_passA(h, t)
            qkv_finish(h)

        # ---------- v for all heads ----------
        def vphase_tt(tt):
            v_ps = fps.tile([128, 192], F32, tag="flex")
            for c in range(CCH):
                mm(v_ps[:], xs[c][:, tt * 128:(tt + 1) * 128], wv_sb[c][:],
                   c == 0, zero_bias and c == CCH - 1)
            if not zero_bias:
                mm(v_ps[:], ones_row[0:1, 0:128], bv_sb[:], False, True)
            # strided copy of 3 head-blocks into v3i (+ ones col at 64);
            # on ACT (copy is in the ln/exp table set) -- ACT idles in lead-in
            dst = v3i[:].rearrange("p (h k n) -> p h k n", h=HP, k=KB)
            nc.scalar.activation(
                dst[:, :, tt, 0:64],
                v_ps[:, 0:192].rearrange("p (h n) -> p h n", h=HP),
                AF.Copy, bias=0.0, scale=1.0)

        # ---------- attention ----------
        # 16 k-blocks in groups of 2 (one 2-bank PSUM tile per group)
        G2 = [(2 * g, 2 * g + 1) for g in range(8)]

        def epilogue(h, qt, o_ps, proj=False, tail=False):
            # o_ps: [128 q, 4*65] -- per q-block 128: 64 head-dims + denom col
            for qb in range(4):
                tt = qt * 4 + qb
                cs = slice(qb * 65, qb * 65 + 64)
                rec = tp1.tile([128, 1], F32, tag="rec")
                nc.vector.reciprocal(rec[:], o_ps[:, qb * 65 + 64:qb * 65 + 65])
                o_n = tp1.tile([128, 64], BF16, tag="o_n")
                nc.vector.tensor_scalar_mul(o_n[:], o_ps[:, cs], rec[:])
                tr_ps = fps.tile([64, 128], BF16, tag="flex")
                nc.tensor.transpose(tr_ps[:], o_n[:], ident_sb[:])
                if h < 2:
                    dst = oall_a[h * 64:(h + 1) * 64,
                                 tt * 128:(tt + 1) * 128]
                else:
                    dst = oall_b[:, tt * 128:(tt + 1) * 128]
                nc.vector.tensor_copy(dst, tr_ps[:])
                if proj:
                    proj_tt(tt, on_act=tail)

        def smm(spool, h, kbs, qs):
            s_ps = spool.tile([128, 1024], F32, tag="s")
            for j, kb in enumerate(kbs):
                mm(s_ps[:, j * 512:(j + 1) * 512],
                   kT(h)[:, kb * 128:(kb + 1) * 128], qT(h)[:, qs], True, True)
            return s_ps

        def pexp_of(s_ps):
            px = pe.tile([128, 1024], BF16, tag="pexp")
            nc.scalar.activation(px[:], s_ps[:], AF.Exp, bias=0.0, scale=0.125)
            return px

        def omm(o_ps, h, kbs, px):
            # o_ps was zeroed by memset; start=True would wipe the whole 2KB
            # psum zero-region, clobbering sibling q-blocks' accumulators.
            for j, kb in enumerate(kbs):
                for qb in range(4):
                    mm(o_ps[:, qb * 65:(qb + 1) * 65],
                       px[:, j * 512 + qb * 128:j * 512 + (qb + 1) * 128],
                       v3i[:, (h * KB + kb) * 65:(h * KB + kb) * 65 + 65],
                       False, False)

        # ---------- partial projection (one 128-token tile) ----------
        def proj_tt(tt, on_act=False):
            po = tp.tile([128, C], BF16, tag="po")
            for half in range(2):
                cs = slice(half * 384, (half + 1) * 384)
                p_ps = fps.tile([128, 384], F32, tag="flex")
                mm(p_ps[:], oall_a[:, tt * 128:(tt + 1) * 128],
                   wp0_sb[:, cs], True, False)
                mm(p_ps[:], oall_b[:, tt * 128:(tt + 1) * 128],
                   wp1_sb[:, cs], False, True)
                if on_act:  # tail: ACT is idle once the last exps drain
                    nc.scalar.activation(po[:, cs], p_ps[:],
                                         AF.Copy, bias=0.0, scale=1.0)
                else:
                    nc.vector.tensor_copy(po[:, cs], p_ps[:])
            nc.sync.dma_start(out[tt * 128:(tt + 1) * 128, :], po[:])

        # epilogues are software-pipelined: each qt's epilogue is emitted
        # after group 1 of the NEXT qt, so the next qt's S-matmuls are not
        # queued behind the epilogue's DVE->PE transpose chain.
        pending_epi = []

        def drain_epi():
            while pending_epi:
                epilogue(*pending_epi.pop(0))

        # S-matmuls run one group ahead of pexp/omm in the PE queue, so the
        # next group's S is computed while ACT works and ACT is never starved
        # behind an omm burst at the PE queue head. The in-flight group
        # carries across phase boundaries and is drained at the very end.
        pipe = []  # [(h, s_ps, kbs, o_ps)]

        def drain_pipe():
            while pipe:
                ph, ps, pk, po = pipe.pop(0)
                px = pexp_of(ps)
                omm(po, ph, pk, px)

        def attn_single(h, extra=None, proj=False, tail=False):
            for qt in range(NT):
                qs = slice(qt * 512, (qt + 1) * 512)
                o_ps = (oA if qt % 2 == 0 else oB).tile([128, 260], F32, tag="o")
                # zero via a tiny PE matmul: start=True wipes the psum
                # zero-region; keeps the zeroing off the busy DVE queue
                mm(o_ps[:], ident_sb[0:1, :], zrow[:], True, True)
                for g, kbs in enumerate(G2):
                    # prep at g==6: the 2-3 buffered pexp groups on the ACT
                    # queue absorb the prep matmul burst in the PE queue
                    if g == 6 and extra is not None:
                        extra(qt)
                    s_ps = smm(sA if g % 2 == 0 else sB, h, kbs, qs)
                    drain_pipe()
                    pipe.append((h, s_ps, kbs, o_ps))
                    if g == 1:
                        drain_epi()
                pending_epi.append((h, qt, o_ps, proj, tail and qt == NT - 1))

        def prep(h):
            def extra(qt):
                if qt == 0:
                    qkv_passA(h, 0)
                    qkv_passA(h, 1)
                elif qt == 1:
                    qkv_passA(h, 2)
                    qkv_passA(h, 3)
                elif qt == 2:
                    qkv_finish(h)
            return extra

        nc.gpsimd.memset(sv[:], 0.0)
        # warm up the PE p-state ramp during the initial DMA wait: ~9us of
        # junk matmuls so the real qkv matmuls start at full clock
        warm = oB.tile([128, 260], F32, tag="o")
        for _ in range(28):
            mm(warm[:], zrow[0:1, 0:128], zrow[0:1, :], True, True)
        for t in range(NT):
            qkv_passA(0, t)
            qkv_finish0_lnexp(t)
            for tt in range(4 * t, 4 * t + 4):
                vphase_tt(tt)
            finish_tile(0, t)
        attn_single(0, extra=prep(1))
        attn_single(1, extra=prep(2))
        attn_single(2, proj=True, tail=True)
        drain_pipe()
        drain_epi()

    if split_waits:
        _split_waits(nc)
    return nc


def _split_waits(nc):
    """This walrus build lowers at most one sync-wait per instruction (the
    matmul LDW struct rejects 2+). Move excess waits onto NoOps inserted
    just before, on the same engine queue — queues are in-order, so the
    constraint is preserved exactly."""
    k = 0
    for fn in nc.m.functions:
        for bb in fn.blocks:
            il = bb.instructions
            idx = 0
            while idx < len(il):
                inst = il[idx]
                si = inst.sync_info
                eng = getattr(inst, "engine", None)
                if (si is not None and len(si.on_wait) > 1
                        and eng is not None
                        and str(eng) != "EngineType.Unassigned"):
                    waits = list(si.on_wait)
                    inst.sync_info = mybir.SyncInfo(
                        on_wait=[waits[-1]], on_update=list(si.on_update))
                    for w in waits[:-1]:
                        nop = mybir.InstNoOp(
                            name=f"I-waitnop-{k}", engine=eng, ins=[], outs=[],
                            sync_info=mybir.SyncInfo(on_wait=[w], on_update=[]))
                        k += 1
                        il.insert(idx, nop)
                        idx += 1
                idx += 1


def _prep_core_inputs(core, x, rope_cos, rope_sin, qkv_kernel, qkv_bias,
                      proj_kernel, proj_bias, q_norm_w, k_norm_w):
    b = core // 4
    heads = [3 * (core % 4) + i for i in range(HP)]

    wq = qkv_kernel.reshape(C, 3, H, HD)
    bq = qkv_bias.reshape(3, H, HD)

    xT = np.ascontiguousarray(x[b].T).astype(BF)

    wqk = np.empty((C, HP * 128), np.float32)
    bqk = np.empty((1, HP * 128), np.float32)
    for i, h in enumerate(heads):
        wqk[:, i * 128:i * 128 + 64] = wq[:, 0, h, PERM]
        wqk[:, i * 128 + 64:(i + 1) * 128] = wq[:, 1, h, PERM]
        bqk[0, i * 128:i * 128 + 64] = bq[0, h, PERM]
        bqk[0, i * 128 + 64:(i + 1) * 128] = bq[1, h, PERM]

    wv = np.zeros((C, 192), np.float32)
    bv = np.zeros((1, 192), np.float32)
    for i, h in enumerate(heads):
        wv[:, i * 64:(i + 1) * 64] = wq[:, 2, h, :]
        bv[0, i * 64:(i + 1) * 64] = bq[2, h, :]

    cosT = rope_cos.T  # (HD, N)
    sinT = rope_sin.T
    cos2w = np.empty((128, N), np.float32)
    sinSw = np.empty((128, N), np.float32)
    cos2w[0:64] = cosT[PERM] * q_norm_w[PERM][:, None]
    cos2w[64:128] = cosT[PERM] * k_norm_w[PERM][:, None]
    sinSw[0:64] = SIGN[:, None] * sinT[PERM] * q_norm_w[PERM][:, None]
    sinSw[64:128] = SIGN[:, None] * sinT[PERM] * k_norm_w[PERM][:, None]

    onesd = np.ones((128, 512), np.float32)
    onespd = np.zeros((128, 2), np.float32)
    onespd[0:64, 0] = 1.0    # col0: ones on q rows
    onespd[64:128, 1] = 1.0  # col1: ones on k rows
    vones = np.ones((128, HP * KB), np.float32)

    sel4 = np.zeros((128, 512), np.float32)
    for t in range(NT):
        sel4[32 * t, t * 128:t * 128 + 64] = 1.0
        sel4[32 * t + 1, t * 128 + 64:(t + 1) * 128] = 1.0
    ident = np.eye(128, dtype=np.float32)

    rows = np.concatenate([np.arange(h * HD, (h + 1) * HD) for h in heads])
    wp = proj_kernel[rows, :].astype(BF)

    return {"xT": xT, "wqk": wqk.astype(BF), "wv": wv.astype(BF),
            "bqk": bqk.astype(BF), "bv": bv.astype(BF),
            "cos2w": cos2w.astype(BF), "sinSw": sinSw.astype(BF),
            "sel4": sel4.astype(BF), "ident": ident.astype(BF),
            "wp": wp, "onesd": onesd.astype(BF), "onespd": onespd.astype(BF),
            "vones": vones.astype(BF)}


def kernel(x, rope_cos, rope_sin, qkv_kernel, qkv_bias, proj_kernel,
           proj_bias, q_norm_w, k_norm_w, _trace=False):
    args = [np.asarray(a, dtype=np.float32) for a in
            (x, rope_cos, rope_sin, qkv_kernel, qkv_bias, proj_kernel,
             proj_bias, q_norm_w, k_norm_w)]
    zb = (not np.any(args[4])) and True
    in_maps = [_prep_core_inputs(c, *args) for c in range(NCORES)]

    key = ("nc", zb)
    if key not in _NC_CACHE:
        _NC_CACHE[key] = build_nc(zero_bias=zb)
    nc = _NC_CACHE[key]

    res = run_bass_kernel_spmd(nc, in_maps, core_ids=list(range(NCORES)),
                               trace=_trace)
    parts = [np.asarray(res.results[c]["out"], dtype=np.float32)
             for c in range(NCORES)]
    out = np.empty((B, N, C), np.float32)
    pb = np.asarray(proj_bias, dtype=np.float32)
    for b in range(B):
        out[b] = parts[4 * b] + parts[4 * b + 1] + parts[4 * b + 2] + parts[4 * b + 3] + pb
    if _trace:
        kernel.last_results = res
    return out


# revision 14
# speedup vs baseline: 1.5009x; 1.0083x over previous
"""Multi-head attention (RMSNorm-QK + RoPE + softmax + proj) on 8 Trainium2 cores.

Sharding: core c handles batch b = c//4 and heads [3*(c%4), 3*(c%4)+3).
Each core computes qkv for its heads, flash-style attention, and a partial
projection over its heads' channels; the host sums the 4 partials per batch.

Layout tricks (bf16 data path, fp32 PSUM accumulation):
 - q^T/k^T layout [head_dim, tokens]; head-dim rows permuted so the RoPE
   half-swap is an intra-quadrant stream_shuffle.
 - RMS-norm: sum(q^2) via ones-pair matmul; rsqrt = exp(-0.5*ln(x)) so the
   whole kernel uses one ACT table set (natural_log_exp_and_others).
 - rsqrt scales broadcast across partitions on the (idle) GPSIMD engine.
 - softmax without max-subtraction (logits bounded by RMS norm); denominators
   via an appended ones-column in the PV matmul; 1/denom on DVE reciprocal.
 - projection partials DMA'd straight from PSUM to DRAM.
"""
import sys

for _p in ("/opt/trn_rl_repo", "/opt/trn_rl_repo/concourse"):
    if _p not in sys.path:
        sys.path.insert(0, _p)

import numpy as np
import ml_dtypes
from contextlib import ExitStack

import concourse.bass as bass
import concourse.tile as tile
import concourse.mybir as mybir
from concourse.bass_utils import run_bass_kernel_spmd

F32 = mybir.dt.float32
BF16 = mybir.dt.bfloat16
AF = mybir.ActivationFunctionType
BF = ml_dtypes.bfloat16

B, N, C = 2, 2048, 768
H, HD = 12, 64
HP = 3            # heads per core
NCORES = 8
CCH = C // 128    # 6 contraction chunks
NT = N // 512     # 4 token tiles of 512
KB = N // 128     # 16 k-blocks of 128
EPS = 1e-6

SWAP_MASK = [(i + 16) % 32 for i in range(32)]
# head-dim permutation: pair-exchange (d <-> d+32) becomes intra-quadrant
PERM = np.concatenate([np.arange(0, 16), np.arange(32, 48),
                       np.arange(16, 32), np.arange(48, 64)])
SIGN = np.where(PERM < 32, -1.0, 1.0).astype(np.float32)

_NC_CACHE = {}


def build_nc(split_waits=True, zero_bias=True):
    nc = bass.Bass(target_bir_lowering=True)
    xT = nc.declare_dram_parameter("xT", [C, N], BF16, isOutput=False)
    wqk = nc.declare_dram_parameter("wqk", [C, HP * 128], BF16, isOutput=False)
    wv = nc.declare_dram_parameter("wv", [C, 192], BF16, isOutput=False)
    bqk = nc.declare_dram_parameter("bqk", [1, HP * 128], BF16, isOutput=False)
    bv = nc.declare_dram_parameter("bv", [1, 192], BF16, isOutput=False)
    cos2w = nc.declare_dram_parameter("cos2w", [128, N], BF16, isOutput=False)
    sinSw = nc.declare_dram_parameter("sinSw", [128, N], BF16, isOutput=False)
    sel4 = nc.declare_dram_parameter("sel4", [128, 512], BF16, isOutput=False)
    ident = nc.declare_dram_parameter("ident", [128, 128], BF16, isOutput=False)
    wp = nc.declare_dram_parameter("wp", [HP * HD, C], BF16, isOutput=False)
    onesd = nc.declare_dram_parameter("onesd", [128, 512], BF16, isOutput=False)
    onespd = nc.declare_dram_parameter("onespd", [128, 2], BF16, isOutput=False)
    vones = nc.declare_dram_parameter("vones", [128, HP * KB], BF16, isOutput=False)
    out = nc.declare_dram_parameter("out", [N, C], BF16, isOutput=True)

    with tile.TileContext(nc) as tc, ExitStack() as ctx:
        sb = ctx.enter_context(tc.tile_pool(name="sb", bufs=1))
        tp = ctx.enter_context(tc.tile_pool(name="tp", bufs=2))
        pe = ctx.enter_context(tc.tile_pool(name="pe", bufs=4))   # pexp
        tp1 = ctx.enter_context(tc.tile_pool(name="tp1", bufs=2))
        fps = ctx.enter_context(tc.tile_pool(name="fps", bufs=2, space="PSUM"))
        sA = ctx.enter_context(tc.tile_pool(name="sA", bufs=1, space="PSUM"))
        sB = ctx.enter_context(tc.tile_pool(name="sB", bufs=1, space="PSUM"))
        oA = ctx.enter_context(tc.tile_pool(name="oA", bufs=1, space="PSUM"))
        oB = ctx.enter_context(tc.tile_pool(name="oB", bufs=1, space="PSUM"))

        # ---------- prologue: loads + consts ----------
        # Consolidated DMAs (one descriptor set each) to dodge per-DMA queue
        # overheads; SP carries x halves, ACT carries weights/tables, Pool
        # only runs memsets so the lead-in rope muls aren't queued behind DMA.
        xall = sb.tile([128, CCH * N], BF16, tag="xall")
        xs = [xall[:, c * N:(c + 1) * N] for c in range(CCH)]
        wqk_all = sb.tile([128, CCH * HP * 128], BF16, tag="wqk_all")
        wqk_sb = [wqk_all[:, c * HP * 128:(c + 1) * HP * 128]
                  for c in range(CCH)]
        wv_all = sb.tile([128, CCH * 192], BF16, tag="wv_all")
        wv_sb = [wv_all[:, c * 192:(c + 1) * 192] for c in range(CCH)]
        cos_sb = sb.tile([128, N], BF16, tag="cos")
        sin_sb = sb.tile([128, N], BF16, tag="sin")
        onesp = sb.tile([128, 2], BF16, tag="onesp")

        xsrc = xT[:, :].rearrange("(c p) n -> p c n", c=CCH)
        xdst = xall[:].rearrange("p (c n) -> p c n", c=CCH)
        nc.scalar.dma_start(
            wqk_all[:].rearrange("p (c n) -> p c n", c=CCH),
            wqk[:, :].rearrange("(c p) n -> p c n", c=CCH))
        zrow = sb.tile([1, 260], BF16, tag="zrow")
        nc.gpsimd.memset(zrow[:], 0.0)
        nc.sync.dma_start(xdst[:, :, 0:512], xsrc[:, :, 0:512])
        nc.scalar.dma_start(
            wv_all[:].rearrange("p (c n) -> p c n", c=CCH),
            wv[:, :].rearrange("(c p) n -> p c n", c=CCH))
        nc.scalar.dma_start(cos_sb[:], cos2w[:, :])
        nc.sync.dma_start(xdst[:, :, 512:1024], xsrc[:, :, 512:1024])
        nc.sync.dma_start(sin_sb[:], sinSw[:, :])
        nc.sync.dma_start(onesp[:], onespd[:, :])
        nc.sync.dma_start(xdst[:, :, 1024:1536], xsrc[:, :, 1024:1536])
        nc.sync.dma_start(xdst[:, :, 1536:2048], xsrc[:, :, 1536:2048])

        eps_t = sb.tile([128, 1], F32, tag="eps")
        nc.gpsimd.memset(eps_t[:], EPS)
        v3i = sb.tile([128, HP * KB * 65], BF16, tag="v3i")  # [v_h(kb) | 1] blocks
        nc.gpsimd.memset(
            v3i[:].rearrange("p (b n) -> p b n", n=65)[:, :, 64:65], 1.0)

        sel_sb = sb.tile([128, 512], BF16, tag="sel")
        nc.sync.dma_start(sel_sb[:], sel4[:, :])
        ident_sb = sb.tile([128, 128], BF16, tag="ident")
        nc.sync.dma_start(ident_sb[:], ident[:, :])
        bqk_sb = sb.tile([1, HP * 128], BF16, tag="bqk")
        bv_sb = sb.tile([1, 192], BF16, tag="bv")
        ones_row = sb.tile([1, 512], BF16, tag="ones_row")
        if not zero_bias:
            nc.sync.dma_start(bqk_sb[:], bqk[:, :])
            nc.sync.dma_start(bv_sb[:], bv[:, :])
            nc.sync.dma_start(ones_row[:], onesd[0:1, :])
        wp0_sb = sb.tile([128, C], BF16, tag="wp0")
        nc.scalar.dma_start(wp0_sb[:], wp[0:128, :])
        wp1_sb = sb.tile([64, C], BF16, tag="wp1")
        nc.scalar.dma_start(wp1_sb[:], wp[128:192, :])

        # qT/kT packed by head pairs so S-matmul operands share a base partition
        q12 = sb.tile([128, N], BF16, tag="q12")   # qT(0) rows 0:64, qT(1) rows 64:128
        k12 = sb.tile([128, N], BF16, tag="k12")
        q3 = sb.tile([64, N], BF16, tag="q3")
        k3 = sb.tile([64, N], BF16, tag="k3")

        def qT(h):
            return (q12[0:64], q12[64:128], q3[:])[h]

        def kT(h):
            return (k12[0:64], k12[64:128], k3[:])[h]

        oall_a = sb.tile([128, N], BF16, tag="oall_a")   # heads 0,1 O^T
        oall_b = sb.tile([64, N], BF16, tag="oall_b")    # head 2 O^T
        t4_all = sb.tile([128, N], BF16, tag="t4_all")
        s_sb = sb.tile([128, 512], F32, tag="s_sb")
        nc.gpsimd.memset(s_sb[:], 1.0)
        lnv = sb.tile([128, 512], F32, tag="lnv")
        sv = sb.tile([128, 512], BF16, tag="sv")

        def mm(out_ap, lhsT, rhs, start, stop):
            nc.tensor.matmul(out_ap, lhsT, rhs,
                             start=start, stop=stop, skip_group_check=True)

        # ---------- qkv for head h ----------
        def qkv_passA(h, t, ssb_on_act=False):
            ts = slice(t * 512, (t + 1) * 512)
            qk_ps = fps.tile([128, 512], F32, tag="flex")
            for c in range(CCH):
                mm(qk_ps[:], wqk_sb[c][:, h * 128:(h + 1) * 128],
                   xs[c][:, ts], c == 0, zero_bias and c == CCH - 1)
            if not zero_bias:
                mm(qk_ps[:], bqk_sb[:, h * 128:(h + 1) * 128], ones_row[:],
                   False, True)
            t2 = tp.tile([128, 512], F32, tag="t2")
            nc.vector.stream_shuffle(t2[:], qk_ps[:], SWAP_MASK)
            t1 = tp1.tile([128, 512], BF16, tag="t1")
            nc.vector.tensor_mul(t1[:], qk_ps[:], cos_sb[:, ts])
            sq = tp.tile([128, 512], BF16, tag="sq")
            nc.gpsimd.tensor_mul(sq[:], t2[:], t2[:])
            t3 = tp1.tile([128, 512], BF16, tag="t3")
            nc.gpsimd.tensor_mul(t3[:], t2[:], sin_sb[:, ts])
            sm_ps = fps.tile([2, 512], F32, tag="flex")
            mm(sm_ps[:], onesp[:], sq[:], True, True)
            nc.vector.tensor_copy(s_sb[32 * t:32 * t + 2, :], sm_ps[:])
            nc.vector.tensor_add(t4_all[:, ts], t1[:], t3[:])
            return sm_ps

        def finish_tile(h, t):
            ts = slice(t * 512, (t + 1) * 512)
            sqk_ps = fps.tile([128, 512], F32, tag="flex")
            mm(sqk_ps[:], sel_sb[:, t * 128:(t + 1) * 128], sv[:],
               True, True)
            nc.vector.tensor_mul(qT(h)[:, ts], t4_all[0:64, ts],
                                 sqk_ps[0:64, :])
            nc.vector.tensor_mul(kT(h)[:, ts], t4_all[64:128, ts],
                                 sqk_ps[64:128, :])

        def qkv_finish(h):
            nc.scalar.activation(lnv[:], s_sb[:], AF.Ln,
                                 bias=eps_t[:], scale=1.0 / HD)
            nc.scalar.activation(sv[:], lnv[:], AF.Exp, bias=0.0, scale=-0.5)
            for t in range(NT):
                finish_tile(h, t)

        def qkv_finish0_lnexp(t, sm_ps):
            # head 0 (lead-in): per-tile Ln/Exp so kT(0) completes early
            rows = slice(32 * t, 32 * t + 2)
            nc.scalar.activation(lnv[rows, :], s_sb[rows, :], AF.Ln,
                                 bias=eps_t[rows, :], scale=1.0 / HD)
            nc.scalar.activation(sv[rows, :], lnv[rows, :], AF.Exp,
                                 bias=0.0, scale=-0.5)

        def qkv(h):
            for t in range(NT):
                qkv_passA(h, t)
            qkv_finish(h)

        # ---------- v for all heads ----------
        def vphase_tt(tt):
            v_ps = fps.tile([128, 192], F32, tag="flex")
            for c in range(CCH):
                mm(v_ps[:], xs[c][:, tt * 128:(tt + 1) * 128], wv_sb[c][:],
                   c == 0, zero_bias and c == CCH - 1)
            if not zero_bias:
                mm(v_ps[:], ones_row[0:1, 0:128], bv_sb[:], False, True)
            # strided copy of 3 head-blocks into v3i (+ ones col at 64);
            # on DVE, NOT ACT: the in-order ACT queue would stall every
            # attention exp behind a straggling v-copy
            dst = v3i[:].rearrange("p (h k n) -> p h k n", h=HP, k=KB)
            nc.vector.tensor_copy(
                dst[:, :, tt, 0:64],
                v_ps[:, 0:192].rearrange("p (h n) -> p h n", h=HP))

        # ---------- attention ----------
        # 16 k-blocks in groups of 2 (one 2-bank PSUM tile per group)
        G2 = [(2 * g, 2 * g + 1) for g in range(8)]

        def epilogue(h, qt, o_ps, proj=False, tail=False):
            # o_ps: [128 q, 4*65] -- per q-block 128: 64 head-dims + denom col
            for qb in range(4):
                tt = qt * 4 + qb
                cs = slice(qb * 65, qb * 65 + 64)
                rec = tp1.tile([128, 1], F32, tag="rec")
                nc.vector.reciprocal(rec[:], o_ps[:, qb * 65 + 64:qb * 65 + 65])
                o_n = tp1.tile([128, 64], BF16, tag="o_n")
                nc.vector.tensor_scalar_mul(o_n[:], o_ps[:, cs], rec[:])
                tr_ps = fps.tile([64, 128], BF16, tag="flex")
                nc.tensor.transpose(tr_ps[:], o_n[:], ident_sb[:])
                if h < 2:
                    dst = oall_a[h * 64:(h + 1) * 64,
                                 tt * 128:(tt + 1) * 128]
                else:
                    dst = oall_b[:, tt * 128:(tt + 1) * 128]
                nc.vector.tensor_copy(dst, tr_ps[:])
                if proj:
                    proj_tt(tt, on_act=tail)

        def smm(spool, h, kbs, qs):
            s_ps = spool.tile([128, 1024], F32, tag="s")
            for j, kb in enumerate(kbs):
                mm(s_ps[:, j * 512:(j + 1) * 512],
                   kT(h)[:, kb * 128:(kb + 1) * 128], qT(h)[:, qs], True, True)
            return s_ps

        def pexp_of(s_ps):
            px = pe.tile([128, 1024], BF16, tag="pexp")
            nc.scalar.activation(px[:], s_ps[:], AF.Exp, bias=0.0, scale=0.125)
            return px

        def omm(o_ps, h, kbs, px):
            # o_ps was zeroed by memset; start=True would wipe the whole 2KB
            # psum zero-region, clobbering sibling q-blocks' accumulators.
            for j, kb in enumerate(kbs):
                for qb in range(4):
                    mm(o_ps[:, qb * 65:(qb + 1) * 65],
                       px[:, j * 512 + qb * 128:j * 512 + (qb + 1) * 128],
                       v3i[:, (h * KB + kb) * 65:(h * KB + kb) * 65 + 65],
                       False, False)

        # ---------- partial projection (one 128-token tile) ----------
        def proj_tt(tt, on_act=False):
            po = tp.tile([128, C], BF16, tag="po")
            for half in range(2):
                cs = slice(half * 384, (half + 1) * 384)
                p_ps = fps.tile([128, 384], F32, tag="flex")
                mm(p_ps[:], oall_a[:, tt * 128:(tt + 1) * 128],
                   wp0_sb[:, cs], True, False)
                mm(p_ps[:], oall_b[:, tt * 128:(tt + 1) * 128],
                   wp1_sb[:, cs], False, True)
                if on_act:  # tail: ACT is idle once the last exps drain
                    nc.scalar.activation(po[:, cs], p_ps[:],
                                         AF.Copy, bias=0.0, scale=1.0)
                else:
                    nc.vector.tensor_copy(po[:, cs], p_ps[:])
            nc.sync.dma_start(out[tt * 128:(tt + 1) * 128, :], po[:])

        # epilogues are software-pipelined: each qt's epilogue is emitted
        # after group 1 of the NEXT qt, so the next qt's S-matmuls are not
        # queued behind the epilogue's DVE->PE transpose chain.
        pending_epi = []

        def drain_epi():
            while pending_epi:
                epilogue(*pending_epi.pop(0))

        # S-matmuls run one group ahead of pexp/omm in the PE queue, so the
        # next group's S is computed while ACT works and ACT is never starved
        # behind an omm burst at the PE queue head. The in-flight group
        # carries across phase boundaries and is drained at the very end.
        pipe = []  # [(h, s_ps, kbs, o_ps)]

        def drain_pipe():
            while pipe:
                ph, ps, pk, po = pipe.pop(0)
                px = pexp_of(ps)
                omm(po, ph, pk, px)

        def attn_single(h, extra=None, proj=False, tail=False):
            for qt in range(NT):
                qs = slice(qt * 512, (qt + 1) * 512)
                o_ps = (oA if qt % 2 == 0 else oB).tile([128, 260], F32, tag="o")
                # zero via a tiny PE matmul: start=True wipes the psum
                # zero-region; keeps the zeroing off the busy DVE queue
                mm(o_ps[:], ident_sb[0:1, :], zrow[:], True, True)
                for g, kbs in enumerate(G2):
                    # prep at g==6: the 2-3 buffered pexp groups on the ACT
                    # queue absorb the prep matmul burst in the PE queue
                    if g == 6 and extra is not None:
                        extra(qt)
                    s_ps = smm(sA if g % 2 == 0 else sB, h, kbs, qs)
                    drain_pipe()
                    pipe.append((h, s_ps, kbs, o_ps))
                    if g == 1:
                        drain_epi()
                pending_epi.append((h, qt, o_ps, proj, tail and qt == NT - 1))

        def prep(h):
            def extra(qt):
                if qt == 0:
                    qkv_passA(h, 0)
                    qkv_passA(h, 1)
                elif qt == 1:
                    qkv_passA(h, 2)
                    qkv_passA(h, 3)
                elif qt == 2:
                    qkv_finish(h)
            return extra

        nc.gpsimd.memset(sv[:], 0.0)
        # warm up the PE p-state ramp during the initial DMA wait: ~9us of
        # junk matmuls so the real qkv matmuls start at full clock
        warm = oB.tile([128, 260], F32, tag="o")
        for _ in range(16):
            mm(warm[:], zrow[0:1, 0:128], zrow[0:1, :], True, True)
        for t in range(NT):
            smp = qkv_passA(0, t, ssb_on_act=True)
            qkv_finish0_lnexp(t, smp)
            for tt in range(4 * t, 4 * t + 4):
                vphase_tt(tt)
            finish_tile(0, t)
        attn_single(0, extra=prep(1))
        attn_single(1, extra=prep(2))
        attn_single(2, proj=True, tail=True)
        drain_pipe()
        drain_epi()

    if split_waits:
        _split_waits(nc)
    return nc


def _split_waits(nc):
    """This walrus build lowers at most one sync-wait per instruction (the
    matmul LDW struct rejects 2+). Move excess waits onto NoOps inserted
    just before, on the same engine queue — queues are in-order, so the
    constraint is preserved exactly."""
    k = 0
    for fn in nc.m.functions:
        for bb in fn.blocks:
            il = bb.instructions
            idx = 0
            while idx < len(il):
                inst = il[idx]
                si = inst.sync_info
                eng = getattr(inst, "engine", None)
                if (si is not None and len(si.on_wait) > 1
                        and eng is not None
                        and str(eng) != "EngineType.Unassigned"):
                    waits = list(si.on_wait)
                    inst.sync_info = mybir.SyncInfo(
                        on_wait=[waits[-1]], on_update=list(si.on_update))
                    for w in waits[:-1]:
                        nop = mybir.InstNoOp(
                            name=f"I-waitnop-{k}", engine=eng, ins=[], outs=[],
                            sync_info=mybir.SyncInfo(on_wait=[w], on_update=[]))
                        k += 1
                        il.insert(idx, nop)
                        idx += 1
                idx += 1


def _prep_core_inputs(core, x, rope_cos, rope_sin, qkv_kernel, qkv_bias,
                      proj_kernel, proj_bias, q_norm_w, k_norm_w):
    b = core // 4
    heads = [3 * (core % 4) + i for i in range(HP)]

    wq = qkv_kernel.reshape(C, 3, H, HD)
    bq = qkv_bias.reshape(3, H, HD)

    xT = np.ascontiguousarray(x[b].T).astype(BF)

    wqk = np.empty((C, HP * 128), np.float32)
    bqk = np.empty((1, HP * 128), np.float32)
    for i, h in enumerate(heads):
        wqk[:, i * 128:i * 128 + 64] = wq[:, 0, h, PERM]
        wqk[:, i * 128 + 64:(i + 1) * 128] = wq[:, 1, h, PERM]
        bqk[0, i * 128:i * 128 + 64] = bq[0, h, PERM]
        bqk[0, i * 128 + 64:(i + 1) * 128] = bq[1, h, PERM]

    wv = np.zeros((C, 192), np.float32)
    bv = np.zeros((1, 192), np.float32)
    for i, h in enumerate(heads):
        wv[:, i * 64:(i + 1) * 64] = wq[:, 2, h, :]
        bv[0, i * 64:(i + 1) * 64] = bq[2, h, :]

    cosT = rope_cos.T  # (HD, N)
    sinT = rope_sin.T
    cos2w = np.empty((128, N), np.float32)
    sinSw = np.empty((128, N), np.float32)
    cos2w[0:64] = cosT[PERM] * q_norm_w[PERM][:, None]
    cos2w[64:128] = cosT[PERM] * k_norm_w[PERM][:, None]
    sinSw[0:64] = SIGN[:, None] * sinT[PERM] * q_norm_w[PERM][:, None]
    sinSw[64:128] = SIGN[:, None] * sinT[PERM] * k_norm_w[PERM][:, None]

    onesd = np.ones((128, 512), np.float32)
    onespd = np.zeros((128, 2), np.float32)
    onespd[0:64, 0] = 1.0    # col0: ones on q rows
    onespd[64:128, 1] = 1.0  # col1: ones on k rows
    vones = np.ones((128, HP * KB), np.float32)

    sel4 = np.zeros((128, 512), np.float32)
    for t in range(NT):
        sel4[32 * t, t * 128:t * 128 + 64] = 1.0
        sel4[32 * t + 1, t * 128 + 64:(t + 1) * 128] = 1.0
    ident = np.eye(128, dtype=np.float32)

    rows = np.concatenate([np.arange(h * HD, (h + 1) * HD) for h in heads])
    wp = proj_kernel[rows, :].astype(BF)

    return {"xT": xT, "wqk": wqk.astype(BF), "wv": wv.astype(BF),
            "bqk": bqk.astype(BF), "bv": bv.astype(BF),
            "cos2w": cos2w.astype(BF), "sinSw": sinSw.astype(BF),
            "sel4": sel4.astype(BF), "ident": ident.astype(BF),
            "wp": wp, "onesd": onesd.astype(BF), "onespd": onespd.astype(BF),
            "vones": vones.astype(BF)}


def kernel(x, rope_cos, rope_sin, qkv_kernel, qkv_bias, proj_kernel,
           proj_bias, q_norm_w, k_norm_w, _trace=False):
    args = [np.asarray(a, dtype=np.float32) for a in
            (x, rope_cos, rope_sin, qkv_kernel, qkv_bias, proj_kernel,
             proj_bias, q_norm_w, k_norm_w)]
    zb = (not np.any(args[4])) and True
    in_maps = [_prep_core_inputs(c, *args) for c in range(NCORES)]

    key = ("nc", zb)
    if key not in _NC_CACHE:
        _NC_CACHE[key] = build_nc(zero_bias=zb)
    nc = _NC_CACHE[key]

    res = run_bass_kernel_spmd(nc, in_maps, core_ids=list(range(NCORES)),
                               trace=_trace)
    parts = [np.asarray(res.results[c]["out"], dtype=np.float32)
             for c in range(NCORES)]
    out = np.empty((B, N, C), np.float32)
    pb = np.asarray(proj_bias, dtype=np.float32)
    for b in range(B):
        out[b] = parts[4 * b] + parts[4 * b + 1] + parts[4 * b + 2] + parts[4 * b + 3] + pb
    if _trace:
        kernel.last_results = res
    return out


# revision 15
# speedup vs baseline: 1.5041x; 1.0021x over previous
"""Multi-head attention (RMSNorm-QK + RoPE + softmax + proj) on 8 Trainium2 cores.

Sharding: core c handles batch b = c//4 and heads [3*(c%4), 3*(c%4)+3).
Each core computes qkv for its heads, flash-style attention, and a partial
projection over its heads' channels; the host sums the 4 partials per batch.

Layout tricks (bf16 data path, fp32 PSUM accumulation):
 - q^T/k^T layout [head_dim, tokens]; head-dim rows permuted so the RoPE
   half-swap is an intra-quadrant stream_shuffle.
 - RMS-norm: sum(q^2) via ones-pair matmul; rsqrt = exp(-0.5*ln(x)) so the
   whole kernel uses one ACT table set (natural_log_exp_and_others).
 - rsqrt scales broadcast across partitions on the (idle) GPSIMD engine.
 - softmax without max-subtraction (logits bounded by RMS norm); denominators
   via an appended ones-column in the PV matmul; 1/denom on DVE reciprocal.
 - projection partials DMA'd straight from PSUM to DRAM.
"""
import sys

for _p in ("/opt/trn_rl_repo", "/opt/trn_rl_repo/concourse"):
    if _p not in sys.path:
        sys.path.insert(0, _p)

import numpy as np
import ml_dtypes
from contextlib import ExitStack

import concourse.bass as bass
import concourse.tile as tile
import concourse.mybir as mybir
from concourse.bass_utils import run_bass_kernel_spmd

F32 = mybir.dt.float32
BF16 = mybir.dt.bfloat16
AF = mybir.ActivationFunctionType
BF = ml_dtypes.bfloat16

B, N, C = 2, 2048, 768
H, HD = 12, 64
HP = 3            # heads per core
NCORES = 8
CCH = C // 128    # 6 contraction chunks
NT = N // 512     # 4 token tiles of 512
KB = N // 128     # 16 k-blocks of 128
EPS = 1e-6

SWAP_MASK = [(i + 16) % 32 for i in range(32)]
# head-dim permutation: pair-exchange (d <-> d+32) becomes intra-quadrant
PERM = np.concatenate([np.arange(0, 16), np.arange(32, 48),
                       np.arange(16, 32), np.arange(48, 64)])
SIGN = np.where(PERM < 32, -1.0, 1.0).astype(np.float32)

_NC_CACHE = {}


def build_nc(split_waits=True, zero_bias=True):
    nc = bass.Bass(target_bir_lowering=True)
    xT = nc.declare_dram_parameter("xT", [C, N], BF16, isOutput=False)
    wqk = nc.declare_dram_parameter("wqk", [C, HP * 128], BF16, isOutput=False)
    wv = nc.declare_dram_parameter("wv", [C, 192], BF16, isOutput=False)
    bqk = nc.declare_dram_parameter("bqk", [1, HP * 128], BF16, isOutput=False)
    bv = nc.declare_dram_parameter("bv", [1, 192], BF16, isOutput=False)
    cos2w = nc.declare_dram_parameter("cos2w", [128, N], BF16, isOutput=False)
    sinSw = nc.declare_dram_parameter("sinSw", [128, N], BF16, isOutput=False)
    sel4 = nc.declare_dram_parameter("sel4", [128, 512], BF16, isOutput=False)
    ident = nc.declare_dram_parameter("ident", [128, 128], BF16, isOutput=False)
    wp = nc.declare_dram_parameter("wp", [HP * HD, C], BF16, isOutput=False)
    onesd = nc.declare_dram_parameter("onesd", [128, 512], BF16, isOutput=False)
    onespd = nc.declare_dram_parameter("onespd", [128, 2], BF16, isOutput=False)
    vones = nc.declare_dram_parameter("vones", [128, HP * KB], BF16, isOutput=False)
    out = nc.declare_dram_parameter("out", [N, C], BF16, isOutput=True)

    with tile.TileContext(nc) as tc, ExitStack() as ctx:
        sb = ctx.enter_context(tc.tile_pool(name="sb", bufs=1))
        tp = ctx.enter_context(tc.tile_pool(name="tp", bufs=4))
        pe = ctx.enter_context(tc.tile_pool(name="pe", bufs=5))   # pexp
        tp1 = ctx.enter_context(tc.tile_pool(name="tp1", bufs=4))
        fps = ctx.enter_context(tc.tile_pool(name="fps", bufs=2, space="PSUM"))
        sA = ctx.enter_context(tc.tile_pool(name="sA", bufs=1, space="PSUM"))
        sB = ctx.enter_context(tc.tile_pool(name="sB", bufs=1, space="PSUM"))
        oA = ctx.enter_context(tc.tile_pool(name="oA", bufs=1, space="PSUM"))
        oB = ctx.enter_context(tc.tile_pool(name="oB", bufs=1, space="PSUM"))

        # ---------- prologue: loads + consts ----------
        # Consolidated DMAs (one descriptor set each) to dodge per-DMA queue
        # overheads; SP carries x halves, ACT carries weights/tables, Pool
        # only runs memsets so the lead-in rope muls aren't queued behind DMA.
        xall = sb.tile([128, CCH * N], BF16, tag="xall")
        xs = [xall[:, c * N:(c + 1) * N] for c in range(CCH)]
        wqk_all = sb.tile([128, CCH * HP * 128], BF16, tag="wqk_all")
        wqk_sb = [wqk_all[:, c * HP * 128:(c + 1) * HP * 128]
                  for c in range(CCH)]
        wv_all = sb.tile([128, CCH * 192], BF16, tag="wv_all")
        wv_sb = [wv_all[:, c * 192:(c + 1) * 192] for c in range(CCH)]
        cos_sb = sb.tile([128, N], BF16, tag="cos")
        sin_sb = sb.tile([128, N], BF16, tag="sin")
        onesp = sb.tile([128, 2], BF16, tag="onesp")

        xsrc = xT[:, :].rearrange("(c p) n -> p c n", c=CCH)
        xdst = xall[:].rearrange("p (c n) -> p c n", c=CCH)
        nc.scalar.dma_start(
            wqk_all[:].rearrange("p (c n) -> p c n", c=CCH),
            wqk[:, :].rearrange("(c p) n -> p c n", c=CCH))
        zrow = sb.tile([1, 260], BF16, tag="zrow")
        nc.gpsimd.memset(zrow[:], 0.0)
        nc.sync.dma_start(xdst[:, :, 0:512], xsrc[:, :, 0:512])
        nc.scalar.dma_start(
            wv_all[:].rearrange("p (c n) -> p c n", c=CCH),
            wv[:, :].rearrange("(c p) n -> p c n", c=CCH))
        nc.scalar.dma_start(cos_sb[:], cos2w[:, :])
        nc.sync.dma_start(xdst[:, :, 512:1024], xsrc[:, :, 512:1024])
        nc.sync.dma_start(sin_sb[:], sinSw[:, :])
        nc.sync.dma_start(onesp[:], onespd[:, :])
        nc.sync.dma_start(xdst[:, :, 1024:1536], xsrc[:, :, 1024:1536])
        nc.sync.dma_start(xdst[:, :, 1536:2048], xsrc[:, :, 1536:2048])

        eps_t = sb.tile([128, 1], F32, tag="eps")
        nc.gpsimd.memset(eps_t[:], EPS)
        v3i = sb.tile([128, HP * KB * 65], BF16, tag="v3i")  # [v_h(kb) | 1] blocks
        nc.gpsimd.memset(
            v3i[:].rearrange("p (b n) -> p b n", n=65)[:, :, 64:65], 1.0)

        sel_sb = sb.tile([128, 512], BF16, tag="sel")
        nc.sync.dma_start(sel_sb[:], sel4[:, :])
        ident_sb = sb.tile([128, 128], BF16, tag="ident")
        nc.sync.dma_start(ident_sb[:], ident[:, :])
        bqk_sb = sb.tile([1, HP * 128], BF16, tag="bqk")
        bv_sb = sb.tile([1, 192], BF16, tag="bv")
        ones_row = sb.tile([1, 512], BF16, tag="ones_row")
        if not zero_bias:
            nc.sync.dma_start(bqk_sb[:], bqk[:, :])
            nc.sync.dma_start(bv_sb[:], bv[:, :])
            nc.sync.dma_start(ones_row[:], onesd[0:1, :])
        wp0_sb = sb.tile([128, C], BF16, tag="wp0")
        nc.scalar.dma_start(wp0_sb[:], wp[0:128, :])
        wp1_sb = sb.tile([64, C], BF16, tag="wp1")
        nc.scalar.dma_start(wp1_sb[:], wp[128:192, :])

        # qT/kT packed by head pairs so S-matmul operands share a base partition
        q12 = sb.tile([128, N], BF16, tag="q12")   # qT(0) rows 0:64, qT(1) rows 64:128
        k12 = sb.tile([128, N], BF16, tag="k12")
        q3 = sb.tile([64, N], BF16, tag="q3")
        k3 = sb.tile([64, N], BF16, tag="k3")

        def qT(h):
            return (q12[0:64], q12[64:128], q3[:])[h]

        def kT(h):
            return (k12[0:64], k12[64:128], k3[:])[h]

        oall_a = sb.tile([128, N], BF16, tag="oall_a")   # heads 0,1 O^T
        oall_b = sb.tile([64, N], BF16, tag="oall_b")    # head 2 O^T
        t4_all = sb.tile([128, N], BF16, tag="t4_all")
        s_sb = sb.tile([128, 512], F32, tag="s_sb")
        nc.gpsimd.memset(s_sb[:], 1.0)
        lnv = sb.tile([128, 512], F32, tag="lnv")
        sv = sb.tile([128, 512], BF16, tag="sv")

        def mm(out_ap, lhsT, rhs, start, stop):
            nc.tensor.matmul(out_ap, lhsT, rhs,
                             start=start, stop=stop, skip_group_check=True)

        # ---------- qkv for head h ----------
        def qkv_passA(h, t, ssb_on_act=False):
            ts = slice(t * 512, (t + 1) * 512)
            qk_ps = fps.tile([128, 512], F32, tag="flex")
            for c in range(CCH):
                mm(qk_ps[:], wqk_sb[c][:, h * 128:(h + 1) * 128],
                   xs[c][:, ts], c == 0, zero_bias and c == CCH - 1)
            if not zero_bias:
                mm(qk_ps[:], bqk_sb[:, h * 128:(h + 1) * 128], ones_row[:],
                   False, True)
            t2 = tp.tile([128, 512], F32, tag="t2")
            nc.vector.stream_shuffle(t2[:], qk_ps[:], SWAP_MASK)
            t1 = tp1.tile([128, 512], BF16, tag="t1")
            nc.vector.tensor_mul(t1[:], qk_ps[:], cos_sb[:, ts])
            sq = tp.tile([128, 512], BF16, tag="sq")
            nc.gpsimd.tensor_mul(sq[:], t2[:], t2[:])
            t3 = tp1.tile([128, 512], BF16, tag="t3")
            nc.gpsimd.tensor_mul(t3[:], t2[:], sin_sb[:, ts])
            sm_ps = fps.tile([2, 512], F32, tag="flex")
            mm(sm_ps[:], onesp[:], sq[:], True, True)
            nc.vector.tensor_copy(s_sb[32 * t:32 * t + 2, :], sm_ps[:])
            nc.vector.tensor_add(t4_all[:, ts], t1[:], t3[:])
            return sm_ps

        def finish_tile(h, t):
            ts = slice(t * 512, (t + 1) * 512)
            sqk_ps = fps.tile([128, 512], F32, tag="flex")
            mm(sqk_ps[:], sel_sb[:, t * 128:(t + 1) * 128], sv[:],
               True, True)
            nc.vector.tensor_mul(qT(h)[:, ts], t4_all[0:64, ts],
                                 sqk_ps[0:64, :])
            nc.vector.tensor_mul(kT(h)[:, ts], t4_all[64:128, ts],
                                 sqk_ps[64:128, :])

        def qkv_finish(h):
            nc.scalar.activation(lnv[:], s_sb[:], AF.Ln,
                                 bias=eps_t[:], scale=1.0 / HD)
            nc.scalar.activation(sv[:], lnv[:], AF.Exp, bias=0.0, scale=-0.5)
            for t in range(NT):
                finish_tile(h, t)

        def qkv_finish0_lnexp(t, sm_ps):
            # head 0 (lead-in): per-tile Ln/Exp so kT(0) completes early
            rows = slice(32 * t, 32 * t + 2)
            nc.scalar.activation(lnv[rows, :], s_sb[rows, :], AF.Ln,
                                 bias=eps_t[rows, :], scale=1.0 / HD)
            nc.scalar.activation(sv[rows, :], lnv[rows, :], AF.Exp,
                                 bias=0.0, scale=-0.5)

        def qkv(h):
            for t in range(NT):
                qkv_passA(h, t)
            qkv_finish(h)

        # ---------- v for all heads ----------
        def vphase_tt(tt):
            v_ps = fps.tile([128, 192], F32, tag="flex")
            for c in range(CCH):
                mm(v_ps[:], xs[c][:, tt * 128:(tt + 1) * 128], wv_sb[c][:],
                   c == 0, zero_bias and c == CCH - 1)
            if not zero_bias:
                mm(v_ps[:], ones_row[0:1, 0:128], bv_sb[:], False, True)
            # strided copy of 3 head-blocks into v3i (+ ones col at 64);
            # on DVE, NOT ACT: the in-order ACT queue would stall every
            # attention exp behind a straggling v-copy
            dst = v3i[:].rearrange("p (h k n) -> p h k n", h=HP, k=KB)
            nc.vector.tensor_copy(
                dst[:, :, tt, 0:64],
                v_ps[:, 0:192].rearrange("p (h n) -> p h n", h=HP))

        # ---------- attention ----------
        # 16 k-blocks in groups of 2 (one 2-bank PSUM tile per group)
        G2 = [(2 * g, 2 * g + 1) for g in range(8)]

        def epilogue(h, qt, o_ps, proj=False, tail=False):
            # o_ps: [128 q, 4*65] -- per q-block 128: 64 head-dims + denom col
            for qb in range(4):
                tt = qt * 4 + qb
                cs = slice(qb * 65, qb * 65 + 64)
                rec = tp1.tile([128, 1], F32, tag="rec")
                nc.vector.reciprocal(rec[:], o_ps[:, qb * 65 + 64:qb * 65 + 65])
                o_n = tp1.tile([128, 64], BF16, tag="o_n")
                nc.vector.tensor_scalar_mul(o_n[:], o_ps[:, cs], rec[:])
                tr_ps = fps.tile([64, 128], BF16, tag="flex")
                nc.tensor.transpose(tr_ps[:], o_n[:], ident_sb[:])
                if h < 2:
                    dst = oall_a[h * 64:(h + 1) * 64,
                                 tt * 128:(tt + 1) * 128]
                else:
                    dst = oall_b[:, tt * 128:(tt + 1) * 128]
                nc.vector.tensor_copy(dst, tr_ps[:])
                if proj:
                    proj_tt(tt, on_act=tail)

        def smm(spool, h, kbs, qs):
            s_ps = spool.tile([128, 1024], F32, tag="s")
            for j, kb in enumerate(kbs):
                mm(s_ps[:, j * 512:(j + 1) * 512],
                   kT(h)[:, kb * 128:(kb + 1) * 128], qT(h)[:, qs], True, True)
            return s_ps

        def pexp_of(s_ps):
            px = pe.tile([128, 1024], BF16, tag="pexp")
            nc.scalar.activation(px[:], s_ps[:], AF.Exp, bias=0.0, scale=0.125)
            return px

        def omm(o_ps, h, kbs, px):
            # o_ps was zeroed by memset; start=True would wipe the whole 2KB
            # psum zero-region, clobbering sibling q-blocks' accumulators.
            for j, kb in enumerate(kbs):
                for qb in range(4):
                    mm(o_ps[:, qb * 65:(qb + 1) * 65],
                       px[:, j * 512 + qb * 128:j * 512 + (qb + 1) * 128],
                       v3i[:, (h * KB + kb) * 65:(h * KB + kb) * 65 + 65],
                       False, False)

        # ---------- partial projection (one 128-token tile) ----------
        def proj_tt(tt, on_act=False):
            po = tp.tile([128, C], BF16, tag="po")
            for half in range(2):
                cs = slice(half * 384, (half + 1) * 384)
                p_ps = fps.tile([128, 384], F32, tag="flex")
                mm(p_ps[:], oall_a[:, tt * 128:(tt + 1) * 128],
                   wp0_sb[:, cs], True, False)
                mm(p_ps[:], oall_b[:, tt * 128:(tt + 1) * 128],
                   wp1_sb[:, cs], False, True)
                if on_act:  # tail: ACT is idle once the last exps drain
                    nc.scalar.activation(po[:, cs], p_ps[:],
                                         AF.Copy, bias=0.0, scale=1.0)
                else:
                    nc.vector.tensor_copy(po[:, cs], p_ps[:])
            nc.sync.dma_start(out[tt * 128:(tt + 1) * 128, :], po[:])

        # epilogues are software-pipelined: each qt's epilogue is emitted
        # after group 1 of the NEXT qt, so the next qt's S-matmuls are not
        # queued behind the epilogue's DVE->PE transpose chain.
        pending_epi = []

        def drain_epi():
            while pending_epi:
                epilogue(*pending_epi.pop(0))

        # S-matmuls run one group ahead of pexp/omm in the PE queue, so the
        # next group's S is computed while ACT works and ACT is never starved
        # behind an omm burst at the PE queue head. The in-flight group
        # carries across phase boundaries and is drained at the very end.
        pipe = []  # [(h, s_ps, kbs, o_ps)]

        def drain_pipe():
            while pipe:
                ph, ps, pk, po = pipe.pop(0)
                px = pexp_of(ps)
                omm(po, ph, pk, px)

        def attn_single(h, extra=None, proj=False, tail=False):
            for qt in range(NT):
                qs = slice(qt * 512, (qt + 1) * 512)
                o_ps = (oA if qt % 2 == 0 else oB).tile([128, 260], F32, tag="o")
                # zero via a tiny PE matmul: start=True wipes the psum
                # zero-region; keeps the zeroing off the busy DVE queue
                mm(o_ps[:], ident_sb[0:1, :], zrow[:], True, True)
                for g, kbs in enumerate(G2):
                    # prep at g==6: the 2-3 buffered pexp groups on the ACT
                    # queue absorb the prep matmul burst in the PE queue
                    if g == 6 and extra is not None:
                        extra(qt)
                    s_ps = smm(sA if g % 2 == 0 else sB, h, kbs, qs)
                    drain_pipe()
                    pipe.append((h, s_ps, kbs, o_ps))
                    if g == 1:
                        drain_epi()
                pending_epi.append((h, qt, o_ps, proj, tail and qt == NT - 1))

        def prep(h):
            def extra(qt):
                if qt == 0:
                    qkv_passA(h, 0)
                    qkv_passA(h, 1)
                elif qt == 1:
                    qkv_passA(h, 2)
                    qkv_passA(h, 3)
                elif qt == 2:
                    qkv_finish(h)
            return extra

        nc.gpsimd.memset(sv[:], 0.0)
        # warm up the PE p-state ramp during the initial DMA wait: ~9us of
        # junk matmuls so the real qkv matmuls start at full clock
        warm = oB.tile([128, 260], F32, tag="o")
        for _ in range(16):
            mm(warm[:], zrow[0:1, 0:128], zrow[0:1, :], True, True)
        for t in range(NT):
            smp = qkv_passA(0, t, ssb_on_act=True)
            qkv_finish0_lnexp(t, smp)
            for tt in range(4 * t, 4 * t + 4):
                vphase_tt(tt)
            finish_tile(0, t)
        attn_single(0, extra=prep(1))
        attn_single(1, extra=prep(2))
        attn_single(2, proj=True, tail=True)
        drain_pipe()
        drain_epi()

    if split_waits:
        _split_waits(nc)
    return nc


def _split_waits(nc):
    """This walrus build lowers at most one sync-wait per instruction (the
    matmul LDW struct rejects 2+). Move excess waits onto NoOps inserted
    just before, on the same engine queue — queues are in-order, so the
    constraint is preserved exactly."""
    k = 0
    for fn in nc.m.functions:
        for bb in fn.blocks:
            il = bb.instructions
            idx = 0
            while idx < len(il):
                inst = il[idx]
                si = inst.sync_info
                eng = getattr(inst, "engine", None)
                if (si is not None and len(si.on_wait) > 1
                        and eng is not None
                        and str(eng) != "EngineType.Unassigned"):
                    waits = list(si.on_wait)
                    inst.sync_info = mybir.SyncInfo(
                        on_wait=[waits[-1]], on_update=list(si.on_update))
                    for w in waits[:-1]:
                        nop = mybir.InstNoOp(
                            name=f"I-waitnop-{k}", engine=eng, ins=[], outs=[],
                            sync_info=mybir.SyncInfo(on_wait=[w], on_update=[]))
                        k += 1
                        il.insert(idx, nop)
                        idx += 1
                idx += 1


def _prep_core_inputs(core, x, rope_cos, rope_sin, qkv_kernel, qkv_bias,
                      proj_kernel, proj_bias, q_norm_w, k_norm_w):
    b = core // 4
    heads = [3 * (core % 4) + i for i in range(HP)]

    wq = qkv_kernel.reshape(C, 3, H, HD)
    bq = qkv_bias.reshape(3, H, HD)

    xT = np.ascontiguousarray(x[b].T).astype(BF)

    wqk = np.empty((C, HP * 128), np.float32)
    bqk = np.empty((1, HP * 128), np.float32)
    for i, h in enumerate(heads):
        wqk[:, i * 128:i * 128 + 64] = wq[:, 0, h, PERM]
        wqk[:, i * 128 + 64:(i + 1) * 128] = wq[:, 1, h, PERM]
        bqk[0, i * 128:i * 128 + 64] = bq[0, h, PERM]
        bqk[0, i * 128 + 64:(i + 1) * 128] = bq[1, h, PERM]

    wv = np.zeros((C, 192), np.float32)
    bv = np.zeros((1, 192), np.float32)
    for i, h in enumerate(heads):
        wv[:, i * 64:(i + 1) * 64] = wq[:, 2, h, :]
        bv[0, i * 64:(i + 1) * 64] = bq[2, h, :]

    cosT = rope_cos.T  # (HD, N)
    sinT = rope_sin.T
    cos2w = np.empty((128, N), np.float32)
    sinSw = np.empty((128, N), np.float32)
    cos2w[0:64] = cosT[PERM] * q_norm_w[PERM][:, None]
    cos2w[64:128] = cosT[PERM] * k_norm_w[PERM][:, None]
    sinSw[0:64] = SIGN[:, None] * sinT[PERM] * q_norm_w[PERM][:, None]
    sinSw[64:128] = SIGN[:, None] * sinT[PERM] * k_norm_w[PERM][:, None]

    onesd = np.ones((128, 512), np.float32)
    onespd = np.zeros((128, 2), np.float32)
    onespd[0:64, 0] = 1.0    # col0: ones on q rows
    onespd[64:128, 1] = 1.0  # col1: ones on k rows
    vones = np.ones((128, HP * KB), np.float32)

    sel4 = np.zeros((128, 512), np.float32)
    for t in range(NT):
        sel4[32 * t, t * 128:t * 128 + 64] = 1.0
        sel4[32 * t + 1, t * 128 + 64:(t + 1) * 128] = 1.0
    ident = np.eye(128, dtype=np.float32)

    rows = np.concatenate([np.arange(h * HD, (h + 1) * HD) for h in heads])
    wp = proj_kernel[rows, :].astype(BF)

    return {"xT": xT, "wqk": wqk.astype(BF), "wv": wv.astype(BF),
            "bqk": bqk.astype(BF), "bv": bv.astype(BF),
            "cos2w": cos2w.astype(BF), "sinSw": sinSw.astype(BF),
            "sel4": sel4.astype(BF), "ident": ident.astype(BF),
            "wp": wp, "onesd": onesd.astype(BF), "onespd": onespd.astype(BF),
            "vones": vones.astype(BF)}


def kernel(x, rope_cos, rope_sin, qkv_kernel, qkv_bias, proj_kernel,
           proj_bias, q_norm_w, k_norm_w, _trace=False):
    args = [np.asarray(a, dtype=np.float32) for a in
            (x, rope_cos, rope_sin, qkv_kernel, qkv_bias, proj_kernel,
             proj_bias, q_norm_w, k_norm_w)]
    zb = (not np.any(args[4])) and True
    in_maps = [_prep_core_inputs(c, *args) for c in range(NCORES)]

    key = ("nc", zb)
    if key not in _NC_CACHE:
        _NC_CACHE[key] = build_nc(zero_bias=zb)
    nc = _NC_CACHE[key]

    res = run_bass_kernel_spmd(nc, in_maps, core_ids=list(range(NCORES)),
                               trace=_trace)
    parts = [np.asarray(res.results[c]["out"], dtype=np.float32)
             for c in range(NCORES)]
    out = np.empty((B, N, C), np.float32)
    pb = np.asarray(proj_bias, dtype=np.float32)
    for b in range(B):
        out[b] = parts[4 * b] + parts[4 * b + 1] + parts[4 * b + 2] + parts[4 * b + 3] + pb
    if _trace:
        kernel.last_results = res
    return out


# revision 16
# speedup vs baseline: 1.5069x; 1.0019x over previous
"""Multi-head attention (RMSNorm-QK + RoPE + softmax + proj) on 8 Trainium2 cores.

Sharding: core c handles batch b = c//4 and heads [3*(c%4), 3*(c%4)+3).
Each core computes qkv for its heads, flash-style attention, and a partial
projection over its heads' channels; the host sums the 4 partials per batch.

Layout tricks (bf16 data path, fp32 PSUM accumulation):
 - q^T/k^T layout [head_dim, tokens]; head-dim rows permuted so the RoPE
   half-swap is an intra-quadrant stream_shuffle.
 - RMS-norm: sum(q^2) via ones-pair matmul; rsqrt = exp(-0.5*ln(x)) so the
   whole kernel uses one ACT table set (natural_log_exp_and_others).
 - rsqrt scales broadcast across partitions on the (idle) GPSIMD engine.
 - softmax without max-subtraction (logits bounded by RMS norm); denominators
   via an appended ones-column in the PV matmul; 1/denom on DVE reciprocal.
 - projection partials DMA'd straight from PSUM to DRAM.
"""
import sys

for _p in ("/opt/trn_rl_repo", "/opt/trn_rl_repo/concourse"):
    if _p not in sys.path:
        sys.path.insert(0, _p)

import numpy as np
import ml_dtypes
from contextlib import ExitStack

import concourse.bass as bass
import concourse.tile as tile
import concourse.mybir as mybir
from concourse.bass_utils import run_bass_kernel_spmd

F32 = mybir.dt.float32
BF16 = mybir.dt.bfloat16
AF = mybir.ActivationFunctionType
BF = ml_dtypes.bfloat16

B, N, C = 2, 2048, 768
H, HD = 12, 64
HP = 3            # heads per core
NCORES = 8
CCH = C // 128    # 6 contraction chunks
NT = N // 512     # 4 token tiles of 512
KB = N // 128     # 16 k-blocks of 128
EPS = 1e-6

SWAP_MASK = [(i + 16) % 32 for i in range(32)]
# head-dim permutation: pair-exchange (d <-> d+32) becomes intra-quadrant
PERM = np.concatenate([np.arange(0, 16), np.arange(32, 48),
                       np.arange(16, 32), np.arange(48, 64)])
SIGN = np.where(PERM < 32, -1.0, 1.0).astype(np.float32)

_NC_CACHE = {}


def build_nc(split_waits=True, zero_bias=True):
    nc = bass.Bass(target_bir_lowering=True)
    xT = nc.declare_dram_parameter("xT", [C, N], BF16, isOutput=False)
    wqk = nc.declare_dram_parameter("wqk", [C, HP * 128], BF16, isOutput=False)
    wv = nc.declare_dram_parameter("wv", [C, 192], BF16, isOutput=False)
    bqk = nc.declare_dram_parameter("bqk", [1, HP * 128], BF16, isOutput=False)
    bv = nc.declare_dram_parameter("bv", [1, 192], BF16, isOutput=False)
    cos2w = nc.declare_dram_parameter("cos2w", [128, N], BF16, isOutput=False)
    sinSw = nc.declare_dram_parameter("sinSw", [128, N], BF16, isOutput=False)
    sel4 = nc.declare_dram_parameter("sel4", [128, 512], BF16, isOutput=False)
    ident = nc.declare_dram_parameter("ident", [128, 128], BF16, isOutput=False)
    wp = nc.declare_dram_parameter("wp", [HP * HD, C], BF16, isOutput=False)
    onesd = nc.declare_dram_parameter("onesd", [128, 512], BF16, isOutput=False)
    onespd = nc.declare_dram_parameter("onespd", [128, 2], BF16, isOutput=False)
    vones = nc.declare_dram_parameter("vones", [128, HP * KB], BF16, isOutput=False)
    out = nc.declare_dram_parameter("out", [N, C], BF16, isOutput=True)

    with tile.TileContext(nc) as tc, ExitStack() as ctx:
        sb = ctx.enter_context(tc.tile_pool(name="sb", bufs=1))
        tp = ctx.enter_context(tc.tile_pool(name="tp", bufs=4))
        pe = ctx.enter_context(tc.tile_pool(name="pe", bufs=5))   # pexp
        tp1 = ctx.enter_context(tc.tile_pool(name="tp1", bufs=4))
        fps = ctx.enter_context(tc.tile_pool(name="fps", bufs=2, space="PSUM"))
        sA = ctx.enter_context(tc.tile_pool(name="sA", bufs=1, space="PSUM"))
        sB = ctx.enter_context(tc.tile_pool(name="sB", bufs=1, space="PSUM"))
        oA = ctx.enter_context(tc.tile_pool(name="oA", bufs=1, space="PSUM"))
        oB = ctx.enter_context(tc.tile_pool(name="oB", bufs=1, space="PSUM"))

        # ---------- prologue: loads + consts ----------
        # Consolidated DMAs (one descriptor set each) to dodge per-DMA queue
        # overheads; SP carries x halves, ACT carries weights/tables, Pool
        # only runs memsets so the lead-in rope muls aren't queued behind DMA.
        xall = sb.tile([128, CCH * N], BF16, tag="xall")
        xs = [xall[:, c * N:(c + 1) * N] for c in range(CCH)]
        wqk_all = sb.tile([128, CCH * HP * 128], BF16, tag="wqk_all")
        wqk_sb = [wqk_all[:, c * HP * 128:(c + 1) * HP * 128]
                  for c in range(CCH)]
        wv_all = sb.tile([128, CCH * 192], BF16, tag="wv_all")
        wv_sb = [wv_all[:, c * 192:(c + 1) * 192] for c in range(CCH)]
        cos_sb = sb.tile([128, N], BF16, tag="cos")
        sin_sb = sb.tile([128, N], BF16, tag="sin")
        onesp = sb.tile([128, 2], BF16, tag="onesp")

        xsrc = xT[:, :].rearrange("(c p) n -> p c n", c=CCH)
        xdst = xall[:].rearrange("p (c n) -> p c n", c=CCH)
        nc.scalar.dma_start(
            wqk_all[:].rearrange("p (c n) -> p c n", c=CCH),
            wqk[:, :].rearrange("(c p) n -> p c n", c=CCH))
        zrow = sb.tile([1, 260], BF16, tag="zrow")
        nc.gpsimd.memset(zrow[:], 0.0)
        nc.sync.dma_start(xdst[:, :, 0:512], xsrc[:, :, 0:512])
        nc.scalar.dma_start(
            wv_all[:].rearrange("p (c n) -> p c n", c=CCH),
            wv[:, :].rearrange("(c p) n -> p c n", c=CCH))
        nc.scalar.dma_start(cos_sb[:], cos2w[:, :])
        nc.sync.dma_start(xdst[:, :, 512:1024], xsrc[:, :, 512:1024])
        nc.sync.dma_start(sin_sb[:], sinSw[:, :])
        nc.sync.dma_start(onesp[:], onespd[:, :])
        nc.sync.dma_start(xdst[:, :, 1024:1536], xsrc[:, :, 1024:1536])
        nc.sync.dma_start(xdst[:, :, 1536:2048], xsrc[:, :, 1536:2048])

        eps_t = sb.tile([128, 1], F32, tag="eps")
        nc.gpsimd.memset(eps_t[:], EPS)
        v3i = sb.tile([128, HP * KB * 65], BF16, tag="v3i")  # [v_h(kb) | 1] blocks
        nc.gpsimd.memset(
            v3i[:].rearrange("p (b n) -> p b n", n=65)[:, :, 64:65], 1.0)

        sel_sb = sb.tile([128, 512], BF16, tag="sel")
        nc.sync.dma_start(sel_sb[:], sel4[:, :])
        ident_sb = sb.tile([128, 128], BF16, tag="ident")
        nc.sync.dma_start(ident_sb[:], ident[:, :])
        bqk_sb = sb.tile([1, HP * 128], BF16, tag="bqk")
        bv_sb = sb.tile([1, 192], BF16, tag="bv")
        ones_row = sb.tile([1, 512], BF16, tag="ones_row")
        if not zero_bias:
            nc.sync.dma_start(bqk_sb[:], bqk[:, :])
            nc.sync.dma_start(bv_sb[:], bv[:, :])
            nc.sync.dma_start(ones_row[:], onesd[0:1, :])
        wp0_sb = sb.tile([128, C], BF16, tag="wp0")
        nc.scalar.dma_start(wp0_sb[:], wp[0:128, :])
        wp1_sb = sb.tile([64, C], BF16, tag="wp1")
        nc.scalar.dma_start(wp1_sb[:], wp[128:192, :])

        # qT/kT packed by head pairs so S-matmul operands share a base partition
        q12 = sb.tile([128, N], BF16, tag="q12")   # qT(0) rows 0:64, qT(1) rows 64:128
        k12 = sb.tile([128, N], BF16, tag="k12")
        q3 = sb.tile([64, N], BF16, tag="q3")
        k3 = sb.tile([64, N], BF16, tag="k3")

        def qT(h):
            return (q12[0:64], q12[64:128], q3[:])[h]

        def kT(h):
            return (k12[0:64], k12[64:128], k3[:])[h]

        oall_a = sb.tile([128, N], BF16, tag="oall_a")   # heads 0,1 O^T
        oall_b = sb.tile([64, N], BF16, tag="oall_b")    # head 2 O^T
        t4_all = sb.tile([128, N], BF16, tag="t4_all")
        s_sb = sb.tile([128, 512], F32, tag="s_sb")
        nc.gpsimd.memset(s_sb[:], 1.0)
        lnv = sb.tile([128, 512], F32, tag="lnv")
        sv = sb.tile([128, 512], BF16, tag="sv")

        def mm(out_ap, lhsT, rhs, start, stop):
            nc.tensor.matmul(out_ap, lhsT, rhs,
                             start=start, stop=stop, skip_group_check=True)

        # ---------- qkv for head h ----------
        def qkv_passA(h, t, ssb_on_act=False):
            ts = slice(t * 512, (t + 1) * 512)
            qk_ps = fps.tile([128, 512], F32, tag="flex")
            for c in range(CCH):
                mm(qk_ps[:], wqk_sb[c][:, h * 128:(h + 1) * 128],
                   xs[c][:, ts], c == 0, zero_bias and c == CCH - 1)
            if not zero_bias:
                mm(qk_ps[:], bqk_sb[:, h * 128:(h + 1) * 128], ones_row[:],
                   False, True)
            t2 = tp.tile([128, 512], F32, tag="t2")
            nc.vector.stream_shuffle(t2[:], qk_ps[:], SWAP_MASK)
            t1 = tp1.tile([128, 512], BF16, tag="t1")
            nc.vector.tensor_mul(t1[:], qk_ps[:], cos_sb[:, ts])
            sq = tp.tile([128, 512], BF16, tag="sq")
            nc.gpsimd.tensor_mul(sq[:], t2[:], t2[:])
            t3 = tp1.tile([128, 512], BF16, tag="t3")
            nc.gpsimd.tensor_mul(t3[:], t2[:], sin_sb[:, ts])
            sm_ps = fps.tile([2, 512], F32, tag="flex")
            mm(sm_ps[:], onesp[:], sq[:], True, True)
            nc.vector.tensor_copy(s_sb[32 * t:32 * t + 2, :], sm_ps[:])
            nc.vector.tensor_add(t4_all[:, ts], t1[:], t3[:])
            return sm_ps

        def finish_tile(h, t):
            ts = slice(t * 512, (t + 1) * 512)
            sqk_ps = fps.tile([128, 512], F32, tag="flex")
            mm(sqk_ps[:], sel_sb[:, t * 128:(t + 1) * 128], sv[:],
               True, True)
            nc.vector.tensor_mul(qT(h)[:, ts], t4_all[0:64, ts],
                                 sqk_ps[0:64, :])
            nc.vector.tensor_mul(kT(h)[:, ts], t4_all[64:128, ts],
                                 sqk_ps[64:128, :])

        def qkv_finish(h):
            nc.scalar.activation(lnv[:], s_sb[:], AF.Ln,
                                 bias=eps_t[:], scale=1.0 / HD)
            nc.scalar.activation(sv[:], lnv[:], AF.Exp, bias=0.0, scale=-0.5)
            for t in range(NT):
                finish_tile(h, t)

        def qkv_finish0_lnexp(t, sm_ps):
            # head 0 (lead-in): per-tile Ln/Exp so kT(0) completes early
            rows = slice(32 * t, 32 * t + 2)
            nc.scalar.activation(lnv[rows, :], s_sb[rows, :], AF.Ln,
                                 bias=eps_t[rows, :], scale=1.0 / HD)
            nc.scalar.activation(sv[rows, :], lnv[rows, :], AF.Exp,
                                 bias=0.0, scale=-0.5)

        def qkv(h):
            for t in range(NT):
                qkv_passA(h, t)
            qkv_finish(h)

        # ---------- v for all heads ----------
        def vphase_tt(tt):
            v_ps = fps.tile([128, 192], F32, tag="flex")
            for c in range(CCH):
                mm(v_ps[:], xs[c][:, tt * 128:(tt + 1) * 128], wv_sb[c][:],
                   c == 0, zero_bias and c == CCH - 1)
            if not zero_bias:
                mm(v_ps[:], ones_row[0:1, 0:128], bv_sb[:], False, True)
            # strided copy of 3 head-blocks into v3i (+ ones col at 64);
            # on DVE, NOT ACT: the in-order ACT queue would stall every
            # attention exp behind a straggling v-copy
            dst = v3i[:].rearrange("p (h k n) -> p h k n", h=HP, k=KB)
            nc.vector.tensor_copy(
                dst[:, :, tt, 0:64],
                v_ps[:, 0:192].rearrange("p (h n) -> p h n", h=HP))

        # ---------- attention ----------
        # 16 k-blocks in groups of 2 (one 2-bank PSUM tile per group)
        G2 = [(2 * g, 2 * g + 1) for g in range(8)]

        def epilogue(h, qt, o_ps, proj=False, tail=False):
            # o_ps: [128 q, 4*65] -- per q-block 128: 64 head-dims + denom col
            for qb in range(4):
                tt = qt * 4 + qb
                cs = slice(qb * 65, qb * 65 + 64)
                rec = tp1.tile([128, 1], F32, tag="rec")
                nc.vector.reciprocal(rec[:], o_ps[:, qb * 65 + 64:qb * 65 + 65])
                o_n = tp1.tile([128, 64], BF16, tag="o_n")
                nc.vector.tensor_scalar_mul(o_n[:], o_ps[:, cs], rec[:])
                tr_ps = fps.tile([64, 128], BF16, tag="flex")
                nc.tensor.transpose(tr_ps[:], o_n[:], ident_sb[:])
                if h < 2:
                    dst = oall_a[h * 64:(h + 1) * 64,
                                 tt * 128:(tt + 1) * 128]
                else:
                    dst = oall_b[:, tt * 128:(tt + 1) * 128]
                nc.vector.tensor_copy(dst, tr_ps[:])
                if proj:
                    proj_tt(tt, on_act=tail)

        def smm(spool, h, kbs, qs):
            s_ps = spool.tile([128, 1024], F32, tag="s")
            for j, kb in enumerate(kbs):
                mm(s_ps[:, j * 512:(j + 1) * 512],
                   kT(h)[:, kb * 128:(kb + 1) * 128], qT(h)[:, qs], True, True)
            return s_ps

        def pexp_of(s_ps):
            px = pe.tile([128, 1024], BF16, tag="pexp")
            nc.scalar.activation(px[:], s_ps[:], AF.Exp, bias=0.0, scale=0.125)
            return px

        def omm(o_ps, h, kbs, px):
            # o_ps was zeroed by memset; start=True would wipe the whole 2KB
            # psum zero-region, clobbering sibling q-blocks' accumulators.
            for j, kb in enumerate(kbs):
                for qb in range(4):
                    mm(o_ps[:, qb * 65:(qb + 1) * 65],
                       px[:, j * 512 + qb * 128:j * 512 + (qb + 1) * 128],
                       v3i[:, (h * KB + kb) * 65:(h * KB + kb) * 65 + 65],
                       False, False)

        # ---------- partial projection (one 128-token tile) ----------
        def proj_tt(tt, on_act=False):
            po = tp.tile([128, C], BF16, tag="po")
            for half in range(2):
                cs = slice(half * 384, (half + 1) * 384)
                p_ps = fps.tile([128, 384], F32, tag="flex")
                mm(p_ps[:], oall_a[:, tt * 128:(tt + 1) * 128],
                   wp0_sb[:, cs], True, False)
                mm(p_ps[:], oall_b[:, tt * 128:(tt + 1) * 128],
                   wp1_sb[:, cs], False, True)
                if on_act:  # tail: ACT is idle once the last exps drain
                    nc.scalar.activation(po[:, cs], p_ps[:],
                                         AF.Copy, bias=0.0, scale=1.0)
                else:
                    nc.vector.tensor_copy(po[:, cs], p_ps[:])
            nc.sync.dma_start(out[tt * 128:(tt + 1) * 128, :], po[:])

        # epilogues are software-pipelined: each qt's epilogue is emitted
        # after group 1 of the NEXT qt, so the next qt's S-matmuls are not
        # queued behind the epilogue's DVE->PE transpose chain.
        pending_epi = []

        def drain_epi():
            while pending_epi:
                epilogue(*pending_epi.pop(0))

        # S-matmuls run one group ahead of pexp/omm in the PE queue, so the
        # next group's S is computed while ACT works and ACT is never starved
        # behind an omm burst at the PE queue head. The in-flight group
        # carries across phase boundaries and is drained at the very end.
        pipe = []  # [(h, s_ps, kbs, o_ps)]

        def drain_pipe():
            while pipe:
                ph, ps, pk, po = pipe.pop(0)
                px = pexp_of(ps)
                omm(po, ph, pk, px)

        def attn_single(h, extra=None, proj=False, tail=False):
            for qt in range(NT):
                qs = slice(qt * 512, (qt + 1) * 512)
                o_ps = (oA if qt % 2 == 0 else oB).tile([128, 260], F32, tag="o")
                # zero via a tiny PE matmul: start=True wipes the psum
                # zero-region; keeps the zeroing off the busy DVE queue
                mm(o_ps[:], ident_sb[0:1, :], zrow[:], True, True)
                for g, kbs in enumerate(G2):
                    # prep at g==6: the 2-3 buffered pexp groups on the ACT
                    # queue absorb the prep matmul burst in the PE queue
                    if g == 6 and extra is not None:
                        extra(qt)
                    s_ps = smm(sA if g % 2 == 0 else sB, h, kbs, qs)
                    drain_pipe()
                    pipe.append((h, s_ps, kbs, o_ps))
                    if g == 2:
                        drain_epi()
                pending_epi.append((h, qt, o_ps, proj, tail and qt == NT - 1))

        def prep(h):
            def extra(qt):
                if qt == 0:
                    qkv_passA(h, 0)
                    qkv_passA(h, 1)
                elif qt == 1:
                    qkv_passA(h, 2)
                    qkv_passA(h, 3)
                elif qt == 2:
                    qkv_finish(h)
            return extra

        nc.gpsimd.memset(sv[:], 0.0)
        # warm up the PE p-state ramp during the initial DMA wait: ~9us of
        # junk matmuls so the real qkv matmuls start at full clock
        warm = oB.tile([128, 260], F32, tag="o")
        for _ in range(16):
            mm(warm[:], zrow[0:1, 0:128], zrow[0:1, :], True, True)
        for t in range(NT):
            smp = qkv_passA(0, t, ssb_on_act=True)
            qkv_finish0_lnexp(t, smp)
            for tt in range(4 * t, 4 * t + 4):
                vphase_tt(tt)
            finish_tile(0, t)
        attn_single(0, extra=prep(1))
        attn_single(1, extra=prep(2))
        attn_single(2, proj=True, tail=True)
        drain_pipe()
        drain_epi()

    if split_waits:
        _split_waits(nc)
    return nc


def _split_waits(nc):
    """This walrus build lowers at most one sync-wait per instruction (the
    matmul LDW struct rejects 2+). Move excess waits onto NoOps inserted
    just before, on the same engine queue — queues are in-order, so the
    constraint is preserved exactly."""
    k = 0
    for fn in nc.m.functions:
        for bb in fn.blocks:
            il = bb.instructions
            idx = 0
            while idx < len(il):
                inst = il[idx]
                si = inst.sync_info
                eng = getattr(inst, "engine", None)
                if (si is not None and len(si.on_wait) > 1
                        and eng is not None
                        and str(eng) != "EngineType.Unassigned"):
                    waits = list(si.on_wait)
                    inst.sync_info = mybir.SyncInfo(
                        on_wait=[waits[-1]], on_update=list(si.on_update))
                    for w in waits[:-1]:
                        nop = mybir.InstNoOp(
                            name=f"I-waitnop-{k}", engine=eng, ins=[], outs=[],
                            sync_info=mybir.SyncInfo(on_wait=[w], on_update=[]))
                        k += 1
                        il.insert(idx, nop)
                        idx += 1
                idx += 1


def _prep_core_inputs(core, x, rope_cos, rope_sin, qkv_kernel, qkv_bias,
                      proj_kernel, proj_bias, q_norm_w, k_norm_w):
    b = core // 4
    heads = [3 * (core % 4) + i for i in range(HP)]

    wq = qkv_kernel.reshape(C, 3, H, HD)
    bq = qkv_bias.reshape(3, H, HD)

    xT = np.ascontiguousarray(x[b].T).astype(BF)

    wqk = np.empty((C, HP * 128), np.float32)
    bqk = np.empty((1, HP * 128), np.float32)
    for i, h in enumerate(heads):
        wqk[:, i * 128:i * 128 + 64] = wq[:, 0, h, PERM]
        wqk[:, i * 128 + 64:(i + 1) * 128] = wq[:, 1, h, PERM]
        bqk[0, i * 128:i * 128 + 64] = bq[0, h, PERM]
        bqk[0, i * 128 + 64:(i + 1) * 128] = bq[1, h, PERM]

    wv = np.zeros((C, 192), np.float32)
    bv = np.zeros((1, 192), np.float32)
    for i, h in enumerate(heads):
        wv[:, i * 64:(i + 1) * 64] = wq[:, 2, h, :]
        bv[0, i * 64:(i + 1) * 64] = bq[2, h, :]

    cosT = rope_cos.T  # (HD, N)
    sinT = rope_sin.T
    cos2w = np.empty((128, N), np.float32)
    sinSw = np.empty((128, N), np.float32)
    cos2w[0:64] = cosT[PERM] * q_norm_w[PERM][:, None]
    cos2w[64:128] = cosT[PERM] * k_norm_w[PERM][:, None]
    sinSw[0:64] = SIGN[:, None] * sinT[PERM] * q_norm_w[PERM][:, None]
    sinSw[64:128] = SIGN[:, None] * sinT[PERM] * k_norm_w[PERM][:, None]

    onesd = np.ones((128, 512), np.float32)
    onespd = np.zeros((128, 2), np.float32)
    onespd[0:64, 0] = 1.0    # col0: ones on q rows
    onespd[64:128, 1] = 1.0  # col1: ones on k rows
    vones = np.ones((128, HP * KB), np.float32)

    sel4 = np.zeros((128, 512), np.float32)
    for t in range(NT):
        sel4[32 * t, t * 128:t * 128 + 64] = 1.0
        sel4[32 * t + 1, t * 128 + 64:(t + 1) * 128] = 1.0
    ident = np.eye(128, dtype=np.float32)

    rows = np.concatenate([np.arange(h * HD, (h + 1) * HD) for h in heads])
    wp = proj_kernel[rows, :].astype(BF)

    return {"xT": xT, "wqk": wqk.astype(BF), "wv": wv.astype(BF),
            "bqk": bqk.astype(BF), "bv": bv.astype(BF),
            "cos2w": cos2w.astype(BF), "sinSw": sinSw.astype(BF),
            "sel4": sel4.astype(BF), "ident": ident.astype(BF),
            "wp": wp, "onesd": onesd.astype(BF), "onespd": onespd.astype(BF),
            "vones": vones.astype(BF)}


def kernel(x, rope_cos, rope_sin, qkv_kernel, qkv_bias, proj_kernel,
           proj_bias, q_norm_w, k_norm_w, _trace=False):
    args = [np.asarray(a, dtype=np.float32) for a in
            (x, rope_cos, rope_sin, qkv_kernel, qkv_bias, proj_kernel,
             proj_bias, q_norm_w, k_norm_w)]
    zb = (not np.any(args[4])) and True
    in_maps = [_prep_core_inputs(c, *args) for c in range(NCORES)]

    key = ("nc", zb)
    if key not in _NC_CACHE:
        _NC_CACHE[key] = build_nc(zero_bias=zb)
    nc = _NC_CACHE[key]

    res = run_bass_kernel_spmd(nc, in_maps, core_ids=list(range(NCORES)),
                               trace=_trace)
    parts = [np.asarray(res.results[c]["out"], dtype=np.float32)
             for c in range(NCORES)]
    out = np.empty((B, N, C), np.float32)
    pb = np.asarray(proj_bias, dtype=np.float32)
    for b in range(B):
        out[b] = parts[4 * b] + parts[4 * b + 1] + parts[4 * b + 2] + parts[4 * b + 3] + pb
    if _trace:
        kernel.last_results = res
    return out


# revision 17
# speedup vs baseline: 1.5602x; 1.0353x over previous
"""Multi-head attention (RMSNorm-QK + RoPE + softmax + proj) on 8 Trainium2 cores.

Sharding: core c handles batch b = c//4 and heads [3*(c%4), 3*(c%4)+3).
Each core computes qkv for its heads, flash-style attention, and a partial
projection over its heads' channels; the host sums the 4 partials per batch.

Layout tricks (bf16 data path, fp32 PSUM accumulation):
 - q^T/k^T layout [head_dim, tokens]; head-dim rows permuted so the RoPE
   half-swap is an intra-quadrant stream_shuffle.
 - RMS-norm: sum(q^2) via ones-pair matmul; rsqrt = exp(-0.5*ln(x)) so the
   whole kernel uses one ACT table set (natural_log_exp_and_others).
 - rsqrt scales broadcast across partitions on the (idle) GPSIMD engine.
 - softmax without max-subtraction (logits bounded by RMS norm); denominators
   via an appended ones-column in the PV matmul; 1/denom on DVE reciprocal.
 - projection partials DMA'd straight from PSUM to DRAM.
"""
import sys

for _p in ("/opt/trn_rl_repo", "/opt/trn_rl_repo/concourse"):
    if _p not in sys.path:
        sys.path.insert(0, _p)

import numpy as np
import ml_dtypes
from contextlib import ExitStack

import concourse.bass as bass
import concourse.tile as tile
import concourse.mybir as mybir
from concourse.bass_utils import run_bass_kernel_spmd

F32 = mybir.dt.float32
BF16 = mybir.dt.bfloat16
AF = mybir.ActivationFunctionType
BF = ml_dtypes.bfloat16

B, N, C = 2, 2048, 768
H, HD = 12, 64
HP = 3            # heads per core
NCORES = 8
CCH = C // 128    # 6 contraction chunks
NT = N // 512     # 4 token tiles of 512
KB = N // 128     # 16 k-blocks of 128
EPS = 1e-6

SWAP_MASK = [(i + 16) % 32 for i in range(32)]
# head-dim permutation: pair-exchange (d <-> d+32) becomes intra-quadrant
PERM = np.concatenate([np.arange(0, 16), np.arange(32, 48),
                       np.arange(16, 32), np.arange(48, 64)])
SIGN = np.where(PERM < 32, -1.0, 1.0).astype(np.float32)

_NC_CACHE = {}


def build_nc(split_waits=True, zero_bias=True):
    nc = bass.Bass(target_bir_lowering=True)
    xT = nc.declare_dram_parameter("xT", [C, N], BF16, isOutput=False)
    wqk = nc.declare_dram_parameter("wqk", [C, HP * 128], BF16, isOutput=False)
    wv = nc.declare_dram_parameter("wv", [C, 192], BF16, isOutput=False)
    bqk = nc.declare_dram_parameter("bqk", [1, HP * 128], BF16, isOutput=False)
    bv = nc.declare_dram_parameter("bv", [1, 192], BF16, isOutput=False)
    cos2w = nc.declare_dram_parameter("cos2w", [128, N], BF16, isOutput=False)
    sinSw = nc.declare_dram_parameter("sinSw", [128, N], BF16, isOutput=False)
    sel4 = nc.declare_dram_parameter("sel4", [128, 512], BF16, isOutput=False)
    ident = nc.declare_dram_parameter("ident", [128, 128], BF16, isOutput=False)
    wp = nc.declare_dram_parameter("wp", [HP * HD, C], BF16, isOutput=False)
    onesd = nc.declare_dram_parameter("onesd", [128, 512], BF16, isOutput=False)
    onespd = nc.declare_dram_parameter("onespd", [128, 2], BF16, isOutput=False)
    vones = nc.declare_dram_parameter("vones", [128, HP * KB], BF16, isOutput=False)
    out = nc.declare_dram_parameter("out", [N, C], BF16, isOutput=True)

    with tile.TileContext(nc) as tc, ExitStack() as ctx:
        sb = ctx.enter_context(tc.tile_pool(name="sb", bufs=1))
        tp = ctx.enter_context(tc.tile_pool(name="tp", bufs=4))
        pe = ctx.enter_context(tc.tile_pool(name="pe", bufs=5))   # pexp
        tp1 = ctx.enter_context(tc.tile_pool(name="tp1", bufs=4))
        fps = ctx.enter_context(tc.tile_pool(name="fps", bufs=2, space="PSUM"))
        sA = ctx.enter_context(tc.tile_pool(name="sA", bufs=1, space="PSUM"))
        sB = ctx.enter_context(tc.tile_pool(name="sB", bufs=1, space="PSUM"))
        oA = ctx.enter_context(tc.tile_pool(name="oA", bufs=1, space="PSUM"))
        oB = ctx.enter_context(tc.tile_pool(name="oB", bufs=1, space="PSUM"))

        # ---------- prologue: loads + consts ----------
        # Consolidated DMAs (one descriptor set each) to dodge per-DMA queue
        # overheads; SP carries x halves, ACT carries weights/tables, Pool
        # only runs memsets so the lead-in rope muls aren't queued behind DMA.
        xall = sb.tile([128, CCH * N], BF16, tag="xall")
        xs = [xall[:, c * N:(c + 1) * N] for c in range(CCH)]
        wqk_all = sb.tile([128, CCH * HP * 128], BF16, tag="wqk_all")
        wqk_sb = [wqk_all[:, c * HP * 128:(c + 1) * HP * 128]
                  for c in range(CCH)]
        wv_all = sb.tile([128, CCH * 192], BF16, tag="wv_all")
        wv_sb = [wv_all[:, c * 192:(c + 1) * 192] for c in range(CCH)]
        cos_sb = sb.tile([128, N], BF16, tag="cos")
        sin_sb = sb.tile([128, N], BF16, tag="sin")
        onesp = sb.tile([128, 2], BF16, tag="onesp")

        xsrc = xT[:, :].rearrange("(c p) n -> p c n", c=CCH)
        xdst = xall[:].rearrange("p (c n) -> p c n", c=CCH)
        nc.gpsimd.dma_start(
            wqk_all[:].rearrange("p (c n) -> p c n", c=CCH),
            wqk[:, :].rearrange("(c p) n -> p c n", c=CCH))
        zrow = sb.tile([1, 260], BF16, tag="zrow")
        nc.gpsimd.memset(zrow[:], 0.0)
        nc.sync.dma_start(xdst[:, :, 0:512], xsrc[:, :, 0:512])
        nc.scalar.dma_start(cos_sb[:, 0:1024], cos2w[:, 0:1024])
        nc.scalar.dma_start(
            wv_all[:].rearrange("p (c n) -> p c n", c=CCH),
            wv[:, :].rearrange("(c p) n -> p c n", c=CCH))
        nc.scalar.dma_start(cos_sb[:, 1024:2048], cos2w[:, 1024:2048])
        nc.sync.dma_start(sin_sb[:, 0:1024], sinSw[:, 0:1024])
        nc.sync.dma_start(xdst[:, :, 512:1024], xsrc[:, :, 512:1024])
        nc.sync.dma_start(onesp[:], onespd[:, :])
        nc.sync.dma_start(xdst[:, :, 1024:1536], xsrc[:, :, 1024:1536])
        nc.sync.dma_start(sin_sb[:, 1024:2048], sinSw[:, 1024:2048])
        nc.sync.dma_start(xdst[:, :, 1536:2048], xsrc[:, :, 1536:2048])

        eps_t = sb.tile([128, 1], F32, tag="eps")
        nc.gpsimd.memset(eps_t[:], EPS)
        v3i = sb.tile([128, HP * KB * 65], BF16, tag="v3i")  # [v_h(kb) | 1] blocks
        nc.gpsimd.memset(
            v3i[:].rearrange("p (b n) -> p b n", n=65)[:, :, 64:65], 1.0)

        sel_sb = sb.tile([128, 512], BF16, tag="sel")
        nc.sync.dma_start(sel_sb[:], sel4[:, :])
        ident_sb = sb.tile([128, 128], BF16, tag="ident")
        nc.sync.dma_start(ident_sb[:], ident[:, :])
        bqk_sb = sb.tile([1, HP * 128], BF16, tag="bqk")
        bv_sb = sb.tile([1, 192], BF16, tag="bv")
        ones_row = sb.tile([1, 512], BF16, tag="ones_row")
        if not zero_bias:
            nc.sync.dma_start(bqk_sb[:], bqk[:, :])
            nc.sync.dma_start(bv_sb[:], bv[:, :])
            nc.sync.dma_start(ones_row[:], onesd[0:1, :])
        wp0_sb = sb.tile([128, C], BF16, tag="wp0")
        nc.scalar.dma_start(wp0_sb[:], wp[0:128, :])
        wp1_sb = sb.tile([64, C], BF16, tag="wp1")
        nc.scalar.dma_start(wp1_sb[:], wp[128:192, :])

        # qT/kT packed by head pairs so S-matmul operands share a base partition
        q12 = sb.tile([128, N], BF16, tag="q12")   # qT(0) rows 0:64, qT(1) rows 64:128
        k12 = sb.tile([128, N], BF16, tag="k12")
        q3 = sb.tile([64, N], BF16, tag="q3")
        k3 = sb.tile([64, N], BF16, tag="k3")

        def qT(h):
            return (q12[0:64], q12[64:128], q3[:])[h]

        def kT(h):
            return (k12[0:64], k12[64:128], k3[:])[h]

        oall_a = sb.tile([128, N], BF16, tag="oall_a")   # heads 0,1 O^T
        oall_b = sb.tile([64, N], BF16, tag="oall_b")    # head 2 O^T
        t4_all = sb.tile([128, N], BF16, tag="t4_all")
        s_sb = sb.tile([128, 512], F32, tag="s_sb")
        nc.gpsimd.memset(s_sb[:], 1.0)
        lnv = sb.tile([128, 512], F32, tag="lnv")
        sv = sb.tile([128, 512], BF16, tag="sv")

        def mm(out_ap, lhsT, rhs, start, stop):
            nc.tensor.matmul(out_ap, lhsT, rhs,
                             start=start, stop=stop, skip_group_check=True)

        # ---------- qkv for head h ----------
        def qkv_passA(h, t, ssb_on_act=False):
            ts = slice(t * 512, (t + 1) * 512)
            qk_ps = fps.tile([128, 512], F32, tag="flex")
            for c in range(CCH):
                mm(qk_ps[:], wqk_sb[c][:, h * 128:(h + 1) * 128],
                   xs[c][:, ts], c == 0, zero_bias and c == CCH - 1)
            if not zero_bias:
                mm(qk_ps[:], bqk_sb[:, h * 128:(h + 1) * 128], ones_row[:],
                   False, True)
            t2 = tp.tile([128, 512], F32, tag="t2")
            nc.vector.stream_shuffle(t2[:], qk_ps[:], SWAP_MASK)
            t1 = tp1.tile([128, 512], BF16, tag="t1")
            nc.vector.tensor_mul(t1[:], qk_ps[:], cos_sb[:, ts])
            sq = tp.tile([128, 512], BF16, tag="sq")
            nc.gpsimd.tensor_mul(sq[:], t2[:], t2[:])
            t3 = tp1.tile([128, 512], BF16, tag="t3")
            nc.gpsimd.tensor_mul(t3[:], t2[:], sin_sb[:, ts])
            sm_ps = fps.tile([2, 512], F32, tag="flex")
            mm(sm_ps[:], onesp[:], sq[:], True, True)
            nc.vector.tensor_copy(s_sb[32 * t:32 * t + 2, :], sm_ps[:])
            nc.vector.tensor_add(t4_all[:, ts], t1[:], t3[:])
            return sm_ps

        def finish_tile(h, t):
            ts = slice(t * 512, (t + 1) * 512)
            sqk_ps = fps.tile([128, 512], F32, tag="flex")
            mm(sqk_ps[:], sel_sb[:, t * 128:(t + 1) * 128], sv[:],
               True, True)
            nc.vector.tensor_mul(qT(h)[:, ts], t4_all[0:64, ts],
                                 sqk_ps[0:64, :])
            nc.vector.tensor_mul(kT(h)[:, ts], t4_all[64:128, ts],
                                 sqk_ps[64:128, :])

        def qkv_finish(h):
            nc.scalar.activation(lnv[:], s_sb[:], AF.Ln,
                                 bias=eps_t[:], scale=1.0 / HD)
            nc.scalar.activation(sv[:], lnv[:], AF.Exp, bias=0.0, scale=-0.5)
            for t in range(NT):
                finish_tile(h, t)

        def qkv_finish0_lnexp(t, sm_ps):
            # head 0 (lead-in): per-tile Ln/Exp so kT(0) completes early
            rows = slice(32 * t, 32 * t + 2)
            nc.scalar.activation(lnv[rows, :], s_sb[rows, :], AF.Ln,
                                 bias=eps_t[rows, :], scale=1.0 / HD)
            nc.scalar.activation(sv[rows, :], lnv[rows, :], AF.Exp,
                                 bias=0.0, scale=-0.5)

        def qkv(h):
            for t in range(NT):
                qkv_passA(h, t)
            qkv_finish(h)

        # ---------- v for all heads ----------
        def vphase_tt(tt):
            v_ps = fps.tile([128, 192], F32, tag="flex")
            for c in range(CCH):
                mm(v_ps[:], xs[c][:, tt * 128:(tt + 1) * 128], wv_sb[c][:],
                   c == 0, zero_bias and c == CCH - 1)
            if not zero_bias:
                mm(v_ps[:], ones_row[0:1, 0:128], bv_sb[:], False, True)
            # strided copy of 3 head-blocks into v3i (+ ones col at 64);
            # on DVE, NOT ACT: the in-order ACT queue would stall every
            # attention exp behind a straggling v-copy
            dst = v3i[:].rearrange("p (h k n) -> p h k n", h=HP, k=KB)
            nc.vector.tensor_copy(
                dst[:, :, tt, 0:64],
                v_ps[:, 0:192].rearrange("p (h n) -> p h n", h=HP))

        # ---------- attention ----------
        # 16 k-blocks in groups of 2 (one 2-bank PSUM tile per group)
        G2 = [(2 * g, 2 * g + 1) for g in range(8)]

        def epilogue(h, qt, o_ps, proj=False, tail=False):
            # o_ps: [128 q, 4*65] -- per q-block 128: 64 head-dims + denom col
            for qb in range(4):
                tt = qt * 4 + qb
                cs = slice(qb * 65, qb * 65 + 64)
                rec = tp1.tile([128, 1], F32, tag="rec")
                nc.vector.reciprocal(rec[:], o_ps[:, qb * 65 + 64:qb * 65 + 65])
                o_n = tp1.tile([128, 64], BF16, tag="o_n")
                nc.vector.tensor_scalar_mul(o_n[:], o_ps[:, cs], rec[:])
                tr_ps = fps.tile([64, 128], BF16, tag="flex")
                nc.tensor.transpose(tr_ps[:], o_n[:], ident_sb[:])
                if h < 2:
                    dst = oall_a[h * 64:(h + 1) * 64,
                                 tt * 128:(tt + 1) * 128]
                else:
                    dst = oall_b[:, tt * 128:(tt + 1) * 128]
                nc.vector.tensor_copy(dst, tr_ps[:])
                if proj:
                    proj_tt(tt, on_act=tail)

        def smm(spool, h, kbs, qs):
            s_ps = spool.tile([128, 1024], F32, tag="s")
            for j, kb in enumerate(kbs):
                mm(s_ps[:, j * 512:(j + 1) * 512],
                   kT(h)[:, kb * 128:(kb + 1) * 128], qT(h)[:, qs], True, True)
            return s_ps

        def pexp_of(s_ps):
            px = pe.tile([128, 1024], BF16, tag="pexp")
            nc.scalar.activation(px[:], s_ps[:], AF.Exp, bias=0.0, scale=0.125)
            return px

        def omm(o_ps, h, kbs, px):
            # o_ps was zeroed by memset; start=True would wipe the whole 2KB
            # psum zero-region, clobbering sibling q-blocks' accumulators.
            for j, kb in enumerate(kbs):
                for qb in range(4):
                    mm(o_ps[:, qb * 65:(qb + 1) * 65],
                       px[:, j * 512 + qb * 128:j * 512 + (qb + 1) * 128],
                       v3i[:, (h * KB + kb) * 65:(h * KB + kb) * 65 + 65],
                       False, False)

        # ---------- partial projection (one 128-token tile) ----------
        def proj_tt(tt, on_act=False):
            po = tp.tile([128, C], BF16, tag="po")
            for half in range(2):
                cs = slice(half * 384, (half + 1) * 384)
                p_ps = fps.tile([128, 384], F32, tag="flex")
                mm(p_ps[:], oall_a[:, tt * 128:(tt + 1) * 128],
                   wp0_sb[:, cs], True, False)
                mm(p_ps[:], oall_b[:, tt * 128:(tt + 1) * 128],
                   wp1_sb[:, cs], False, True)
                if on_act:  # tail: ACT is idle once the last exps drain
                    nc.scalar.activation(po[:, cs], p_ps[:],
                                         AF.Copy, bias=0.0, scale=1.0)
                else:
                    nc.vector.tensor_copy(po[:, cs], p_ps[:])
            nc.sync.dma_start(out[tt * 128:(tt + 1) * 128, :], po[:])

        # epilogues are software-pipelined: each qt's epilogue is emitted
        # after group 1 of the NEXT qt, so the next qt's S-matmuls are not
        # queued behind the epilogue's DVE->PE transpose chain.
        pending_epi = []

        def drain_epi():
            while pending_epi:
                epilogue(*pending_epi.pop(0))

        # S-matmuls run one group ahead of pexp/omm in the PE queue, so the
        # next group's S is computed while ACT works and ACT is never starved
        # behind an omm burst at the PE queue head. The in-flight group
        # carries across phase boundaries and is drained at the very end.
        pipe = []  # [(h, s_ps, kbs, o_ps)]

        def drain_pipe():
            while pipe:
                ph, ps, pk, po = pipe.pop(0)
                px = pexp_of(ps)
                omm(po, ph, pk, px)

        def attn_single(h, extra=None, proj=False, tail=False):
            for qt in range(NT):
                qs = slice(qt * 512, (qt + 1) * 512)
                o_ps = (oA if qt % 2 == 0 else oB).tile([128, 260], F32, tag="o")
                # zero via a tiny PE matmul: start=True wipes the psum
                # zero-region; keeps the zeroing off the busy DVE queue
                mm(o_ps[:], ident_sb[0:1, :], zrow[:], True, True)
                for g, kbs in enumerate(G2):
                    # prep at g==6: the 2-3 buffered pexp groups on the ACT
                    # queue absorb the prep matmul burst in the PE queue
                    if g == 6 and extra is not None:
                        extra(qt)
                    s_ps = smm(sA if g % 2 == 0 else sB, h, kbs, qs)
                    drain_pipe()
                    pipe.append((h, s_ps, kbs, o_ps))
                    if g == 2:
                        drain_epi()
                pending_epi.append((h, qt, o_ps, proj, tail and qt == NT - 1))

        def prep(h):
            def extra(qt):
                if qt == 0:
                    qkv_passA(h, 0)
                    qkv_passA(h, 1)
                elif qt == 1:
                    qkv_passA(h, 2)
                    qkv_passA(h, 3)
                elif qt == 2:
                    qkv_finish(h)
            return extra

        nc.gpsimd.memset(sv[:], 0.0)
        # warm up the PE p-state ramp during the initial DMA wait: ~9us of
        # junk matmuls so the real qkv matmuls start at full clock
        warm = oB.tile([128, 260], F32, tag="o")
        for _ in range(16):
            mm(warm[:], zrow[0:1, 0:128], zrow[0:1, :], True, True)
        for t in range(NT):
            smp = qkv_passA(0, t, ssb_on_act=True)
            qkv_finish0_lnexp(t, smp)
            for tt in range(4 * t, 4 * t + 4):
                vphase_tt(tt)
            finish_tile(0, t)
        attn_single(0, extra=prep(1))
        attn_single(1, extra=prep(2))
        attn_single(2, proj=True, tail=True)
        drain_pipe()
        drain_epi()

    if split_waits:
        _split_waits(nc)
    return nc


def _split_waits(nc):
    """This walrus build lowers at most one sync-wait per instruction (the
    matmul LDW struct rejects 2+). Move excess waits onto NoOps inserted
    just before, on the same engine queue — queues are in-order, so the
    constraint is preserved exactly."""
    k = 0
    for fn in nc.m.functions:
        for bb in fn.blocks:
            il = bb.instructions
            idx = 0
            while idx < len(il):
                inst = il[idx]
                si = inst.sync_info
                eng = getattr(inst, "engine", None)
                if (si is not None and len(si.on_wait) > 1
                        and eng is not None
                        and str(eng) != "EngineType.Unassigned"):
                    waits = list(si.on_wait)
                    inst.sync_info = mybir.SyncInfo(
                        on_wait=[waits[-1]], on_update=list(si.on_update))
                    for w in waits[:-1]:
                        nop = mybir.InstNoOp(
                            name=f"I-waitnop-{k}", engine=eng, ins=[], outs=[],
                            sync_info=mybir.SyncInfo(on_wait=[w], on_update=[]))
                        k += 1
                        il.insert(idx, nop)
                        idx += 1
                idx += 1


def _prep_core_inputs(core, x, rope_cos, rope_sin, qkv_kernel, qkv_bias,
                      proj_kernel, proj_bias, q_norm_w, k_norm_w):
    b = core // 4
    heads = [3 * (core % 4) + i for i in range(HP)]

    wq = qkv_kernel.reshape(C, 3, H, HD)
    bq = qkv_bias.reshape(3, H, HD)

    xT = np.ascontiguousarray(x[b].T).astype(BF)

    wqk = np.empty((C, HP * 128), np.float32)
    bqk = np.empty((1, HP * 128), np.float32)
    for i, h in enumerate(heads):
        wqk[:, i * 128:i * 128 + 64] = wq[:, 0, h, PERM]
        wqk[:, i * 128 + 64:(i + 1) * 128] = wq[:, 1, h, PERM]
        bqk[0, i * 128:i * 128 + 64] = bq[0, h, PERM]
        bqk[0, i * 128 + 64:(i + 1) * 128] = bq[1, h, PERM]

    wv = np.zeros((C, 192), np.float32)
    bv = np.zeros((1, 192), np.float32)
    for i, h in enumerate(heads):
        wv[:, i * 64:(i + 1) * 64] = wq[:, 2, h, :]
        bv[0, i * 64:(i + 1) * 64] = bq[2, h, :]

    cosT = rope_cos.T  # (HD, N)
    sinT = rope_sin.T
    cos2w = np.empty((128, N), np.float32)
    sinSw = np.empty((128, N), np.float32)
    cos2w[0:64] = cosT[PERM] * q_norm_w[PERM][:, None]
    cos2w[64:128] = cosT[PERM] * k_norm_w[PERM][:, None]
    sinSw[0:64] = SIGN[:, None] * sinT[PERM] * q_norm_w[PERM][:, None]
    sinSw[64:128] = SIGN[:, None] * sinT[PERM] * k_norm_w[PERM][:, None]

    onesd = np.ones((128, 512), np.float32)
    onespd = np.zeros((128, 2), np.float32)
    onespd[0:64, 0] = 1.0    # col0: ones on q rows
    onespd[64:128, 1] = 1.0  # col1: ones on k rows
    vones = np.ones((128, HP * KB), np.float32)

    sel4 = np.zeros((128, 512), np.float32)
    for t in range(NT):
        sel4[32 * t, t * 128:t * 128 + 64] = 1.0
        sel4[32 * t + 1, t * 128 + 64:(t + 1) * 128] = 1.0
    ident = np.eye(128, dtype=np.float32)

    rows = np.concatenate([np.arange(h * HD, (h + 1) * HD) for h in heads])
    wp = proj_kernel[rows, :].astype(BF)

    return {"xT": xT, "wqk": wqk.astype(BF), "wv": wv.astype(BF),
            "bqk": bqk.astype(BF), "bv": bv.astype(BF),
            "cos2w": cos2w.astype(BF), "sinSw": sinSw.astype(BF),
            "sel4": sel4.astype(BF), "ident": ident.astype(BF),
            "wp": wp, "onesd": onesd.astype(BF), "onespd": onespd.astype(BF),
            "vones": vones.astype(BF)}


def kernel(x, rope_cos, rope_sin, qkv_kernel, qkv_bias, proj_kernel,
           proj_bias, q_norm_w, k_norm_w, _trace=False):
    args = [np.asarray(a, dtype=np.float32) for a in
            (x, rope_cos, rope_sin, qkv_kernel, qkv_bias, proj_kernel,
             proj_bias, q_norm_w, k_norm_w)]
    zb = (not np.any(args[4])) and True
    in_maps = [_prep_core_inputs(c, *args) for c in range(NCORES)]

    key = ("nc", zb)
    if key not in _NC_CACHE:
        _NC_CACHE[key] = build_nc(zero_bias=zb)
    nc = _NC_CACHE[key]

    res = run_bass_kernel_spmd(nc, in_maps, core_ids=list(range(NCORES)),
                               trace=_trace)
    parts = [np.asarray(res.results[c]["out"], dtype=np.float32)
             for c in range(NCORES)]
    out = np.empty((B, N, C), np.float32)
    pb = np.asarray(proj_bias, dtype=np.float32)
    for b in range(B):
        out[b] = parts[4 * b] + parts[4 * b + 1] + parts[4 * b + 2] + parts[4 * b + 3] + pb
    if _trace:
        kernel.last_results = res
    return out


# revision 18
# speedup vs baseline: 1.5720x; 1.0075x over previous
"""Multi-head attention (RMSNorm-QK + RoPE + softmax + proj) on 8 Trainium2 cores.

Sharding: core c handles batch b = c//4 and heads [3*(c%4), 3*(c%4)+3).
Each core computes qkv for its heads, flash-style attention, and a partial
projection over its heads' channels; the host sums the 4 partials per batch.

Layout tricks (bf16 data path, fp32 PSUM accumulation):
 - q^T/k^T layout [head_dim, tokens]; head-dim rows permuted so the RoPE
   half-swap is an intra-quadrant stream_shuffle.
 - RMS-norm: sum(q^2) via ones-pair matmul; rsqrt = exp(-0.5*ln(x)) so the
   whole kernel uses one ACT table set (natural_log_exp_and_others).
 - rsqrt scales broadcast across partitions on the (idle) GPSIMD engine.
 - softmax without max-subtraction (logits bounded by RMS norm); denominators
   via an appended ones-column in the PV matmul; 1/denom on DVE reciprocal.
 - projection partials DMA'd straight from PSUM to DRAM.
"""
import sys

for _p in ("/opt/trn_rl_repo", "/opt/trn_rl_repo/concourse"):
    if _p not in sys.path:
        sys.path.insert(0, _p)

import numpy as np
import ml_dtypes
from contextlib import ExitStack

import concourse.bass as bass
import concourse.tile as tile
import concourse.mybir as mybir
from concourse.bass_utils import run_bass_kernel_spmd

F32 = mybir.dt.float32
BF16 = mybir.dt.bfloat16
AF = mybir.ActivationFunctionType
BF = ml_dtypes.bfloat16

B, N, C = 2, 2048, 768
H, HD = 12, 64
HP = 3            # heads per core
NCORES = 8
CCH = C // 128    # 6 contraction chunks
NT = N // 512     # 4 token tiles of 512
KB = N // 128     # 16 k-blocks of 128
EPS = 1e-6

SWAP_MASK = [(i + 16) % 32 for i in range(32)]
# head-dim permutation: pair-exchange (d <-> d+32) becomes intra-quadrant
PERM = np.concatenate([np.arange(0, 16), np.arange(32, 48),
                       np.arange(16, 32), np.arange(48, 64)])
SIGN = np.where(PERM < 32, -1.0, 1.0).astype(np.float32)

_NC_CACHE = {}


def build_nc(split_waits=True, zero_bias=True):
    nc = bass.Bass(target_bir_lowering=True)
    xT = nc.declare_dram_parameter("xT", [C, N], BF16, isOutput=False)
    wqk = nc.declare_dram_parameter("wqk", [C, HP * 128], BF16, isOutput=False)
    wv = nc.declare_dram_parameter("wv", [C, 192], BF16, isOutput=False)
    bqk = nc.declare_dram_parameter("bqk", [1, HP * 128], BF16, isOutput=False)
    bv = nc.declare_dram_parameter("bv", [1, 192], BF16, isOutput=False)
    cos2w = nc.declare_dram_parameter("cos2w", [128, N], BF16, isOutput=False)
    sinSw = nc.declare_dram_parameter("sinSw", [128, N], BF16, isOutput=False)
    sel4 = nc.declare_dram_parameter("sel4", [128, 512], BF16, isOutput=False)
    ident = nc.declare_dram_parameter("ident", [128, 128], BF16, isOutput=False)
    wp = nc.declare_dram_parameter("wp", [HP * HD, C], BF16, isOutput=False)
    onesd = nc.declare_dram_parameter("onesd", [128, 512], BF16, isOutput=False)
    onespd = nc.declare_dram_parameter("onespd", [128, 2], BF16, isOutput=False)
    vones = nc.declare_dram_parameter("vones", [128, HP * KB], BF16, isOutput=False)
    out = nc.declare_dram_parameter("out", [N, C], BF16, isOutput=True)

    with tile.TileContext(nc) as tc, ExitStack() as ctx:
        sb = ctx.enter_context(tc.tile_pool(name="sb", bufs=1))
        tp = ctx.enter_context(tc.tile_pool(name="tp", bufs=4))
        pe = ctx.enter_context(tc.tile_pool(name="pe", bufs=5))   # pexp
        tp1 = ctx.enter_context(tc.tile_pool(name="tp1", bufs=4))
        fps = ctx.enter_context(tc.tile_pool(name="fps", bufs=2, space="PSUM"))
        sA = ctx.enter_context(tc.tile_pool(name="sA", bufs=1, space="PSUM"))
        sB = ctx.enter_context(tc.tile_pool(name="sB", bufs=1, space="PSUM"))
        oA = ctx.enter_context(tc.tile_pool(name="oA", bufs=1, space="PSUM"))
        oB = ctx.enter_context(tc.tile_pool(name="oB", bufs=1, space="PSUM"))

        # ---------- prologue: loads + consts ----------
        # Consolidated DMAs (one descriptor set each) to dodge per-DMA queue
        # overheads; SP carries x halves, ACT carries weights/tables, Pool
        # only runs memsets so the lead-in rope muls aren't queued behind DMA.
        xall = sb.tile([128, CCH * N], BF16, tag="xall")
        xs = [xall[:, c * N:(c + 1) * N] for c in range(CCH)]
        wqk_all = sb.tile([128, CCH * HP * 128], BF16, tag="wqk_all")
        wqk_sb = [wqk_all[:, c * HP * 128:(c + 1) * HP * 128]
                  for c in range(CCH)]
        wv_all = sb.tile([128, CCH * 192], BF16, tag="wv_all")
        wv_sb = [wv_all[:, c * 192:(c + 1) * 192] for c in range(CCH)]
        cos_sb = sb.tile([128, N], BF16, tag="cos")
        sin_sb = sb.tile([128, N], BF16, tag="sin")
        onesp = sb.tile([128, 2], BF16, tag="onesp")

        xsrc = xT[:, :].rearrange("(c p) n -> p c n", c=CCH)
        xdst = xall[:].rearrange("p (c n) -> p c n", c=CCH)
        nc.gpsimd.dma_start(
            wqk_all[:].rearrange("p (c n) -> p c n", c=CCH),
            wqk[:, :].rearrange("(c p) n -> p c n", c=CCH))
        zrow = sb.tile([1, 260], BF16, tag="zrow")
        nc.gpsimd.memset(zrow[:], 0.0)
        nc.sync.dma_start(xdst[:, :, 0:512], xsrc[:, :, 0:512])
        nc.scalar.dma_start(cos_sb[:, 0:1024], cos2w[:, 0:1024])
        nc.scalar.dma_start(
            wv_all[:].rearrange("p (c n) -> p c n", c=CCH),
            wv[:, :].rearrange("(c p) n -> p c n", c=CCH))
        nc.scalar.dma_start(cos_sb[:, 1024:2048], cos2w[:, 1024:2048])
        nc.sync.dma_start(sin_sb[:, 0:1024], sinSw[:, 0:1024])
        nc.sync.dma_start(xdst[:, :, 512:1024], xsrc[:, :, 512:1024])
        nc.sync.dma_start(onesp[:], onespd[:, :])
        nc.sync.dma_start(xdst[:, :, 1024:1536], xsrc[:, :, 1024:1536])
        nc.sync.dma_start(sin_sb[:, 1024:2048], sinSw[:, 1024:2048])
        nc.sync.dma_start(xdst[:, :, 1536:2048], xsrc[:, :, 1536:2048])

        eps_t = sb.tile([128, 1], F32, tag="eps")
        nc.gpsimd.memset(eps_t[:], EPS)
        v3i = sb.tile([128, HP * KB * 65], BF16, tag="v3i")  # [v_h(kb) | 1] blocks
        nc.gpsimd.memset(
            v3i[:].rearrange("p (b n) -> p b n", n=65)[:, :, 64:65], 1.0)

        sel_sb = sb.tile([128, 512], BF16, tag="sel")
        nc.sync.dma_start(sel_sb[:], sel4[:, :])
        ident_sb = sb.tile([128, 128], BF16, tag="ident")
        nc.sync.dma_start(ident_sb[:], ident[:, :])
        bqk_sb = sb.tile([1, HP * 128], BF16, tag="bqk")
        bv_sb = sb.tile([1, 192], BF16, tag="bv")
        ones_row = sb.tile([1, 512], BF16, tag="ones_row")
        if not zero_bias:
            nc.sync.dma_start(bqk_sb[:], bqk[:, :])
            nc.sync.dma_start(bv_sb[:], bv[:, :])
            nc.sync.dma_start(ones_row[:], onesd[0:1, :])
        wp0_sb = sb.tile([128, C], BF16, tag="wp0")
        nc.scalar.dma_start(wp0_sb[:], wp[0:128, :])
        wp1_sb = sb.tile([64, C], BF16, tag="wp1")
        nc.scalar.dma_start(wp1_sb[:], wp[128:192, :])

        # qT/kT packed by head pairs so S-matmul operands share a base partition
        q12 = sb.tile([128, N], BF16, tag="q12")   # qT(0) rows 0:64, qT(1) rows 64:128
        k12 = sb.tile([128, N], BF16, tag="k12")
        q3 = sb.tile([64, N], BF16, tag="q3")
        k3 = sb.tile([64, N], BF16, tag="k3")

        def qT(h):
            return (q12[0:64], q12[64:128], q3[:])[h]

        def kT(h):
            return (k12[0:64], k12[64:128], k3[:])[h]

        oall_a = sb.tile([128, N], BF16, tag="oall_a")   # heads 0,1 O^T
        oall_b = sb.tile([64, N], BF16, tag="oall_b")    # head 2 O^T
        t4_all = sb.tile([128, N], BF16, tag="t4_all")
        s_sb = sb.tile([128, 512], F32, tag="s_sb")
        nc.gpsimd.memset(s_sb[:], 1.0)
        lnv = sb.tile([128, 512], F32, tag="lnv")
        sv = sb.tile([128, 512], BF16, tag="sv")

        def mm(out_ap, lhsT, rhs, start, stop):
            nc.tensor.matmul(out_ap, lhsT, rhs,
                             start=start, stop=stop, skip_group_check=True)

        # ---------- qkv for head h ----------
        def qkv_passA(h, t, ssb_on_act=False):
            ts = slice(t * 512, (t + 1) * 512)
            qk_ps = fps.tile([128, 512], F32, tag="flex")
            for c in range(CCH):
                mm(qk_ps[:], wqk_sb[c][:, h * 128:(h + 1) * 128],
                   xs[c][:, ts], c == 0, zero_bias and c == CCH - 1)
            if not zero_bias:
                mm(qk_ps[:], bqk_sb[:, h * 128:(h + 1) * 128], ones_row[:],
                   False, True)
            t2 = tp.tile([128, 512], F32, tag="t2")
            nc.vector.stream_shuffle(t2[:], qk_ps[:], SWAP_MASK)
            t1 = tp1.tile([128, 512], BF16, tag="t1")
            nc.vector.tensor_mul(t1[:], qk_ps[:], cos_sb[:, ts])
            sq = tp.tile([128, 512], BF16, tag="sq")
            nc.gpsimd.tensor_mul(sq[:], t2[:], t2[:])
            t3 = tp1.tile([128, 512], BF16, tag="t3")
            nc.gpsimd.tensor_mul(t3[:], t2[:], sin_sb[:, ts])
            sm_ps = fps.tile([2, 512], F32, tag="flex")
            mm(sm_ps[:], onesp[:], sq[:], True, True)
            nc.vector.tensor_copy(s_sb[32 * t:32 * t + 2, :], sm_ps[:])
            nc.vector.tensor_add(t4_all[:, ts], t1[:], t3[:])
            return sm_ps

        def finish_tile(h, t):
            ts = slice(t * 512, (t + 1) * 512)
            sqk_ps = fps.tile([128, 512], F32, tag="flex")
            mm(sqk_ps[:], sel_sb[:, t * 128:(t + 1) * 128], sv[:],
               True, True)
            nc.vector.tensor_mul(qT(h)[:, ts], t4_all[0:64, ts],
                                 sqk_ps[0:64, :])
            nc.vector.tensor_mul(kT(h)[:, ts], t4_all[64:128, ts],
                                 sqk_ps[64:128, :])

        def qkv_finish(h):
            nc.scalar.activation(lnv[:], s_sb[:], AF.Ln,
                                 bias=eps_t[:], scale=1.0 / HD)
            nc.scalar.activation(sv[:], lnv[:], AF.Exp, bias=0.0, scale=-0.5)
            for t in range(NT):
                finish_tile(h, t)

        def qkv_finish0_lnexp(t, sm_ps):
            # head 0 (lead-in): per-tile Ln/Exp so kT(0) completes early
            rows = slice(32 * t, 32 * t + 2)
            nc.scalar.activation(lnv[rows, :], s_sb[rows, :], AF.Ln,
                                 bias=eps_t[rows, :], scale=1.0 / HD)
            nc.scalar.activation(sv[rows, :], lnv[rows, :], AF.Exp,
                                 bias=0.0, scale=-0.5)

        def qkv(h):
            for t in range(NT):
                qkv_passA(h, t)
            qkv_finish(h)

        # ---------- v for all heads ----------
        def vphase_tt(tt):
            v_ps = fps.tile([128, 192], F32, tag="flex")
            for c in range(CCH):
                mm(v_ps[:], xs[c][:, tt * 128:(tt + 1) * 128], wv_sb[c][:],
                   c == 0, zero_bias and c == CCH - 1)
            if not zero_bias:
                mm(v_ps[:], ones_row[0:1, 0:128], bv_sb[:], False, True)
            # strided copy of 3 head-blocks into v3i (+ ones col at 64);
            # on DVE, NOT ACT: the in-order ACT queue would stall every
            # attention exp behind a straggling v-copy
            dst = v3i[:].rearrange("p (h k n) -> p h k n", h=HP, k=KB)
            nc.vector.tensor_copy(
                dst[:, :, tt, 0:64],
                v_ps[:, 0:192].rearrange("p (h n) -> p h n", h=HP))

        # ---------- attention ----------
        # 16 k-blocks in groups of 2 (one 2-bank PSUM tile per group)
        G2 = [(2 * g, 2 * g + 1) for g in range(8)]

        def epilogue(h, qt, o_ps, proj=False, tail=False):
            # o_ps: [128 q, 4*65] -- per q-block 128: 64 head-dims + denom col
            for qb in range(4):
                tt = qt * 4 + qb
                cs = slice(qb * 65, qb * 65 + 64)
                rec = tp1.tile([128, 1], F32, tag="rec")
                nc.vector.reciprocal(rec[:], o_ps[:, qb * 65 + 64:qb * 65 + 65])
                o_n = tp1.tile([128, 64], BF16, tag="o_n")
                nc.vector.tensor_scalar_mul(o_n[:], o_ps[:, cs], rec[:])
                tr_ps = fps.tile([64, 128], BF16, tag="flex")
                nc.tensor.transpose(tr_ps[:], o_n[:], ident_sb[:])
                if h < 2:
                    dst = oall_a[h * 64:(h + 1) * 64,
                                 tt * 128:(tt + 1) * 128]
                else:
                    dst = oall_b[:, tt * 128:(tt + 1) * 128]
                nc.vector.tensor_copy(dst, tr_ps[:])
                if proj:
                    proj_tt(tt, on_act=tail)

        def smm(spool, h, kbs, qs):
            s_ps = spool.tile([128, 1024], F32, tag="s")
            with tc.high_priority(offset=37):
                for j, kb in enumerate(kbs):
                    mm(s_ps[:, j * 512:(j + 1) * 512],
                       kT(h)[:, kb * 128:(kb + 1) * 128], qT(h)[:, qs],
                       True, True)
            return s_ps

        def pexp_of(s_ps):
            px = pe.tile([128, 1024], BF16, tag="pexp")
            nc.scalar.activation(px[:], s_ps[:], AF.Exp, bias=0.0, scale=0.125)
            return px

        def omm(o_ps, h, kbs, px):
            # o_ps was zeroed by memset; start=True would wipe the whole 2KB
            # psum zero-region, clobbering sibling q-blocks' accumulators.
            for j, kb in enumerate(kbs):
                for qb in range(4):
                    mm(o_ps[:, qb * 65:(qb + 1) * 65],
                       px[:, j * 512 + qb * 128:j * 512 + (qb + 1) * 128],
                       v3i[:, (h * KB + kb) * 65:(h * KB + kb) * 65 + 65],
                       False, False)

        # ---------- partial projection (one 128-token tile) ----------
        def proj_tt(tt, on_act=False):
            po = tp.tile([128, C], BF16, tag="po")
            for half in range(2):
                cs = slice(half * 384, (half + 1) * 384)
                p_ps = fps.tile([128, 384], F32, tag="flex")
                mm(p_ps[:], oall_a[:, tt * 128:(tt + 1) * 128],
                   wp0_sb[:, cs], True, False)
                mm(p_ps[:], oall_b[:, tt * 128:(tt + 1) * 128],
                   wp1_sb[:, cs], False, True)
                if on_act:  # tail: ACT is idle once the last exps drain
                    nc.scalar.activation(po[:, cs], p_ps[:],
                                         AF.Copy, bias=0.0, scale=1.0)
                else:
                    nc.vector.tensor_copy(po[:, cs], p_ps[:])
            nc.sync.dma_start(out[tt * 128:(tt + 1) * 128, :], po[:])

        # epilogues are software-pipelined: each qt's epilogue is emitted
        # after group 1 of the NEXT qt, so the next qt's S-matmuls are not
        # queued behind the epilogue's DVE->PE transpose chain.
        pending_epi = []

        def drain_epi():
            while pending_epi:
                epilogue(*pending_epi.pop(0))

        # S-matmuls run one group ahead of pexp/omm in the PE queue, so the
        # next group's S is computed while ACT works and ACT is never starved
        # behind an omm burst at the PE queue head. The in-flight group
        # carries across phase boundaries and is drained at the very end.
        pipe = []  # [(h, s_ps, kbs, o_ps)]

        def drain_pipe():
            while pipe:
                ph, ps, pk, po = pipe.pop(0)
                px = pexp_of(ps)
                omm(po, ph, pk, px)

        def attn_single(h, extra=None, proj=False, tail=False):
            for qt in range(NT):
                qs = slice(qt * 512, (qt + 1) * 512)
                o_ps = (oA if qt % 2 == 0 else oB).tile([128, 260], F32, tag="o")
                # zero via a tiny PE matmul: start=True wipes the psum
                # zero-region; keeps the zeroing off the busy DVE queue
                mm(o_ps[:], ident_sb[0:1, :], zrow[:], True, True)
                for g, kbs in enumerate(G2):
                    # prep at g==6: the 2-3 buffered pexp groups on the ACT
                    # queue absorb the prep matmul burst in the PE queue
                    if g == 6 and extra is not None:
                        extra(qt)
                    s_ps = smm(sA if g % 2 == 0 else sB, h, kbs, qs)
                    drain_pipe()
                    pipe.append((h, s_ps, kbs, o_ps))
                    if g == 2:
                        drain_epi()
                pending_epi.append((h, qt, o_ps, proj, tail and qt == NT - 1))

        def prep(h):
            def extra(qt):
                if qt == 0:
                    qkv_passA(h, 0)
                    qkv_passA(h, 1)
                elif qt == 1:
                    qkv_passA(h, 2)
                    qkv_passA(h, 3)
                elif qt == 2:
                    qkv_finish(h)
            return extra

        nc.gpsimd.memset(sv[:], 0.0)
        # warm up the PE p-state ramp during the initial DMA wait: ~9us of
        # junk matmuls so the real qkv matmuls start at full clock
        warm = oB.tile([128, 260], F32, tag="o")
        for _ in range(12):
            mm(warm[:], zrow[0:1, 0:128], zrow[0:1, :], True, True)
        for t in range(NT):
            smp = qkv_passA(0, t, ssb_on_act=True)
            qkv_finish0_lnexp(t, smp)
            for tt in range(4 * t, 4 * t + 4):
                vphase_tt(tt)
            finish_tile(0, t)
        attn_single(0, extra=prep(1))
        attn_single(1, extra=prep(2))
        attn_single(2, proj=True, tail=True)
        drain_pipe()
        drain_epi()

    if split_waits:
        _split_waits(nc)
    return nc


def _split_waits(nc):
    """This walrus build lowers at most one sync-wait per instruction (the
    matmul LDW struct rejects 2+). Move excess waits onto NoOps inserted
    just before, on the same engine queue — queues are in-order, so the
    constraint is preserved exactly."""
    k = 0
    for fn in nc.m.functions:
        for bb in fn.blocks:
            il = bb.instructions
            idx = 0
            while idx < len(il):
                inst = il[idx]
                si = inst.sync_info
                eng = getattr(inst, "engine", None)
                if (si is not None and len(si.on_wait) > 1
                        and eng is not None
                        and str(eng) != "EngineType.Unassigned"):
                    waits = list(si.on_wait)
                    inst.sync_info = mybir.SyncInfo(
                        on_wait=[waits[-1]], on_update=list(si.on_update))
                    for w in waits[:-1]:
                        nop = mybir.InstNoOp(
                            name=f"I-waitnop-{k}", engine=eng, ins=[], outs=[],
                            sync_info=mybir.SyncInfo(on_wait=[w], on_update=[]))
                        k += 1
                        il.insert(idx, nop)
                        idx += 1
                idx += 1


def _prep_core_inputs(core, x, rope_cos, rope_sin, qkv_kernel, qkv_bias,
                      proj_kernel, proj_bias, q_norm_w, k_norm_w):
    b = core // 4
    heads = [3 * (core % 4) + i for i in range(HP)]

    wq = qkv_kernel.reshape(C, 3, H, HD)
    bq = qkv_bias.reshape(3, H, HD)

    xT = np.ascontiguousarray(x[b].T).astype(BF)

    wqk = np.empty((C, HP * 128), np.float32)
    bqk = np.empty((1, HP * 128), np.float32)
    for i, h in enumerate(heads):
        wqk[:, i * 128:i * 128 + 64] = wq[:, 0, h, PERM]
        wqk[:, i * 128 + 64:(i + 1) * 128] = wq[:, 1, h, PERM]
        bqk[0, i * 128:i * 128 + 64] = bq[0, h, PERM]
        bqk[0, i * 128 + 64:(i + 1) * 128] = bq[1, h, PERM]

    wv = np.zeros((C, 192), np.float32)
    bv = np.zeros((1, 192), np.float32)
    for i, h in enumerate(heads):
        wv[:, i * 64:(i + 1) * 64] = wq[:, 2, h, :]
        bv[0, i * 64:(i + 1) * 64] = bq[2, h, :]

    cosT = rope_cos.T  # (HD, N)
    sinT = rope_sin.T
    cos2w = np.empty((128, N), np.float32)
    sinSw = np.empty((128, N), np.float32)
    cos2w[0:64] = cosT[PERM] * q_norm_w[PERM][:, None]
    cos2w[64:128] = cosT[PERM] * k_norm_w[PERM][:, None]
    sinSw[0:64] = SIGN[:, None] * sinT[PERM] * q_norm_w[PERM][:, None]
    sinSw[64:128] = SIGN[:, None] * sinT[PERM] * k_norm_w[PERM][:, None]

    onesd = np.ones((128, 512), np.float32)
    onespd = np.zeros((128, 2), np.float32)
    onespd[0:64, 0] = 1.0    # col0: ones on q rows
    onespd[64:128, 1] = 1.0  # col1: ones on k rows
    vones = np.ones((128, HP * KB), np.float32)

    sel4 = np.zeros((128, 512), np.float32)
    for t in range(NT):
        sel4[32 * t, t * 128:t * 128 + 64] = 1.0
        sel4[32 * t + 1, t * 128 + 64:(t + 1) * 128] = 1.0
    ident = np.eye(128, dtype=np.float32)

    rows = np.concatenate([np.arange(h * HD, (h + 1) * HD) for h in heads])
    wp = proj_kernel[rows, :].astype(BF)

    return {"xT": xT, "wqk": wqk.astype(BF), "wv": wv.astype(BF),
            "bqk": bqk.astype(BF), "bv": bv.astype(BF),
            "cos2w": cos2w.astype(BF), "sinSw": sinSw.astype(BF),
            "sel4": sel4.astype(BF), "ident": ident.astype(BF),
            "wp": wp, "onesd": onesd.astype(BF), "onespd": onespd.astype(BF),
            "vones": vones.astype(BF)}


def kernel(x, rope_cos, rope_sin, qkv_kernel, qkv_bias, proj_kernel,
           proj_bias, q_norm_w, k_norm_w, _trace=False):
    args = [np.asarray(a, dtype=np.float32) for a in
            (x, rope_cos, rope_sin, qkv_kernel, qkv_bias, proj_kernel,
             proj_bias, q_norm_w, k_norm_w)]
    zb = (not np.any(args[4])) and True
    in_maps = [_prep_core_inputs(c, *args) for c in range(NCORES)]

    key = ("nc", zb)
    if key not in _NC_CACHE:
        _NC_CACHE[key] = build_nc(zero_bias=zb)
    nc = _NC_CACHE[key]

    res = run_bass_kernel_spmd(nc, in_maps, core_ids=list(range(NCORES)),
                               trace=_trace)
    parts = [np.asarray(res.results[c]["out"], dtype=np.float32)
             for c in range(NCORES)]
    out = np.empty((B, N, C), np.float32)
    pb = np.asarray(proj_bias, dtype=np.float32)
    for b in range(B):
        out[b] = parts[4 * b] + parts[4 * b + 1] + parts[4 * b + 2] + parts[4 * b + 3] + pb
    if _trace:
        kernel.last_results = res
    return out


# revision 19
# speedup vs baseline: 1.5730x; 1.0007x over previous
"""Multi-head attention (RMSNorm-QK + RoPE + softmax + proj) on 8 Trainium2 cores.

Sharding: core c handles batch b = c//4 and heads [3*(c%4), 3*(c%4)+3).
Each core computes qkv for its heads, flash-style attention, and a partial
projection over its heads' channels; the host sums the 4 partials per batch.

Layout tricks (bf16 data path, fp32 PSUM accumulation):
 - q^T/k^T layout [head_dim, tokens]; head-dim rows permuted so the RoPE
   half-swap is an intra-quadrant stream_shuffle.
 - RMS-norm: sum(q^2) via ones-pair matmul; rsqrt = exp(-0.5*ln(x)) so the
   whole kernel uses one ACT table set (natural_log_exp_and_others).
 - rsqrt scales broadcast across partitions on the (idle) GPSIMD engine.
 - softmax without max-subtraction (logits bounded by RMS norm); denominators
   via an appended ones-column in the PV matmul; 1/denom on DVE reciprocal.
 - projection partials DMA'd straight from PSUM to DRAM.
"""
import sys

for _p in ("/opt/trn_rl_repo", "/opt/trn_rl_repo/concourse"):
    if _p not in sys.path:
        sys.path.insert(0, _p)

import numpy as np
import ml_dtypes
from contextlib import ExitStack

import concourse.bass as bass
import concourse.tile as tile
import concourse.mybir as mybir
from concourse.bass_utils import run_bass_kernel_spmd

F32 = mybir.dt.float32
BF16 = mybir.dt.bfloat16
AF = mybir.ActivationFunctionType
BF = ml_dtypes.bfloat16

B, N, C = 2, 2048, 768
H, HD = 12, 64
HP = 3            # heads per core
NCORES = 8
CCH = C // 128    # 6 contraction chunks
NT = N // 512     # 4 token tiles of 512
KB = N // 128     # 16 k-blocks of 128
EPS = 1e-6

SWAP_MASK = [(i + 16) % 32 for i in range(32)]
# head-dim permutation: pair-exchange (d <-> d+32) becomes intra-quadrant
PERM = np.concatenate([np.arange(0, 16), np.arange(32, 48),
                       np.arange(16, 32), np.arange(48, 64)])
SIGN = np.where(PERM < 32, -1.0, 1.0).astype(np.float32)

_NC_CACHE = {}


def build_nc(split_waits=True, zero_bias=True):
    nc = bass.Bass(target_bir_lowering=True)
    xT = nc.declare_dram_parameter("xT", [C, N], BF16, isOutput=False)
    wqk = nc.declare_dram_parameter("wqk", [C, HP * 128], BF16, isOutput=False)
    wv = nc.declare_dram_parameter("wv", [C, 192], BF16, isOutput=False)
    bqk = nc.declare_dram_parameter("bqk", [1, HP * 128], BF16, isOutput=False)
    bv = nc.declare_dram_parameter("bv", [1, 192], BF16, isOutput=False)
    cos2w = nc.declare_dram_parameter("cos2w", [128, N], BF16, isOutput=False)
    sinSw = nc.declare_dram_parameter("sinSw", [128, N], BF16, isOutput=False)
    sel4 = nc.declare_dram_parameter("sel4", [128, 512], BF16, isOutput=False)
    ident = nc.declare_dram_parameter("ident", [128, 128], BF16, isOutput=False)
    wp = nc.declare_dram_parameter("wp", [HP * HD, C], BF16, isOutput=False)
    onesd = nc.declare_dram_parameter("onesd", [128, 512], BF16, isOutput=False)
    onespd = nc.declare_dram_parameter("onespd", [128, 2], BF16, isOutput=False)
    vones = nc.declare_dram_parameter("vones", [128, HP * KB], BF16, isOutput=False)
    out = nc.declare_dram_parameter("out", [N, C], BF16, isOutput=True)

    with tile.TileContext(nc) as tc, ExitStack() as ctx:
        sb = ctx.enter_context(tc.tile_pool(name="sb", bufs=1))
        tp = ctx.enter_context(tc.tile_pool(name="tp", bufs=4))
        pe = ctx.enter_context(tc.tile_pool(name="pe", bufs=5))   # pexp
        tp1 = ctx.enter_context(tc.tile_pool(name="tp1", bufs=4))
        fps = ctx.enter_context(tc.tile_pool(name="fps", bufs=2, space="PSUM"))
        sA = ctx.enter_context(tc.tile_pool(name="sA", bufs=1, space="PSUM"))
        sB = ctx.enter_context(tc.tile_pool(name="sB", bufs=1, space="PSUM"))
        oA = ctx.enter_context(tc.tile_pool(name="oA", bufs=1, space="PSUM"))
        oB = ctx.enter_context(tc.tile_pool(name="oB", bufs=1, space="PSUM"))

        # ---------- prologue: loads + consts ----------
        # Consolidated DMAs (one descriptor set each) to dodge per-DMA queue
        # overheads; SP carries x halves, ACT carries weights/tables, Pool
        # only runs memsets so the lead-in rope muls aren't queued behind DMA.
        xall = sb.tile([128, CCH * N], BF16, tag="xall")
        xs = [xall[:, c * N:(c + 1) * N] for c in range(CCH)]
        wqk_all = sb.tile([128, CCH * HP * 128], BF16, tag="wqk_all")
        wqk_sb = [wqk_all[:, c * HP * 128:(c + 1) * HP * 128]
                  for c in range(CCH)]
        wv_all = sb.tile([128, CCH * 192], BF16, tag="wv_all")
        wv_sb = [wv_all[:, c * 192:(c + 1) * 192] for c in range(CCH)]
        cos_sb = sb.tile([128, N], BF16, tag="cos")
        sin_sb = sb.tile([128, N], BF16, tag="sin")
        onesp = sb.tile([128, 2], BF16, tag="onesp")

        xsrc = xT[:, :].rearrange("(c p) n -> p c n", c=CCH)
        xdst = xall[:].rearrange("p (c n) -> p c n", c=CCH)
        nc.gpsimd.dma_start(
            wqk_all[:].rearrange("p (c n) -> p c n", c=CCH),
            wqk[:, :].rearrange("(c p) n -> p c n", c=CCH))
        zrow = sb.tile([1, 260], BF16, tag="zrow")
        nc.gpsimd.memset(zrow[:], 0.0)
        nc.sync.dma_start(xdst[:, :, 0:512], xsrc[:, :, 0:512])
        nc.scalar.dma_start(cos_sb[:, 0:1024], cos2w[:, 0:1024])
        nc.scalar.dma_start(
            wv_all[:].rearrange("p (c n) -> p c n", c=CCH),
            wv[:, :].rearrange("(c p) n -> p c n", c=CCH))
        nc.scalar.dma_start(cos_sb[:, 1024:2048], cos2w[:, 1024:2048])
        nc.sync.dma_start(sin_sb[:, 0:1024], sinSw[:, 0:1024])
        nc.sync.dma_start(xdst[:, :, 512:1024], xsrc[:, :, 512:1024])
        nc.sync.dma_start(onesp[:], onespd[:, :])
        nc.sync.dma_start(xdst[:, :, 1024:1536], xsrc[:, :, 1024:1536])
        nc.sync.dma_start(sin_sb[:, 1024:2048], sinSw[:, 1024:2048])
        nc.sync.dma_start(xdst[:, :, 1536:2048], xsrc[:, :, 1536:2048])

        eps_t = sb.tile([128, 1], F32, tag="eps")
        nc.gpsimd.memset(eps_t[:], EPS)
        v3i = sb.tile([128, HP * KB * 65], BF16, tag="v3i")  # [v_h(kb) | 1] blocks
        nc.gpsimd.memset(
            v3i[:].rearrange("p (b n) -> p b n", n=65)[:, :, 64:65], 1.0)

        sel_sb = sb.tile([128, 512], BF16, tag="sel")
        nc.sync.dma_start(sel_sb[:], sel4[:, :])
        ident_sb = sb.tile([128, 128], BF16, tag="ident")
        nc.sync.dma_start(ident_sb[:], ident[:, :])
        bqk_sb = sb.tile([1, HP * 128], BF16, tag="bqk")
        bv_sb = sb.tile([1, 192], BF16, tag="bv")
        ones_row = sb.tile([1, 512], BF16, tag="ones_row")
        if not zero_bias:
            nc.sync.dma_start(bqk_sb[:], bqk[:, :])
            nc.sync.dma_start(bv_sb[:], bv[:, :])
            nc.sync.dma_start(ones_row[:], onesd[0:1, :])
        wp0_sb = sb.tile([128, C], BF16, tag="wp0")
        nc.scalar.dma_start(wp0_sb[:], wp[0:128, :])
        wp1_sb = sb.tile([64, C], BF16, tag="wp1")
        nc.scalar.dma_start(wp1_sb[:], wp[128:192, :])

        # qT/kT packed by head pairs so S-matmul operands share a base partition
        q12 = sb.tile([128, N], BF16, tag="q12")   # qT(0) rows 0:64, qT(1) rows 64:128
        k12 = sb.tile([128, N], BF16, tag="k12")
        q3 = sb.tile([64, N], BF16, tag="q3")
        k3 = sb.tile([64, N], BF16, tag="k3")

        def qT(h):
            return (q12[0:64], q12[64:128], q3[:])[h]

        def kT(h):
            return (k12[0:64], k12[64:128], k3[:])[h]

        oall_a = sb.tile([128, N], BF16, tag="oall_a")   # heads 0,1 O^T
        oall_b = sb.tile([64, N], BF16, tag="oall_b")    # head 2 O^T
        t4_all = sb.tile([128, N], BF16, tag="t4_all")
        s_sb = sb.tile([128, 512], F32, tag="s_sb")
        nc.gpsimd.memset(s_sb[:], 1.0)
        lnv = sb.tile([128, 512], F32, tag="lnv")
        sv = sb.tile([128, 512], BF16, tag="sv")

        def mm(out_ap, lhsT, rhs, start, stop):
            nc.tensor.matmul(out_ap, lhsT, rhs,
                             start=start, stop=stop, skip_group_check=True)

        # ---------- qkv for head h ----------
        def qkv_passA(h, t, ssb_on_act=False):
            ts = slice(t * 512, (t + 1) * 512)
            qk_ps = fps.tile([128, 512], F32, tag="flex")
            for c in range(CCH):
                mm(qk_ps[:], wqk_sb[c][:, h * 128:(h + 1) * 128],
                   xs[c][:, ts], c == 0, zero_bias and c == CCH - 1)
            if not zero_bias:
                mm(qk_ps[:], bqk_sb[:, h * 128:(h + 1) * 128], ones_row[:],
                   False, True)
            t2 = tp.tile([128, 512], F32, tag="t2")
            nc.vector.stream_shuffle(t2[:], qk_ps[:], SWAP_MASK)
            t1 = tp1.tile([128, 512], BF16, tag="t1")
            nc.vector.tensor_mul(t1[:], qk_ps[:], cos_sb[:, ts])
            sq = tp.tile([128, 512], BF16, tag="sq")
            nc.gpsimd.tensor_mul(sq[:], t2[:], t2[:])
            t3 = tp1.tile([128, 512], BF16, tag="t3")
            nc.gpsimd.tensor_mul(t3[:], t2[:], sin_sb[:, ts])
            sm_ps = fps.tile([2, 512], F32, tag="flex")
            mm(sm_ps[:], onesp[:], sq[:], True, True)
            nc.vector.tensor_copy(s_sb[32 * t:32 * t + 2, :], sm_ps[:])
            nc.vector.tensor_add(t4_all[:, ts], t1[:], t3[:])
            return sm_ps

        def finish_tile(h, t):
            ts = slice(t * 512, (t + 1) * 512)
            sqk_ps = fps.tile([128, 512], F32, tag="flex")
            mm(sqk_ps[:], sel_sb[:, t * 128:(t + 1) * 128], sv[:],
               True, True)
            nc.vector.tensor_mul(qT(h)[:, ts], t4_all[0:64, ts],
                                 sqk_ps[0:64, :])
            nc.vector.tensor_mul(kT(h)[:, ts], t4_all[64:128, ts],
                                 sqk_ps[64:128, :])

        def qkv_finish(h):
            nc.scalar.activation(lnv[:], s_sb[:], AF.Ln,
                                 bias=eps_t[:], scale=1.0 / HD)
            nc.scalar.activation(sv[:], lnv[:], AF.Exp, bias=0.0, scale=-0.5)
            for t in range(NT):
                finish_tile(h, t)

        def qkv_finish0_lnexp(t, sm_ps):
            # head 0 (lead-in): per-tile Ln/Exp so kT(0) completes early
            rows = slice(32 * t, 32 * t + 2)
            nc.scalar.activation(lnv[rows, :], s_sb[rows, :], AF.Ln,
                                 bias=eps_t[rows, :], scale=1.0 / HD)
            nc.scalar.activation(sv[rows, :], lnv[rows, :], AF.Exp,
                                 bias=0.0, scale=-0.5)

        def qkv(h):
            for t in range(NT):
                qkv_passA(h, t)
            qkv_finish(h)

        # ---------- v for all heads ----------
        def vphase_tt(tt):
            v_ps = fps.tile([128, 192], F32, tag="flex")
            for c in range(CCH):
                mm(v_ps[:], xs[c][:, tt * 128:(tt + 1) * 128], wv_sb[c][:],
                   c == 0, zero_bias and c == CCH - 1)
            if not zero_bias:
                mm(v_ps[:], ones_row[0:1, 0:128], bv_sb[:], False, True)
            # strided copy of 3 head-blocks into v3i (+ ones col at 64);
            # on DVE, NOT ACT: the in-order ACT queue would stall every
            # attention exp behind a straggling v-copy
            dst = v3i[:].rearrange("p (h k n) -> p h k n", h=HP, k=KB)
            nc.vector.tensor_copy(
                dst[:, :, tt, 0:64],
                v_ps[:, 0:192].rearrange("p (h n) -> p h n", h=HP))

        # ---------- attention ----------
        # 16 k-blocks in groups of 2 (one 2-bank PSUM tile per group)
        G2 = [(2 * g, 2 * g + 1) for g in range(8)]

        def epilogue(h, qt, o_ps, proj=False, tail=False):
            # o_ps: [128 q, 4*65] -- per q-block 128: 64 head-dims + denom col
            for qb in range(4):
                tt = qt * 4 + qb
                cs = slice(qb * 65, qb * 65 + 64)
                rec = tp1.tile([128, 1], F32, tag="rec")
                nc.vector.reciprocal(rec[:], o_ps[:, qb * 65 + 64:qb * 65 + 65])
                o_n = tp1.tile([128, 64], BF16, tag="o_n")
                nc.vector.tensor_scalar_mul(o_n[:], o_ps[:, cs], rec[:])
                tr_ps = fps.tile([64, 128], BF16, tag="flex")
                nc.tensor.transpose(tr_ps[:], o_n[:], ident_sb[:])
                if h < 2:
                    dst = oall_a[h * 64:(h + 1) * 64,
                                 tt * 128:(tt + 1) * 128]
                else:
                    dst = oall_b[:, tt * 128:(tt + 1) * 128]
                nc.vector.tensor_copy(dst, tr_ps[:])
                if proj:
                    proj_tt(tt, on_act=tail)

        def smm(spool, h, kbs, qs):
            s_ps = spool.tile([128, 1024], F32, tag="s")
            with tc.high_priority(offset=30):
                for j, kb in enumerate(kbs):
                    mm(s_ps[:, j * 512:(j + 1) * 512],
                       kT(h)[:, kb * 128:(kb + 1) * 128], qT(h)[:, qs],
                       True, True)
            return s_ps

        def pexp_of(s_ps):
            px = pe.tile([128, 1024], BF16, tag="pexp")
            nc.scalar.activation(px[:], s_ps[:], AF.Exp, bias=0.0, scale=0.125)
            return px

        def omm(o_ps, h, kbs, px):
            # o_ps was zeroed; start=True would wipe the whole 2KB psum
            # zero-region, clobbering sibling q-blocks' accumulators.
            # Deprioritized: omm consumes px and can lag; smm must not.
            with tc.high_priority(offset=-15):
                for j, kb in enumerate(kbs):
                    for qb in range(4):
                        mm(o_ps[:, qb * 65:(qb + 1) * 65],
                           px[:, j * 512 + qb * 128:j * 512 + (qb + 1) * 128],
                           v3i[:, (h * KB + kb) * 65:(h * KB + kb) * 65 + 65],
                           False, False)

        # ---------- partial projection (one 128-token tile) ----------
        def proj_tt(tt, on_act=False):
            po = tp.tile([128, C], BF16, tag="po")
            for half in range(2):
                cs = slice(half * 384, (half + 1) * 384)
                p_ps = fps.tile([128, 384], F32, tag="flex")
                mm(p_ps[:], oall_a[:, tt * 128:(tt + 1) * 128],
                   wp0_sb[:, cs], True, False)
                mm(p_ps[:], oall_b[:, tt * 128:(tt + 1) * 128],
                   wp1_sb[:, cs], False, True)
                if on_act:  # tail: ACT is idle once the last exps drain
                    nc.scalar.activation(po[:, cs], p_ps[:],
                                         AF.Copy, bias=0.0, scale=1.0)
                else:
                    nc.vector.tensor_copy(po[:, cs], p_ps[:])
            nc.sync.dma_start(out[tt * 128:(tt + 1) * 128, :], po[:])

        # epilogues are software-pipelined: each qt's epilogue is emitted
        # after group 1 of the NEXT qt, so the next qt's S-matmuls are not
        # queued behind the epilogue's DVE->PE transpose chain.
        pending_epi = []

        def drain_epi():
            while pending_epi:
                epilogue(*pending_epi.pop(0))

        # S-matmuls run one group ahead of pexp/omm in the PE queue, so the
        # next group's S is computed while ACT works and ACT is never starved
        # behind an omm burst at the PE queue head. The in-flight group
        # carries across phase boundaries and is drained at the very end.
        pipe = []  # [(h, s_ps, kbs, o_ps)]

        def drain_pipe():
            while pipe:
                ph, ps, pk, po = pipe.pop(0)
                px = pexp_of(ps)
                omm(po, ph, pk, px)

        def attn_single(h, extra=None, proj=False, tail=False):
            for qt in range(NT):
                qs = slice(qt * 512, (qt + 1) * 512)
                o_ps = (oA if qt % 2 == 0 else oB).tile([128, 260], F32, tag="o")
                # zero via a tiny PE matmul: start=True wipes the psum
                # zero-region; keeps the zeroing off the busy DVE queue
                mm(o_ps[:], ident_sb[0:1, :], zrow[:], True, True)
                for g, kbs in enumerate(G2):
                    # prep at g==6: the 2-3 buffered pexp groups on the ACT
                    # queue absorb the prep matmul burst in the PE queue
                    if g == 6 and extra is not None:
                        extra(qt)
                    s_ps = smm(sA if g % 2 == 0 else sB, h, kbs, qs)
                    drain_pipe()
                    pipe.append((h, s_ps, kbs, o_ps))
                    if g == 2:
                        drain_epi()
                pending_epi.append((h, qt, o_ps, proj, tail and qt == NT - 1))

        def prep(h):
            def extra(qt):
                if qt == 0:
                    qkv_passA(h, 0)
                    qkv_passA(h, 1)
                elif qt == 1:
                    qkv_passA(h, 2)
                    qkv_passA(h, 3)
                elif qt == 2:
                    qkv_finish(h)
            return extra

        nc.gpsimd.memset(sv[:], 0.0)
        # warm up the PE p-state ramp during the initial DMA wait: ~9us of
        # junk matmuls so the real qkv matmuls start at full clock
        warm = oB.tile([128, 260], F32, tag="o")
        for _ in range(12):
            mm(warm[:], zrow[0:1, 0:128], zrow[0:1, :], True, True)
        for t in range(NT):
            smp = qkv_passA(0, t, ssb_on_act=True)
            qkv_finish0_lnexp(t, smp)
            for tt in range(4 * t, 4 * t + 4):
                vphase_tt(tt)
            finish_tile(0, t)
        attn_single(0, extra=prep(1))
        attn_single(1, extra=prep(2))
        attn_single(2, proj=True, tail=True)
        drain_pipe()
        drain_epi()

    if split_waits:
        _split_waits(nc)
    return nc


def _split_waits(nc):
    """This walrus build lowers at most one sync-wait per instruction (the
    matmul LDW struct rejects 2+). Move excess waits onto NoOps inserted
    just before, on the same engine queue — queues are in-order, so the
    constraint is preserved exactly."""
    k = 0
    for fn in nc.m.functions:
        for bb in fn.blocks:
            il = bb.instructions
            idx = 0
            while idx < len(il):
                inst = il[idx]
                si = inst.sync_info
                eng = getattr(inst, "engine", None)
                if (si is not None and len(si.on_wait) > 1
                        and eng is not None
                        and str(eng) != "EngineType.Unassigned"):
                    waits = list(si.on_wait)
                    inst.sync_info = mybir.SyncInfo(
                        on_wait=[waits[-1]], on_update=list(si.on_update))
                    for w in waits[:-1]:
                        nop = mybir.InstNoOp(
                            name=f"I-waitnop-{k}", engine=eng, ins=[], outs=[],
                            sync_info=mybir.SyncInfo(on_wait=[w], on_update=[]))
                        k += 1
                        il.insert(idx, nop)
                        idx += 1
                idx += 1


def _prep_core_inputs(core, x, rope_cos, rope_sin, qkv_kernel, qkv_bias,
                      proj_kernel, proj_bias, q_norm_w, k_norm_w):
    b = core // 4
    heads = [3 * (core % 4) + i for i in range(HP)]

    wq = qkv_kernel.reshape(C, 3, H, HD)
    bq = qkv_bias.reshape(3, H, HD)

    xT = np.ascontiguousarray(x[b].T).astype(BF)

    wqk = np.empty((C, HP * 128), np.float32)
    bqk = np.empty((1, HP * 128), np.float32)
    for i, h in enumerate(heads):
        wqk[:, i * 128:i * 128 + 64] = wq[:, 0, h, PERM]
        wqk[:, i * 128 + 64:(i + 1) * 128] = wq[:, 1, h, PERM]
        bqk[0, i * 128:i * 128 + 64] = bq[0, h, PERM]
        bqk[0, i * 128 + 64:(i + 1) * 128] = bq[1, h, PERM]

    wv = np.zeros((C, 192), np.float32)
    bv = np.zeros((1, 192), np.float32)
    for i, h in enumerate(heads):
        wv[:, i * 64:(i + 1) * 64] = wq[:, 2, h, :]
        bv[0, i * 64:(i + 1) * 64] = bq[2, h, :]

    cosT = rope_cos.T  # (HD, N)
    sinT = rope_sin.T
    cos2w = np.empty((128, N), np.float32)
    sinSw = np.empty((128, N), np.float32)
    cos2w[0:64] = cosT[PERM] * q_norm_w[PERM][:, None]
    cos2w[64:128] = cosT[PERM] * k_norm_w[PERM][:, None]
    sinSw[0:64] = SIGN[:, None] * sinT[PERM] * q_norm_w[PERM][:, None]
    sinSw[64:128] = SIGN[:, None] * sinT[PERM] * k_norm_w[PERM][:, None]

    onesd = np.ones((128, 512), np.float32)
    onespd = np.zeros((128, 2), np.float32)
    onespd[0:64, 0] = 1.0    # col0: ones on q rows
    onespd[64:128, 1] = 1.0  # col1: ones on k rows
    vones = np.ones((128, HP * KB), np.float32)

    sel4 = np.zeros((128, 512), np.float32)
    for t in range(NT):
        sel4[32 * t, t * 128:t * 128 + 64] = 1.0
        sel4[32 * t + 1, t * 128 + 64:(t + 1) * 128] = 1.0
    ident = np.eye(128, dtype=np.float32)

    rows = np.concatenate([np.arange(h * HD, (h + 1) * HD) for h in heads])
    wp = proj_kernel[rows, :].astype(BF)

    return {"xT": xT, "wqk": wqk.astype(BF), "wv": wv.astype(BF),
            "bqk": bqk.astype(BF), "bv": bv.astype(BF),
            "cos2w": cos2w.astype(BF), "sinSw": sinSw.astype(BF),
            "sel4": sel4.astype(BF), "ident": ident.astype(BF),
            "wp": wp, "onesd": onesd.astype(BF), "onespd": onespd.astype(BF),
            "vones": vones.astype(BF)}


def kernel(x, rope_cos, rope_sin, qkv_kernel, qkv_bias, proj_kernel,
           proj_bias, q_norm_w, k_norm_w, _trace=False):
    args = [np.asarray(a, dtype=np.float32) for a in
            (x, rope_cos, rope_sin, qkv_kernel, qkv_bias, proj_kernel,
             proj_bias, q_norm_w, k_norm_w)]
    zb = (not np.any(args[4])) and True
    in_maps = [_prep_core_inputs(c, *args) for c in range(NCORES)]

    key = ("nc", zb)
    if key not in _NC_CACHE:
        _NC_CACHE[key] = build_nc(zero_bias=zb)
    nc = _NC_CACHE[key]

    res = run_bass_kernel_spmd(nc, in_maps, core_ids=list(range(NCORES)),
                               trace=_trace)
    parts = [np.asarray(res.results[c]["out"], dtype=np.float32)
             for c in range(NCORES)]
    out = np.empty((B, N, C), np.float32)
    pb = np.asarray(proj_bias, dtype=np.float32)
    for b in range(B):
        out[b] = parts[4 * b] + parts[4 * b + 1] + parts[4 * b + 2] + parts[4 * b + 3] + pb
    if _trace:
        kernel.last_results = res
    return out


# revision 20
# speedup vs baseline: 1.5732x; 1.0001x over previous
"""Multi-head attention (RMSNorm-QK + RoPE + softmax + proj) on 8 Trainium2 cores.

Sharding: core c handles batch b = c//4 and heads [3*(c%4), 3*(c%4)+3).
Each core computes qkv for its heads, flash-style attention, and a partial
projection over its heads' channels; the host sums the 4 partials per batch.

Layout tricks (bf16 data path, fp32 PSUM accumulation):
 - q^T/k^T layout [head_dim, tokens]; head-dim rows permuted so the RoPE
   half-swap is an intra-quadrant stream_shuffle.
 - RMS-norm: sum(q^2) via ones-pair matmul; rsqrt = exp(-0.5*ln(x)) so the
   whole kernel uses one ACT table set (natural_log_exp_and_others).
 - rsqrt scales broadcast across partitions on the (idle) GPSIMD engine.
 - softmax without max-subtraction (logits bounded by RMS norm); denominators
   via an appended ones-column in the PV matmul; 1/denom on DVE reciprocal.
 - projection partials DMA'd straight from PSUM to DRAM.
"""
import sys

for _p in ("/opt/trn_rl_repo", "/opt/trn_rl_repo/concourse"):
    if _p not in sys.path:
        sys.path.insert(0, _p)

import numpy as np
import ml_dtypes
from contextlib import ExitStack

import concourse.bass as bass
import concourse.tile as tile
import concourse.mybir as mybir
from concourse.bass_utils import run_bass_kernel_spmd

F32 = mybir.dt.float32
BF16 = mybir.dt.bfloat16
AF = mybir.ActivationFunctionType
BF = ml_dtypes.bfloat16

B, N, C = 2, 2048, 768
H, HD = 12, 64
HP = 3            # heads per core
NCORES = 8
CCH = C // 128    # 6 contraction chunks
NT = N // 512     # 4 token tiles of 512
KB = N // 128     # 16 k-blocks of 128
EPS = 1e-6

SWAP_MASK = [(i + 16) % 32 for i in range(32)]
# head-dim permutation: pair-exchange (d <-> d+32) becomes intra-quadrant
PERM = np.concatenate([np.arange(0, 16), np.arange(32, 48),
                       np.arange(16, 32), np.arange(48, 64)])
SIGN = np.where(PERM < 32, -1.0, 1.0).astype(np.float32)

_NC_CACHE = {}


def build_nc(split_waits=True, zero_bias=True):
    nc = bass.Bass(target_bir_lowering=True)
    xT = nc.declare_dram_parameter("xT", [C, N], BF16, isOutput=False)
    wqk = nc.declare_dram_parameter("wqk", [C, HP * 128], BF16, isOutput=False)
    wv = nc.declare_dram_parameter("wv", [C, 192], BF16, isOutput=False)
    bqk = nc.declare_dram_parameter("bqk", [1, HP * 128], BF16, isOutput=False)
    bv = nc.declare_dram_parameter("bv", [1, 192], BF16, isOutput=False)
    cos2w = nc.declare_dram_parameter("cos2w", [128, N], BF16, isOutput=False)
    sinSw = nc.declare_dram_parameter("sinSw", [128, N], BF16, isOutput=False)
    sel4 = nc.declare_dram_parameter("sel4", [128, 512], BF16, isOutput=False)
    ident = nc.declare_dram_parameter("ident", [128, 128], BF16, isOutput=False)
    wp = nc.declare_dram_parameter("wp", [HP * HD, C], BF16, isOutput=False)
    onesd = nc.declare_dram_parameter("onesd", [128, 512], BF16, isOutput=False)
    onespd = nc.declare_dram_parameter("onespd", [128, 2], BF16, isOutput=False)
    vones = nc.declare_dram_parameter("vones", [128, HP * KB], BF16, isOutput=False)
    out = nc.declare_dram_parameter("out", [N, C], BF16, isOutput=True)

    with tile.TileContext(nc) as tc, ExitStack() as ctx:
        sb = ctx.enter_context(tc.tile_pool(name="sb", bufs=1))
        tp = ctx.enter_context(tc.tile_pool(name="tp", bufs=4))
        pe = ctx.enter_context(tc.tile_pool(name="pe", bufs=5))   # pexp
        tp1 = ctx.enter_context(tc.tile_pool(name="tp1", bufs=4))
        fps = ctx.enter_context(tc.tile_pool(name="fps", bufs=2, space="PSUM"))
        sA = ctx.enter_context(tc.tile_pool(name="sA", bufs=1, space="PSUM"))
        sB = ctx.enter_context(tc.tile_pool(name="sB", bufs=1, space="PSUM"))
        oA = ctx.enter_context(tc.tile_pool(name="oA", bufs=1, space="PSUM"))
        oB = ctx.enter_context(tc.tile_pool(name="oB", bufs=1, space="PSUM"))

        # ---------- prologue: loads + consts ----------
        # Consolidated DMAs (one descriptor set each) to dodge per-DMA queue
        # overheads; SP carries x halves, ACT carries weights/tables, Pool
        # only runs memsets so the lead-in rope muls aren't queued behind DMA.
        xall = sb.tile([128, CCH * N], BF16, tag="xall")
        xs = [xall[:, c * N:(c + 1) * N] for c in range(CCH)]
        wqk_all = sb.tile([128, CCH * HP * 128], BF16, tag="wqk_all")
        wqk_sb = [wqk_all[:, c * HP * 128:(c + 1) * HP * 128]
                  for c in range(CCH)]
        wv_all = sb.tile([128, CCH * 192], BF16, tag="wv_all")
        wv_sb = [wv_all[:, c * 192:(c + 1) * 192] for c in range(CCH)]
        cos_sb = sb.tile([128, N], BF16, tag="cos")
        sin_sb = sb.tile([128, N], BF16, tag="sin")
        onesp = sb.tile([128, 2], BF16, tag="onesp")

        xsrc = xT[:, :].rearrange("(c p) n -> p c n", c=CCH)
        xdst = xall[:].rearrange("p (c n) -> p c n", c=CCH)
        nc.gpsimd.dma_start(
            wqk_all[:].rearrange("p (c n) -> p c n", c=CCH),
            wqk[:, :].rearrange("(c p) n -> p c n", c=CCH))
        zrow = sb.tile([1, 260], BF16, tag="zrow")
        nc.gpsimd.memset(zrow[:], 0.0)
        nc.sync.dma_start(xdst[:, :, 0:512], xsrc[:, :, 0:512])
        nc.scalar.dma_start(cos_sb[:, 0:1024], cos2w[:, 0:1024])
        nc.scalar.dma_start(
            wv_all[:].rearrange("p (c n) -> p c n", c=CCH),
            wv[:, :].rearrange("(c p) n -> p c n", c=CCH))
        nc.scalar.dma_start(cos_sb[:, 1024:2048], cos2w[:, 1024:2048])
        nc.sync.dma_start(sin_sb[:, 0:1024], sinSw[:, 0:1024])
        nc.sync.dma_start(xdst[:, :, 512:1024], xsrc[:, :, 512:1024])
        nc.sync.dma_start(onesp[:], onespd[:, :])
        nc.sync.dma_start(xdst[:, :, 1024:1536], xsrc[:, :, 1024:1536])
        nc.sync.dma_start(sin_sb[:, 1024:2048], sinSw[:, 1024:2048])
        nc.sync.dma_start(xdst[:, :, 1536:2048], xsrc[:, :, 1536:2048])

        eps_t = sb.tile([128, 1], F32, tag="eps")
        nc.gpsimd.memset(eps_t[:], EPS)
        v3i = sb.tile([128, HP * KB * 65], BF16, tag="v3i")  # [v_h(kb) | 1] blocks
        nc.gpsimd.memset(
            v3i[:].rearrange("p (b n) -> p b n", n=65)[:, :, 64:65], 1.0)

        sel_sb = sb.tile([128, 512], BF16, tag="sel")
        nc.sync.dma_start(sel_sb[:], sel4[:, :])
        ident_sb = sb.tile([128, 128], BF16, tag="ident")
        nc.sync.dma_start(ident_sb[:], ident[:, :])
        bqk_sb = sb.tile([1, HP * 128], BF16, tag="bqk")
        bv_sb = sb.tile([1, 192], BF16, tag="bv")
        ones_row = sb.tile([1, 512], BF16, tag="ones_row")
        if not zero_bias:
            nc.sync.dma_start(bqk_sb[:], bqk[:, :])
            nc.sync.dma_start(bv_sb[:], bv[:, :])
            nc.sync.dma_start(ones_row[:], onesd[0:1, :])
        wp0_sb = sb.tile([128, C], BF16, tag="wp0")
        nc.scalar.dma_start(wp0_sb[:], wp[0:128, :])
        wp1_sb = sb.tile([64, C], BF16, tag="wp1")
        nc.scalar.dma_start(wp1_sb[:], wp[128:192, :])

        # qT/kT packed by head pairs so S-matmul operands share a base partition
        q12 = sb.tile([128, N], BF16, tag="q12")   # qT(0) rows 0:64, qT(1) rows 64:128
        k12 = sb.tile([128, N], BF16, tag="k12")
        q3 = sb.tile([64, N], BF16, tag="q3")
        k3 = sb.tile([64, N], BF16, tag="k3")

        def qT(h):
            return (q12[0:64], q12[64:128], q3[:])[h]

        def kT(h):
            return (k12[0:64], k12[64:128], k3[:])[h]

        oall_a = sb.tile([128, N], BF16, tag="oall_a")   # heads 0,1 O^T
        oall_b = sb.tile([64, N], BF16, tag="oall_b")    # head 2 O^T
        t4_all = sb.tile([128, N], BF16, tag="t4_all")
        s_sb = sb.tile([128, 512], F32, tag="s_sb")
        nc.gpsimd.memset(s_sb[:], 1.0)
        lnv = sb.tile([128, 512], F32, tag="lnv")
        sv = sb.tile([128, 512], BF16, tag="sv")

        def mm(out_ap, lhsT, rhs, start, stop):
            nc.tensor.matmul(out_ap, lhsT, rhs,
                             start=start, stop=stop, skip_group_check=True)

        # ---------- qkv for head h ----------
        def qkv_passA(h, t, ssb_on_act=False):
            ts = slice(t * 512, (t + 1) * 512)
            qk_ps = fps.tile([128, 512], F32, tag="flex")
            for c in range(CCH):
                mm(qk_ps[:], wqk_sb[c][:, h * 128:(h + 1) * 128],
                   xs[c][:, ts], c == 0, zero_bias and c == CCH - 1)
            if not zero_bias:
                mm(qk_ps[:], bqk_sb[:, h * 128:(h + 1) * 128], ones_row[:],
                   False, True)
            t2 = tp.tile([128, 512], F32, tag="t2")
            nc.vector.stream_shuffle(t2[:], qk_ps[:], SWAP_MASK)
            t1 = tp1.tile([128, 512], BF16, tag="t1")
            nc.vector.tensor_mul(t1[:], qk_ps[:], cos_sb[:, ts])
            sq = tp.tile([128, 512], BF16, tag="sq")
            nc.gpsimd.tensor_mul(sq[:], t2[:], t2[:])
            t3 = tp1.tile([128, 512], BF16, tag="t3")
            nc.gpsimd.tensor_mul(t3[:], t2[:], sin_sb[:, ts])
            sm_ps = fps.tile([2, 512], F32, tag="flex")
            mm(sm_ps[:], onesp[:], sq[:], True, True)
            nc.vector.tensor_copy(s_sb[32 * t:32 * t + 2, :], sm_ps[:])
            nc.vector.tensor_add(t4_all[:, ts], t1[:], t3[:])
            return sm_ps

        def finish_tile(h, t):
            ts = slice(t * 512, (t + 1) * 512)
            sqk_ps = fps.tile([128, 512], F32, tag="flex")
            mm(sqk_ps[:], sel_sb[:, t * 128:(t + 1) * 128], sv[:],
               True, True)
            nc.vector.tensor_mul(qT(h)[:, ts], t4_all[0:64, ts],
                                 sqk_ps[0:64, :])
            nc.vector.tensor_mul(kT(h)[:, ts], t4_all[64:128, ts],
                                 sqk_ps[64:128, :])

        def qkv_finish(h):
            nc.scalar.activation(lnv[:], s_sb[:], AF.Ln,
                                 bias=eps_t[:], scale=1.0 / HD)
            nc.scalar.activation(sv[:], lnv[:], AF.Exp, bias=0.0, scale=-0.5)
            for t in range(NT):
                finish_tile(h, t)

        def qkv_finish0_lnexp(t, sm_ps):
            # head 0 (lead-in): per-tile Ln/Exp so kT(0) completes early
            rows = slice(32 * t, 32 * t + 2)
            nc.scalar.activation(lnv[rows, :], s_sb[rows, :], AF.Ln,
                                 bias=eps_t[rows, :], scale=1.0 / HD)
            nc.scalar.activation(sv[rows, :], lnv[rows, :], AF.Exp,
                                 bias=0.0, scale=-0.5)

        def qkv(h):
            for t in range(NT):
                qkv_passA(h, t)
            qkv_finish(h)

        # ---------- v for all heads ----------
        def vphase_tt(tt):
            v_ps = fps.tile([128, 192], F32, tag="flex")
            for c in range(CCH):
                mm(v_ps[:], xs[c][:, tt * 128:(tt + 1) * 128], wv_sb[c][:],
                   c == 0, zero_bias and c == CCH - 1)
            if not zero_bias:
                mm(v_ps[:], ones_row[0:1, 0:128], bv_sb[:], False, True)
            # strided copy of 3 head-blocks into v3i (+ ones col at 64);
            # on DVE, NOT ACT: the in-order ACT queue would stall every
            # attention exp behind a straggling v-copy
            dst = v3i[:].rearrange("p (h k n) -> p h k n", h=HP, k=KB)
            nc.vector.tensor_copy(
                dst[:, :, tt, 0:64],
                v_ps[:, 0:192].rearrange("p (h n) -> p h n", h=HP))

        # ---------- attention ----------
        # 16 k-blocks in groups of 2 (one 2-bank PSUM tile per group)
        G2 = [(2 * g, 2 * g + 1) for g in range(8)]

        def epilogue(h, qt, o_ps, proj=False, tail=False):
            # o_ps: [128 q, 4*65] -- per q-block 128: 64 head-dims + denom col
            for qb in range(4):
                tt = qt * 4 + qb
                cs = slice(qb * 65, qb * 65 + 64)
                rec = tp1.tile([128, 1], F32, tag="rec")
                nc.vector.reciprocal(rec[:], o_ps[:, qb * 65 + 64:qb * 65 + 65])
                o_n = tp1.tile([128, 64], BF16, tag="o_n")
                nc.vector.tensor_scalar_mul(o_n[:], o_ps[:, cs], rec[:])
                tr_ps = fps.tile([64, 128], BF16, tag="flex")
                nc.tensor.transpose(tr_ps[:], o_n[:], ident_sb[:])
                if h < 2:
                    dst = oall_a[h * 64:(h + 1) * 64,
                                 tt * 128:(tt + 1) * 128]
                else:
                    dst = oall_b[:, tt * 128:(tt + 1) * 128]
                nc.vector.tensor_copy(dst, tr_ps[:])
                if proj:
                    proj_tt(tt, on_act=tail)

        def smm(spool, h, kbs, qs):
            s_ps = spool.tile([128, 1024], F32, tag="s")
            with tc.high_priority(offset=31):
                for j, kb in enumerate(kbs):
                    mm(s_ps[:, j * 512:(j + 1) * 512],
                       kT(h)[:, kb * 128:(kb + 1) * 128], qT(h)[:, qs],
                       True, True)
            return s_ps

        def pexp_of(s_ps):
            px = pe.tile([128, 1024], BF16, tag="pexp")
            nc.scalar.activation(px[:], s_ps[:], AF.Exp, bias=0.0, scale=0.125)
            return px

        def omm(o_ps, h, kbs, px):
            # o_ps was zeroed; start=True would wipe the whole 2KB psum
            # zero-region, clobbering sibling q-blocks' accumulators.
            # Deprioritized: omm consumes px and can lag; smm must not.
            with tc.high_priority(offset=-15):
                for j, kb in enumerate(kbs):
                    for qb in range(4):
                        mm(o_ps[:, qb * 65:(qb + 1) * 65],
                           px[:, j * 512 + qb * 128:j * 512 + (qb + 1) * 128],
                           v3i[:, (h * KB + kb) * 65:(h * KB + kb) * 65 + 65],
                           False, False)

        # ---------- partial projection (one 128-token tile) ----------
        def proj_tt(tt, on_act=False):
            po = tp.tile([128, C], BF16, tag="po")
            for half in range(2):
                cs = slice(half * 384, (half + 1) * 384)
                p_ps = fps.tile([128, 384], F32, tag="flex")
                mm(p_ps[:], oall_a[:, tt * 128:(tt + 1) * 128],
                   wp0_sb[:, cs], True, False)
                mm(p_ps[:], oall_b[:, tt * 128:(tt + 1) * 128],
                   wp1_sb[:, cs], False, True)
                if on_act:  # tail: ACT is idle once the last exps drain
                    nc.scalar.activation(po[:, cs], p_ps[:],
                                         AF.Copy, bias=0.0, scale=1.0)
                else:
                    nc.vector.tensor_copy(po[:, cs], p_ps[:])
            nc.sync.dma_start(out[tt * 128:(tt + 1) * 128, :], po[:])

        # epilogues are software-pipelined: each qt's epilogue is emitted
        # after group 1 of the NEXT qt, so the next qt's S-matmuls are not
        # queued behind the epilogue's DVE->PE transpose chain.
        pending_epi = []

        def drain_epi():
            while pending_epi:
                epilogue(*pending_epi.pop(0))

        # S-matmuls run one group ahead of pexp/omm in the PE queue, so the
        # next group's S is computed while ACT works and ACT is never starved
        # behind an omm burst at the PE queue head. The in-flight group
        # carries across phase boundaries and is drained at the very end.
        pipe = []  # [(h, s_ps, kbs, o_ps)]

        def drain_pipe():
            while pipe:
                ph, ps, pk, po = pipe.pop(0)
                px = pexp_of(ps)
                omm(po, ph, pk, px)

        def attn_single(h, extra=None, proj=False, tail=False):
            for qt in range(NT):
                qs = slice(qt * 512, (qt + 1) * 512)
                o_ps = (oA if qt % 2 == 0 else oB).tile([128, 260], F32, tag="o")
                # zero via a tiny PE matmul: start=True wipes the psum
                # zero-region; keeps the zeroing off the busy DVE queue
                mm(o_ps[:], ident_sb[0:1, :], zrow[:], True, True)
                for g, kbs in enumerate(G2):
                    # prep at g==6: the 2-3 buffered pexp groups on the ACT
                    # queue absorb the prep matmul burst in the PE queue
                    if g == 6 and extra is not None:
                        extra(qt)
                    s_ps = smm(sA if g % 2 == 0 else sB, h, kbs, qs)
                    drain_pipe()
                    pipe.append((h, s_ps, kbs, o_ps))
                    if g == 2:
                        drain_epi()
                pending_epi.append((h, qt, o_ps, proj, tail and qt == NT - 1))

        def prep(h):
            def extra(qt):
                if qt == 0:
                    qkv_passA(h, 0)
                    qkv_passA(h, 1)
                elif qt == 1:
                    qkv_passA(h, 2)
                    qkv_passA(h, 3)
                elif qt == 2:
                    qkv_finish(h)
            return extra

        nc.gpsimd.memset(sv[:], 0.0)
        # warm up the PE p-state ramp during the initial DMA wait: ~9us of
        # junk matmuls so the real qkv matmuls start at full clock
        warm = oB.tile([128, 260], F32, tag="o")
        for _ in range(12):
            mm(warm[:], zrow[0:1, 0:128], zrow[0:1, :], True, True)
        for t in range(NT):
            smp = qkv_passA(0, t, ssb_on_act=True)
            qkv_finish0_lnexp(t, smp)
            for tt in range(4 * t, 4 * t + 4):
                vphase_tt(tt)
            finish_tile(0, t)
        attn_single(0, extra=prep(1))
        attn_single(1, extra=prep(2))
        attn_single(2, proj=True, tail=True)
        drain_pipe()
        drain_epi()

    if split_waits:
        _split_waits(nc)
    return nc


def _split_waits(nc):
    """This walrus build lowers at most one sync-wait per instruction (the
    matmul LDW struct rejects 2+). Move excess waits onto NoOps inserted
    just before, on the same engine queue — queues are in-order, so the
    constraint is preserved exactly."""
    k = 0
    for fn in nc.m.functions:
        for bb in fn.blocks:
            il = bb.instructions
            idx = 0
            while idx < len(il):
                inst = il[idx]
                si = inst.sync_info
                eng = getattr(inst, "engine", None)
                if (si is not None and len(si.on_wait) > 1
                        and eng is not None
                        and str(eng) != "EngineType.Unassigned"):
                    waits = list(si.on_wait)
                    inst.sync_info = mybir.SyncInfo(
                        on_wait=[waits[-1]], on_update=list(si.on_update))
                    for w in waits[:-1]:
                        nop = mybir.InstNoOp(
                            name=f"I-waitnop-{k}", engine=eng, ins=[], outs=[],
                            sync_info=mybir.SyncInfo(on_wait=[w], on_update=[]))
                        k += 1
                        il.insert(idx, nop)
                        idx += 1
                idx += 1


def _prep_core_inputs(core, x, rope_cos, rope_sin, qkv_kernel, qkv_bias,
                      proj_kernel, proj_bias, q_norm_w, k_norm_w):
    b = core // 4
    heads = [3 * (core % 4) + i for i in range(HP)]

    wq = qkv_kernel.reshape(C, 3, H, HD)
    bq = qkv_bias.reshape(3, H, HD)

    xT = np.ascontiguousarray(x[b].T).astype(BF)

    wqk = np.empty((C, HP * 128), np.float32)
    bqk = np.empty((1, HP * 128), np.float32)
    for i, h in enumerate(heads):
        wqk[:, i * 128:i * 128 + 64] = wq[:, 0, h, PERM]
        wqk[:, i * 128 + 64:(i + 1) * 128] = wq[:, 1, h, PERM]
        bqk[0, i * 128:i * 128 + 64] = bq[0, h, PERM]
        bqk[0, i * 128 + 64:(i + 1) * 128] = bq[1, h, PERM]

    wv = np.zeros((C, 192), np.float32)
    bv = np.zeros((1, 192), np.float32)
    for i, h in enumerate(heads):
        wv[:, i * 64:(i + 1) * 64] = wq[:, 2, h, :]
        bv[0, i * 64:(i + 1) * 64] = bq[2, h, :]

    cosT = rope_cos.T  # (HD, N)
    sinT = rope_sin.T
    cos2w = np.empty((128, N), np.float32)
    sinSw = np.empty((128, N), np.float32)
    cos2w[0:64] = cosT[PERM] * q_norm_w[PERM][:, None]
    cos2w[64:128] = cosT[PERM] * k_norm_w[PERM][:, None]
    sinSw[0:64] = SIGN[:, None] * sinT[PERM] * q_norm_w[PERM][:, None]
    sinSw[64:128] = SIGN[:, None] * sinT[PERM] * k_norm_w[PERM][:, None]

    onesd = np.ones((128, 512), np.float32)
    onespd = np.zeros((128, 2), np.float32)
    onespd[0:64, 0] = 1.0    # col0: ones on q rows
    onespd[64:128, 1] = 1.0  # col1: ones on k rows
    vones = np.ones((128, HP * KB), np.float32)

    sel4 = np.zeros((128, 512), np.float32)
    for t in range(NT):
        sel4[32 * t, t * 128:t * 128 + 64] = 1.0
        sel4[32 * t + 1, t * 128 + 64:(t + 1) * 128] = 1.0
    ident = np.eye(128, dtype=np.float32)

    rows = np.concatenate([np.arange(h * HD, (h + 1) * HD) for h in heads])
    wp = proj_kernel[rows, :].astype(BF)

    return {"xT": xT, "wqk": wqk.astype(BF), "wv": wv.astype(BF),
            "bqk": bqk.astype(BF), "bv": bv.astype(BF),
            "cos2w": cos2w.astype(BF), "sinSw": sinSw.astype(BF),
            "sel4": sel4.astype(BF), "ident": ident.astype(BF),
            "wp": wp, "onesd": onesd.astype(BF), "onespd": onespd.astype(BF),
            "vones": vones.astype(BF)}


def kernel(x, rope_cos, rope_sin, qkv_kernel, qkv_bias, proj_kernel,
           proj_bias, q_norm_w, k_norm_w, _trace=False):
    args = [np.asarray(a, dtype=np.float32) for a in
            (x, rope_cos, rope_sin, qkv_kernel, qkv_bias, proj_kernel,
             proj_bias, q_norm_w, k_norm_w)]
    zb = (not np.any(args[4])) and True
    in_maps = [_prep_core_inputs(c, *args) for c in range(NCORES)]

    key = ("nc", zb)
    if key not in _NC_CACHE:
        _NC_CACHE[key] = build_nc(zero_bias=zb)
    nc = _NC_CACHE[key]

    res = run_bass_kernel_spmd(nc, in_maps, core_ids=list(range(NCORES)),
                               trace=_trace)
    parts = [np.asarray(res.results[c]["out"], dtype=np.float32)
             for c in range(NCORES)]
    out = np.empty((B, N, C), np.float32)
    pb = np.asarray(proj_bias, dtype=np.float32)
    for b in range(B):
        out[b] = parts[4 * b] + parts[4 * b + 1] + parts[4 * b + 2] + parts[4 * b + 3] + pb
    if _trace:
        kernel.last_results = res
    return out
